# revision 1
# baseline (speedup 1.0000x reference)
"""ETSFormer forward pass on 8 Trainium2 NeuronCores (Bass/Tile).

Data-parallel over batch: 32 samples -> 8 cores x 4 samples, weights
replicated, no collectives. The reference's FFT machinery is computed
exactly without FFTs:
  - freq_attention: dense DFT matmuls + hardware top-8 (vector.max) mask
  - mhesa / level exponential smoothing: the reference FFT cross-correlation
    is exactly a first-order EMA -> hardware prefix scan (tensor_tensor_scan)
  - fourier_extrapolate: exact slice (Dirichlet kernel identity)

Precision: the top-4 frequency mask is extremely sensitive (2e-4 relative
amp noise -> 2.6e-2 output error), so every GEMM feeding a ranking (conv,
rfft both layers, irfft/mhesa/FF of layer 0) runs in fp32; post-ranking
paths (layer-1 irfft/mhesa via lgT, level, damp, output head) run fp32r.
"""
import numpy as np
from contextlib import ExitStack

import concourse.bass as bass
import concourse.bacc as bacc
import concourse.tile as tile
from concourse import mybir
from concourse.bass_utils import run_bass_kernel_spmd

F32 = mybir.dt.float32
F32R = mybir.dt.float32r
BF16 = mybir.dt.bfloat16
AF = mybir.ActivationFunctionType
ALU = mybir.AluOpType

N = 1024
D = 512
TF = 7
HEADS = 8
DH = D // HEADS
L = 2
S = 4
NCORES = 8
HOR = 96
FD = 2048
NT = N // 128   # 8
ND = D // 128   # 4
NM = FD // 128  # 16

_CACHE = {}
OMA_BCAST = True


def _dft_consts():
    if "dft" not in _CACHE:
        t = np.arange(N)
        f = np.arange(513)
        ang = 2.0 * np.pi * np.outer(t, f) / N
        cos = np.cos(ang)
        sin = np.sin(ang)
        dft = np.zeros((N, 1024), np.float64)
        dft[:, 0:512] = cos[:, 0:512]
        dft[:, 512] = cos[:, 512]
        dft[:, 513:1024] = sin[:, 1:512]
        c = np.full(513, 2.0)
        c[0] = 1.0
        c[512] = 1.0
        ib = np.zeros((1024, N), np.float64)
        ib[0:512, :] = (c[0:512, None] / N) * cos[:, 0:512].T
        ib[512, :] = (1.0 / N) * cos[:, 512]
        ib[513:1024, :] = (2.0 / N) * sin[:, 1:512].T
        _CACHE["dft"] = dft.astype(np.float32)
        _CACHE["ib"] = ib.astype(np.float32)
    return _CACHE["dft"], _CACHE["ib"]


def _sl(i, w=128):
    return slice(i * w, (i + 1) * w)


def _split_hi(x):
    import ml_dtypes
    return x.astype(ml_dtypes.bfloat16)


def _split_lo(x):
    import ml_dtypes
    hi = x.astype(ml_dtypes.bfloat16).astype(np.float32)
    return (x - hi).astype(ml_dtypes.bfloat16)


def _pack_w1(w1):
    # per-m contiguous [128, hi(4x128) | lo(4x128)] bf16 tiles: one DMA per
    # m-chunk in the FF loop instead of eight 128x128 strided patches
    hi, lo = _split_hi(w1), _split_lo(w1)
    out = np.zeros((NM, 128, 1024), hi.dtype)
    for m in range(NM):
        for kt in range(ND):
            out[m, :, 128 * kt:128 * (kt + 1)] = hi[_sl(kt), _sl(m)]
            out[m, :, 512 + 128 * kt:640 + 128 * kt] = lo[_sl(kt), _sl(m)]
    return out


def _pack_cpk(inputs):
    # cols: gpre(4) | bpre(4) | ffb1(16) | ffb2(4), each D/FD vector folded
    # into [128, k] column blocks -- one DMA instead of 24
    out = np.zeros((128, 28), np.float32)
    out[:, 0:4] = np.asarray(inputs["ff_pre_g"], np.float32).reshape(4, 128).T
    out[:, 4:8] = np.asarray(inputs["ff_pre_b"], np.float32).reshape(4, 128).T
    out[:, 8:24] = np.asarray(inputs["ff_b1"], np.float32).reshape(16, 128).T
    out[:, 24:28] = np.asarray(inputs["ff_b2"], np.float32).reshape(4, 128).T
    return out


def _pack_lcol(inputs):
    # per layer: init(4 cols) | bin(4 cols)
    out = np.zeros((L, 128, 8), np.float32)
    ini = np.asarray(inputs["mhesa_init"], np.float32).reshape(L, D)
    bi = np.asarray(inputs["mhesa_bin"], np.float32)
    for l in range(L):
        out[l, :, 0:4] = ini[l].reshape(4, 128).T
        out[l, :, 4:8] = bi[l].reshape(4, 128).T
    return out


def _build_w2d(conv_w, conv_b):
    # rows 32k+c hold conv_w[:, c, k] (32-aligned partition groups so the
    # on-device shifted copies keep legal base partitions); row 95 is the
    # bias row, paired with an all-ones row 95 of xsh on device.
    w2d = np.zeros((96, D), np.float32)
    for k in range(3):
        for c in range(TF):
            w2d[32 * k + c] = conv_w[:, c, k]
    w2d[95] = conv_b
    return w2d


def _hh(h):
    return slice(h * 512, (h + 1) * 512)


class K:
    def __init__(self):
        nc = bacc.Bacc()
        self.nc = nc
        p = nc.declare_dram_parameter
        self.d_xT = p("xT", [S * TF, N], F32, isOutput=False)
        self.d_w2d = p("w2d", [96, D], F32, isOutput=False)
        self.d_ones1 = p("ones1", [1, N], F32, isOutput=False)
        self.d_dft = p("dft", [N, 1024], F32, isOutput=False)
        self.d_dfthl = p("dfthl", [N, 2048], BF16, isOutput=False)
        self.d_ibr = p("ibr", [1024, N], F32R, isOutput=False)
        self.d_winr = p("winr", [D, D], F32R, isOutput=False)
        self.d_woutr = p("woutr", [D, D], F32R, isOutput=False)
        self.d_ibhl = p("ibhl", [1024, 2048], BF16, isOutput=False)
        self.d_winhl = p("winhl", [D, 2 * D], BF16, isOutput=False)
        self.d_wouthl = p("wouthl", [D, 2 * D], BF16, isOutput=False)
        self.d_idn = p("idn", [128, 128], F32, isOutput=False)
        self.d_e8 = p("e8", [HEADS, D], F32, isOutput=False)
        self.d_bout = p("boutr", [L, 1, D], F32, isOutput=False)
        self.d_lcolp = p("lcolp", [L, 128, 8], F32, isOutput=False)
        self.d_al8 = p("alpha8", [L, HEADS, 1], F32, isOutput=False)
        self.d_ffw1t = p("ffw1t", [NM, 128, 2 * ND * 128], BF16, isOutput=False)
        self.d_ffw2hl = p("ffw2hl", [FD, 2 * D], BF16, isOutput=False)
        self.d_cpkp = p("cpkp", [128, 28], F32, isOutput=False)
        self.d_gpost = p("gpostr", [1, D], F32, isOutput=False)
        self.d_bpost = p("bpostr", [1, D], F32, isOutput=False)
        self.d_wg = p("lvwg", [L, D, TF], F32, isOutput=False)
        self.d_wp = p("lvwp", [L, D, TF], F32, isOutput=False)
        self.d_bg = p("lvbg", [L, 1, TF], F32, isOutput=False)
        self.d_bp = p("lvbp", [L, 1, TF], F32, isOutput=False)
        self.d_alv = p("lvalpha", [L, 1, 1], F32, isOutput=False)
        self.d_damp = p("damp8", [HEADS, 1], F32, isOutput=False)
        self.d_outw = p("outw", [128, ND * TF], F32, isOutput=False)
        self.d_outb = p("outbr", [1, TF], F32, isOutput=False)
        self.d_out = p("outT", [S * TF, HOR], F32, isOutput=True)
        self.zmid = nc.dram_tensor("zmid", [S, N, D], F32)
        self.xtmid = nc.dram_tensor("xtmid", [S, TF, N], F32)

    # psum bank helper: tag-based reuse of the 8 banks
    def bank(self, i, shape=(128, 512), dtype=F32):
        tl = self.psp.tile(list(shape), dtype, tag=f"bk{i}", name=f"bk{i}")
        return tl

    def build(self):
        nc = self.nc
        with ExitStack() as ctx:
            self.tc = ctx.enter_context(tile.TileContext(nc))
            tc = self.tc
            top = ctx.enter_context(tc.tile_pool(name="top", bufs=1))

            idn = top.tile([128, 128], F32, name="idn")
            nc.sync.dma_start(idn[:], self.d_idn[:])
            idnr = top.tile([128, 128], F32R, name="idnr")
            nc.vector.tensor_copy(idnr[:], idn[:])
            self.idnr = idnr
            ones = top.tile([128, 128], F32, name="ones")
            nc.vector.memset(ones[:], 1.0)
            w2d = top.tile([96, D], F32, name="w2d")
            nc.sync.dma_start(w2d[:], self.d_w2d[:])
            outbr = top.tile([1, TF], F32, name="outbr")
            nc.sync.dma_start(outbr[:], self.d_outb[:])
            self.outbt = outbr
            # col pack: gpre(4) | bpre(4)
            cpk = top.tile([128, 28], F32, name="cpk")
            nc.sync.dma_start(cpk[:], self.d_cpkp[:])
            outw = top.tile([128, ND * TF], F32, name="outw")
            nc.sync.dma_start(outw[:], self.d_outw[:])
            eps = top.tile([128, 1], F32, name="eps")
            nc.vector.memset(eps[:], 1e-5)
            self.epst = eps
            gbt = top.tile([128, D], F32, name="gbt")
            bbt = top.tile([128, D], F32, name="bbt")
            self.gbt, self.bbt = gbt, bbt
            agg = top.tile([128, ND * HOR], F32, name="agg")
            csd = top.tile([128, ND * HOR], F32, name="csd")

            self.idn, self.ones, self.cpk = idn, ones, cpk
            self.w2dt_, self.aggt, self.csdt = w2d, agg, csd
            self.outwt = outw

            self.psp = ctx.enter_context(
                tc.tile_pool(name="ps", bufs=1, space="PSUM"))
            # both layers' constants resident; samples run L0->L1 back to
            # back so L1's DVE-heavy tail overlaps the next sample's
            # PE-heavy head, and z4 never round-trips through DRAM
            lay0p = ctx.enter_context(tc.tile_pool(name="lay0", bufs=1))
            lay1p = ctx.enter_context(tc.tile_pool(name="lay1", bufs=1))
            with tc.tile_pool(name="ini", bufs=1) as ini:
                e8 = ini.tile([HEADS, D], F32, name="e8")
                nc.sync.dma_start(e8[:], self.d_e8[:])
                self.e8t = e8
                self._damp_cs(ini, self.psp)
                lay = [self._layer_consts(0, lay0p),
                       self._layer_consts(1, lay1p)]
            wk = ctx.enter_context(tc.tile_pool(name="wk", bufs=1))
            for s in range(S):
                z4 = self._sample(0, s, lay[0], wk)
                self._sample(1, s, lay[1], wk, zin=z4)
                self._output(s, wk)

        nc.compile()
        return nc

    # ---------- dampening cumsum -> csd [128, ND*HOR] ----------
    def _damp_cs(self, ini, inips):
        nc = self.nc
        ones = self.ones
        dcol = ini.tile([HEADS, 1], F32, name="dcol")
        nc.sync.dma_start(dcol[:], self.d_damp[:])
        df = ini.tile([HEADS, 1], F32, name="dfsig")
        nc.scalar.activation(df[:], dcol[:], AF.Sigmoid)
        dfb = ini.tile([HEADS, HOR], F32, name="dfb")
        nc.scalar.activation(dfb[:], ones[0:HEADS, 0:HOR], AF.Identity,
                             scale=df[:, 0:1])
        zer = ini.tile([HEADS, HOR], F32, name="zer8")
        nc.vector.memset(zer[:], 0.0)
        dfp = ini.tile([HEADS, HOR], F32, name="dfp")
        nc.vector.tensor_tensor_scan(dfp[:], dfb[:], zer[:], 1.0,
                                     op0=ALU.mult, op1=ALU.add)
        cs8 = ini.tile([HEADS, HOR], F32, name="cs8")
        nc.vector.tensor_tensor_scan(cs8[:], ones[0:HEADS, 0:HOR], dfp[:], 0.0,
                                     op0=ALU.mult, op1=ALU.add)
        for dt in range(ND):
            pini = inips.tile([128, HOR], F32, tag=f"bk{dt}", name="pini")
            nc.tensor.matmul(pini[:], self.e8t[:, _sl(dt)], cs8[:],
                             start=True, stop=True)
            nc.scalar.copy(self.csdt[:, dt * HOR:(dt + 1) * HOR], pini[:])
        # hoisted FF post-LN gamma/beta broadcasts (layer-invariant)
        rows = ini.tile([1, 1024], F32, name="rows")
        nc.sync.dma_start(rows[0:1, 0:512], self.d_gpost[:])
        nc.sync.dma_start(rows[0:1, 512:1024], self.d_bpost[:])
        pgb = inips.tile([128, D], F32, tag="bk4", name="pgb")
        nc.tensor.matmul(pgb[:], self.ones[0:1, 0:128],
                         rows[0:1, 0:512], start=True, stop=True)
        nc.scalar.copy(self.gbt[:], pgb[:])
        pbb = inips.tile([128, D], F32, tag="bk5", name="pbb")
        nc.tensor.matmul(pbb[:], self.ones[0:1, 0:128],
                         rows[0:1, 512:1024], start=True, stop=True)
        nc.scalar.copy(self.bbt[:], pbb[:])

    # ---------- per-layer constants ----------
    def _layer_consts(self, l, layp):
        nc = self.nc
        ones = self.ones
        last = l == L - 1
        lay = {"l": l, "last": last}

        if last:
            win = [layp.tile([128, D], F32R, name=f"win{k}") for k in range(ND)]
            wout = [layp.tile([128, D], F32R, name=f"wout{k}")
                    for k in range(ND)]
            for kt in range(ND):
                nc.scalar.dma_start(win[kt][:], self.d_winr[_sl(kt), :])
                nc.scalar.dma_start(wout[kt][:], self.d_woutr[_sl(kt), :])
        else:
            # bf16 hi|lo packed (cols 0:512 hi, 512:1024 lo)
            win = [layp.tile([128, 2 * D], BF16, name=f"win{k}")
                   for k in range(ND)]
            wout = [layp.tile([128, 2 * D], BF16, name=f"wout{k}")
                    for k in range(ND)]
            for kt in range(ND):
                nc.scalar.dma_start(win[kt][:], self.d_winhl[_sl(kt), :])
                nc.scalar.dma_start(wout[kt][:], self.d_wouthl[_sl(kt), :])

        # lrows: p0 = bout[512]; p32 = bg[7] then bp at cols 16..23
        lrows = layp.tile([128, 512], F32, name="lrows")
        nc.sync.dma_start(lrows[0:1, 0:D], self.d_bout[l, :, :])
        nc.sync.dma_start(lrows[32:33, 0:TF], self.d_bg[l, :, :])
        nc.sync.dma_start(lrows[32:33, 16:16 + TF], self.d_bp[l, :, :])

        # bout broadcast [128, D] (replaces per-tile bias matmuls)
        boutb = layp.tile([128, D], F32, name="boutb")
        pbo = self.psp.tile([128, D], F32, tag="bk7", name="pbo")
        nc.tensor.matmul(pbo[:], ones[0:1, 0:128], lrows[0:1, 0:D],
                         start=True, stop=True)
        nc.scalar.copy(boutb[:], pbo[:])

        # lcol pack [128, 16]: al(4) oma(4) init(4) bi(4); plus lv cols [7,1]
        # cols 18/19: level bg/bp as [7,1] columns
        lcol = layp.tile([128, 24], F32, name="lcol")
        nc.sync.dma_start(lcol[0:TF, 18:19],
                          self.d_bg[l, :, :].rearrange("a b -> b a"))
        nc.sync.dma_start(lcol[0:TF, 19:20],
                          self.d_bp[l, :, :].rearrange("a b -> b a"))
        al8 = layp.tile([HEADS, 1], F32, tag="al8t", name="al8")
        nc.sync.dma_start(al8[:], self.d_al8[l, :, :])
        al8s = layp.tile([HEADS, 1], F32, tag="al8s", name="al8s")
        nc.scalar.activation(al8s[:], al8[:], AF.Sigmoid)
        for dt in range(ND):
            pal = self.psp.tile([128, 1], F32, tag="bk0", name="pal")
            nc.tensor.matmul(pal[:], self.e8t[:, _sl(dt)], al8s[:],
                             start=True, stop=True)
            nc.scalar.copy(lcol[:, dt:dt + 1], pal[:])
        nc.sync.dma_start(lcol[:, 8:16], self.d_lcolp[l, :, :])
        for dt in range(ND):
            nc.vector.tensor_scalar(lcol[:, 4 + dt:5 + dt], lcol[:, dt:dt + 1],
                                    -1.0, 1.0, op0=ALU.mult, op1=ALU.add)
        nc.vector.tensor_sub(lcol[:, 12:16], lcol[:, 12:16], lcol[:, 8:12])
        # col 20:24 = al*(bi-init) + (1-al)*init -- the scan-initial folded
        # into xd[0] so the scan can run with a 0.0 immediate initial
        bi = layp.tile([128, ND], F32, tag="bitmp", name="bitmp")
        nc.vector.tensor_mul(lcol[:, 20:24], lcol[:, 0:4], lcol[:, 12:16])
        nc.vector.tensor_mul(bi[:], lcol[:, 4:8], lcol[:, 8:12])
        nc.vector.tensor_add(lcol[:, 20:24], lcol[:, 20:24], bi[:])
        # level alpha
        alv = layp.tile([1, 1], F32, tag="alvt", name="alv")
        nc.sync.dma_start(alv[:], self.d_alv[l, :, :])
        alvs = layp.tile([1, 1], F32, tag="alvst", name="alvs")
        nc.scalar.activation(alvs[:], alv[:], AF.Sigmoid)
        pv = self.psp.tile([TF, 1], F32, tag="bk1", name="palv")
        nc.tensor.matmul(pv[:], ones[0:1, 0:TF], alvs[:], start=True, stop=True)
        nc.scalar.copy(lcol[0:TF, 16:17], pv[:])
        nc.vector.tensor_scalar(lcol[0:TF, 17:18], lcol[0:TF, 16:17], -1.0, 1.0,
                                op0=ALU.mult, op1=ALU.add)

        # level weights [128, TF] x4 packed [128, 2*ND*TF], as fp32r
        lwf = layp.tile([128, 2 * ND * TF], F32, tag="lwf", name="lwf")
        for kt in range(ND):
            nc.sync.dma_start(lwf[:, kt * TF:(kt + 1) * TF], self.d_wg[l, _sl(kt), :])
            nc.sync.dma_start(lwf[:, (ND + kt) * TF:(ND + kt + 1) * TF],
                              self.d_wp[l, _sl(kt), :])
        lw = layp.tile([128, 2 * ND * TF], F32R, name="lw")
        nc.vector.tensor_copy(lw[:], lwf[:])

        lay.update(win=win, wout=wout, lrows=lrows, lcol=lcol, lw=lw,
                   boutb=boutb)
        return lay

    # ---------- one sample through one layer ----------
    def _sample(self, l, s, lay, wk, zin=None):
        nc = self.nc
        ones, idn = self.ones, self.idn
        last = lay["last"]
        agg = self.aggt

        def aggsl(dt):
            return self.aggt[:, dt * HOR:(dt + 1) * HOR]

        # --- z input: conv (l0) or handed over in SBUF from l0 (l1)
        if l == 0:
            # agg is per-sample now; clear it (waits on prior _output read)
            nc.gpsimd.memset(agg[:], 0.0)
            z = [wk.tile([128, D], F32, tag=f"B1_{tt}", name=f"z{tt}")
                 for tt in range(NT)]
            # low-rank path: x is rank-7, so z = xsh^T @ w2d (rows 32k+c hold
            # the 3 shifts of the 7 channels; row 95 = ones * conv_b) and
            # DFT(z) = w2d^T @ (xsh^T @ dft) -- the DFT runs in the 96-dim
            # input space instead of the 512-dim channel space.
            xsh = wk.tile([96, N], F32, tag="xsh", name="xsh")
            xts = wk.tile([TF, N], F32, tag="xts", name="xts")
            nc.sync.dma_start(xts[:], self.d_xT[s * TF:(s + 1) * TF, :])
            nc.gpsimd.memset(xsh[:], 0.0)
            nc.gpsimd.tensor_copy(xsh[0:TF, 1:N], xts[:, 0:N - 1])
            nc.gpsimd.tensor_copy(xsh[32:32 + TF, 0:N], xts[:, 0:N])
            nc.gpsimd.tensor_copy(xsh[64:64 + TF, 0:N - 1], xts[:, 1:N])
            nc.sync.dma_start(xsh[95:96, :], self.d_ones1[:])
            xshT = [wk.tile([128, 96], F32, tag=f"xshT{tt}", name=f"xshT{tt}")
                    for tt in range(NT)]
            psF1A = self.bank(2, shape=(96, 512))
            psF1B = self.bank(3, shape=(96, 512))
            for tt in range(NT):
                pz = self.bank(tt % 2)
                nc.tensor.matmul(pz[:], xsh[:, _sl(tt)], self.w2dt_[:],
                                 start=True, stop=True)
                nc.scalar.copy(z[tt][:], pz[:])
                pxT = self.bank(6, shape=(128, 96))
                nc.tensor.transpose(pxT[:], xsh[:, _sl(tt)], idn[0:96, 0:96])
                nc.scalar.copy(xshT[tt][:], pxT[:])
                dftk = wk.tile([128, 1024], F32, tag=f"dftk{tt % 2}",
                               name="dftk")
                nc.sync.dma_start(dftk[:], self.d_dft[_sl(tt), :])
                nc.tensor.matmul(psF1A[:], xshT[tt][:], dftk[:, 0:512],
                                 start=(tt == 0), stop=(tt == NT - 1))
                nc.tensor.matmul(psF1B[:], xshT[tt][:], dftk[:, 512:1024],
                                 start=(tt == 0), stop=(tt == NT - 1))
            F1s = wk.tile([96, 1024], F32, tag="lvp", name="F1s")
            nc.scalar.copy(F1s[:, 0:512], psF1A[:])
            nc.scalar.copy(F1s[:, 512:1024], psF1B[:])
            psA = [self.bank(ct) for ct in range(ND)]
            psB = [self.bank(4 + ct) for ct in range(ND)]
            for ct in range(ND):
                nc.tensor.matmul(psA[ct][:], self.w2dt_[:, _sl(ct)],
                                 F1s[:, 0:512], start=True, stop=True)
                nc.tensor.matmul(psB[ct][:], self.w2dt_[:, _sl(ct)],
                                 F1s[:, 512:1024], start=True, stop=True)
            ibkpf0 = []
            for pf in range(2):
                ibkp = wk.tile([128, 2048], BF16, tag=f"ibk{pf % 2}",
                               name="ibk")
                nc.sync.dma_start(ibkp[:], self.d_ibhl[_sl(pf), :])
                ibkpf0.append(ibkp)
        else:
            z, zhl = zin

            # --- rfft via bf16 hi/lo 3-term split (exact to ~2^-17)
            psA = [self.bank(ct) for ct in range(ND)]
            psB = [self.bank(4 + ct) for ct in range(ND)]
            for kt in range(NT):
                dftk = wk.tile([128, 2048], BF16, tag=f"dftk{kt % 2}",
                               name="dftk")
                nc.sync.dma_start(dftk[:], self.d_dfthl[_sl(kt), :])
                st0 = kt == 0
                sp = kt == NT - 1
                for ct in range(ND):
                    zh = zhl[kt][:, _sl(ct)]
                    zl = zhl[kt][:, 512 + 128 * ct:640 + 128 * ct]
                    nc.tensor.matmul(psA[ct][:], zh, dftk[:, 0:512],
                                     start=st0, stop=False)
                    nc.tensor.matmul(psA[ct][:], zh, dftk[:, 1024:1536],
                                     start=False, stop=False)
                    nc.tensor.matmul(psB[ct][:], zh, dftk[:, 512:1024],
                                     start=st0, stop=False)
                    nc.tensor.matmul(psB[ct][:], zh, dftk[:, 1536:2048],
                                     start=False, stop=False)
                    nc.tensor.matmul(psA[ct][:], zl, dftk[:, 0:512],
                                     start=False, stop=sp)
                    nc.tensor.matmul(psB[ct][:], zl, dftk[:, 512:1024],
                                     start=False, stop=sp)
            # prefetch the first two irfft ib stripes while the mask runs
            ibkpf = []
            for pf in range(2):
                ibkp = wk.tile([128, 1024], F32R, tag=f"ibk{pf % 2}",
                               name="ibk")
                nc.sync.dma_start(ibkp[:], self.d_ibr[_sl(pf), :])
                ibkpf.append(ibkp)

        # --- top-4 mask -> filt [ND][128, 1024] ([c, f])
        filt = [wk.tile([128, 1024], F32R if last else F32,
                        tag=f"A1_{ct}", name=f"filt{ct}")
                for ct in range(ND)]
        for ct in range(ND):
            sqA = wk.tile([128, 512], F32,
                          tag="sqA" if ct % 2 == 0 else "lnscr2", name="sqA")
            nc.scalar.activation(sqA[:], psA[ct][:], AF.Square)
            sqB = wk.tile([128, 512], F32,
                          tag="w2m0" if ct % 2 == 0 else "w2m1", name="sqB")
            nc.scalar.activation(sqB[:], psB[ct][:], AF.Square)
            amp2 = wk.tile([128, 513], F32,
                           tag="amp2" if ct % 2 == 0 else "lnscr", name="amp2")
            nc.vector.tensor_add(amp2[:, 1:512], sqA[:, 1:512], sqB[:, 1:512])
            nc.scalar.copy(amp2[:, 0:1], sqA[:, 0:1])
            nc.scalar.copy(amp2[:, 512:513], sqB[:, 0:1])
            top8 = wk.tile([128, 8], F32, tag="top8", name="top8")
            nc.vector.max(top8[:], amp2[:])
            kth = top8[:, 3:4]
            nc.vector.scalar_tensor_tensor(filt[ct][:, 0:512], amp2[:, 0:512],
                                           kth, psA[ct][:],
                                           op0=ALU.is_ge, op1=ALU.mult)
            nc.vector.scalar_tensor_tensor(filt[ct][:, 513:1024], amp2[:, 1:512],
                                           kth, psB[ct][:, 1:512],
                                           op0=ALU.is_ge, op1=ALU.mult)
            nc.vector.scalar_tensor_tensor(filt[ct][:, 512:513], amp2[:, 512:513],
                                           kth, psB[ct][:, 0:1],
                                           op0=ALU.is_ge, op1=ALU.mult)

        # --- transpose filt -> filtT [f, c]; L0 splits to bf16 hi|lo
        if last:
            filtT = [wk.tile([128, 512], F32R, tag=f"B2_{ft}",
                             name=f"filtT{ft}") for ft in range(NT)]
            for ft in range(NT):
                pT = self.bank(ft % 4, dtype=F32R)
                for ct in range(ND):
                    nc.tensor.transpose(pT[:, _sl(ct)], filt[ct][:, _sl(ft)],
                                        self.idnr[:])
                if ft % 2 == 0:
                    nc.scalar.copy(filtT[ft][:], pT[:])
                else:
                    nc.vector.tensor_copy(filtT[ft][:], pT[:])
        else:
            filtT = [wk.tile([128, 1024], BF16, tag=f"B2_{ft}",
                             name=f"fthl{ft}") for ft in range(NT)]
            for ft in range(NT):
                pT = self.bank(ft % 4)
                for ct in range(ND):
                    nc.tensor.transpose(pT[:, _sl(ct)], filt[ct][:, _sl(ft)],
                                        idn[:])
                nc.scalar.copy(filtT[ft][:, 0:512], pT[:])
                nc.vector.tensor_sub(filtT[ft][:, 512:1024], pT[:],
                                     filtT[ft][:, 0:512])

        # --- irfft (ib streamed, 8 banks) -> lp, z2
        pl = [self.bank(tt) for tt in range(NT)]
        if last:
            for ft in range(NT):
                if ft < 2:
                    ibk = ibkpf[ft]
                else:
                    ibk = wk.tile([128, 1024], F32R, tag=f"ibk{ft % 2}",
                                  name="ibk")
                    nc.sync.dma_start(ibk[:], self.d_ibr[_sl(ft), :])
                for tt in range(NT):
                    nc.tensor.matmul(pl[tt][:], ibk[:, _sl(tt)], filtT[ft][:],
                                     start=(ft == 0), stop=(ft == NT - 1))
        else:
            for ft in range(NT):
                if ft < 2:
                    ibk = ibkpf0[ft]
                else:
                    ibk = wk.tile([128, 2048], BF16, tag=f"ibk{ft % 2}",
                                  name="ibk")
                    nc.sync.dma_start(ibk[:], self.d_ibhl[_sl(ft), :])
                for tt in range(NT):
                    ibh = ibk[:, _sl(tt)]
                    ibl = ibk[:, 1024 + 128 * tt:1152 + 128 * tt]
                    nc.tensor.matmul(pl[tt][:], ibh, filtT[ft][:, 0:512],
                                     start=(ft == 0), stop=False)
                    nc.tensor.matmul(pl[tt][:], ibh, filtT[ft][:, 512:1024],
                                     start=False, stop=False)
                    nc.tensor.matmul(pl[tt][:], ibl, filtT[ft][:, 0:512],
                                     start=False, stop=(ft == NT - 1))
        lp = [wk.tile([128, D], F32R, tag=f"B3_{tt}", name=f"lp{tt}")
              for tt in range(NT)]
        z2 = [wk.tile([128, D], F32, tag=f"B4_{tt}", name=f"z2_{tt}")
              for tt in range(NT)]
        for tt in range(NT):
            # z2 before lp: in l1 the lp tiles reuse z's memory (tag B3)
            nc.vector.tensor_sub(z2[tt][:], z[tt][:], pl[tt][:])
            nc.scalar.copy(lp[tt][:], pl[tt][:])

        # --- lpT [ND][128, N] (tag A2) + extrap + perT; then free
        lpT = [wk.tile([128, N], F32R, tag=f"A2_{dt}", name=f"lpT{dt}")
               for dt in range(ND)]
        for h in range(2):
            for dt in range(ND):
                pT = self.bank(dt, dtype=F32R)
                for q in range(4):
                    nc.tensor.transpose(pT[:, _sl(q)], lp[h * 4 + q][:, _sl(dt)],
                                        self.idnr[:])
                if h == 0:
                    nc.scalar.copy(lpT[dt][:, _hh(h)], pT[:])
                    nc.vector.tensor_add(aggsl(dt), aggsl(dt),
                                         lpT[dt][:, 0:HOR])
                else:
                    nc.vector.tensor_copy(lpT[dt][:, _hh(h)], pT[:])
        perT = wk.tile([TF, N], F32, tag="dftk0", name="perT")
        for h in range(2):
            pp = self.bank(2 + h)
            for kt in range(ND):
                nc.tensor.matmul(pp[0:TF, :], lay["lw"][:, (ND + kt) * TF:(ND + kt + 1) * TF],
                                 lpT[kt][:, _hh(h)],
                                 start=(kt == 0), stop=(kt == ND - 1))
            nc.scalar.copy(perT[:, _hh(h)], pp[0:TF, :])

        # --- z2T (tag A2 reuse after lpT dead); L0 packs bf16 hi|lo
        if last:
            z2T = [wk.tile([128, N], F32R, tag=f"A2_{dt}", name=f"z2T{dt}")
                   for dt in range(ND)]
            for h in range(2):
                for dt in range(ND):
                    pT = self.bank(dt)
                    for q in range(4):
                        nc.tensor.transpose(pT[:, _sl(q)],
                                            z2[h * 4 + q][:, _sl(dt)], idn[:])
                    nc.vector.tensor_copy(z2T[dt][:, _hh(h)], pT[:])
        else:
            z2T = [wk.tile([128, 2 * N], BF16, tag=f"A2_{dt}",
                           name=f"z2Thl{dt}") for dt in range(ND)]
            for h in range(2):
                for dt in range(ND):
                    pT = self.bank(dt)
                    for q in range(4):
                        nc.tensor.transpose(pT[:, _sl(q)],
                                            z2[h * 4 + q][:, _sl(dt)], idn[:])
                    nc.scalar.copy(z2T[dt][:, _hh(h)], pT[:])
                    nc.vector.tensor_sub(
                        z2T[dt][:, N + 512 * h:N + 512 * h + 512], pT[:],
                        z2T[dt][:, _hh(h)])

        # --- win GEMM -> xinT (tag A1 reuse: filt dead)
        xinT = [wk.tile([128, N], F32, tag=f"A1_{dt}", name=f"xinT{dt}")
                for dt in range(ND)]
        for h in range(2):
            for dt in range(ND):
                px = self.bank(4 + dt % 2)
                if last:
                    for kt in range(ND):
                        nc.tensor.matmul(px[:], lay["win"][kt][:, _sl(dt)],
                                         z2T[kt][:, _hh(h)],
                                         start=(kt == 0), stop=(kt == ND - 1))
                else:
                    for kt in range(ND):
                        wh = lay["win"][kt][:, _sl(dt)]
                        wl = lay["win"][kt][:, 512 + 128 * dt:640 + 128 * dt]
                        zh = z2T[kt][:, _hh(h)]
                        zl = z2T[kt][:, N + 512 * h:N + 512 * h + 512]
                        nc.tensor.matmul(px[:], wh, zh,
                                         start=(kt == 0), stop=False)
                        nc.tensor.matmul(px[:], wh, zl,
                                         start=False, stop=False)
                        nc.tensor.matmul(px[:], wl, zh,
                                         start=False, stop=(kt == ND - 1))
                # fold the per-head alpha scale into the psum->sbuf copy
                nc.scalar.activation(xinT[dt][:, _hh(h)], px[:], AF.Identity,
                                     scale=lay["lcol"][:, dt:dt + 1])

        # --- xd -> scan -> sT (tag A2 reuse: z2T dead); alternate DVE/Pool
        # by dt parity so the serial scan chain splits across two engines
        lc = lay["lcol"]
        if last:
            sT = [wk.tile([128, N], F32R, tag=f"A2_{dt}", name=f"sT{dt}")
                  for dt in range(ND)]
            sTsc = sT
        else:
            sTsc = [wk.tile([128, N], F32, tag=f"A1_{dt}", name=f"sTf{dt}")
                    for dt in range(ND)]
            sT = [wk.tile([128, 2 * N], BF16, tag=f"A2_{dt}",
                          name=f"sThl{dt}") for dt in range(ND)]
        for dt in range(ND):
            eng = nc.vector if dt % 2 == 0 else nc.gpsimd
            xd = wk.tile([128, N], F32, tag="xsh" if dt % 2 == 0 else "dftk1",
                         name="xd")
            eng.tensor_sub(xd[:, 1:N], xinT[dt][:, 1:N], xinT[dt][:, 0:N - 1])
            # xinT is pre-scaled by alpha; col 20+dt folds in the scan initial
            nc.vector.tensor_scalar_add(xd[:, 0:1], xinT[dt][:, 0:1],
                                        lc[:, 20 + dt:21 + dt])
            omab_ap = lc[:, 4 + dt:5 + dt].broadcast_to([128, N])
            nc.vector.tensor_tensor_scan(sTsc[dt][:], omab_ap, xd[:], 0.0,
                                         op0=ALU.mult, op1=ALU.add)
            if not last:
                eng.tensor_copy(sT[dt][:, 0:N], sTsc[dt][:])
                eng.tensor_sub(sT[dt][:, N:2 * N], sTsc[dt][:],
                               sT[dt][:, 0:N])

        # --- wout GEMM -> lg [t,d] (tag B2 reuse: filtT dead) (+ z3 if l0)
        lg = [wk.tile([128, D], F32R, tag=f"B2_{tt}", name=f"lg{tt}")
              for tt in range(NT)]
        for tt in range(NT):
            pg = self.bank(tt % 2)
            if last:
                for kt in range(ND):
                    nc.tensor.matmul(pg[:], sT[kt][:, _sl(tt)],
                                     lay["wout"][kt][:],
                                     start=(kt == 0), stop=(kt == ND - 1))
            else:
                for kt in range(ND):
                    sh = sT[kt][:, _sl(tt)]
                    sl_ = sT[kt][:, N + 128 * tt:N + 128 * tt + 128]
                    nc.tensor.matmul(pg[:], sh, lay["wout"][kt][:, 0:512],
                                     start=(kt == 0), stop=False)
                    nc.tensor.matmul(pg[:], sh, lay["wout"][kt][:, 512:1024],
                                     start=False, stop=False)
                    nc.tensor.matmul(pg[:], sl_, lay["wout"][kt][:, 0:512],
                                     start=False, stop=(kt == ND - 1))
            nc.vector.tensor_add(lg[tt][:], pg[:], lay["boutb"][:])
            if not last:
                # z3 overwrites z (tag B1): z dead after z2
                nc.vector.tensor_sub(z[tt][:], z2[tt][:], lg[tt][:])
        z3 = z

        # exact last-growth column for damp (avoids f32r transpose truncation,
        # which the dampening cumsum amplifies)
        lglast = wk.tile([1, D], F32, tag="sqA", name="lglast")
        nc.gpsimd.dma_start(lglast[:], lg[NT - 1][127:128, :])
        lgl4 = wk.tile([128, ND], F32, tag="top8", name="lgl4")
        pTl = self.bank(7, shape=(128, ND))
        for dt in range(ND):
            nc.tensor.matmul(pTl[:, dt:dt + 1], lglast[0:1, _sl(dt)],
                             ones[0:1, 0:1], start=True, stop=True)
        nc.scalar.copy(lgl4[:], pTl[:])

        # --- lgT via transposes (tag A1 reuse: xinT dead)
        lgT = [wk.tile([128, N], F32R, tag=f"A1_{dt}", name=f"lgT{dt}")
               for dt in range(ND)]
        for h in range(2):
            for dt in range(ND):
                pT = self.bank(2 + dt % 2, dtype=F32R)
                for q in range(4):
                    nc.tensor.transpose(pT[:, _sl(q)], lg[h * 4 + q][:, _sl(dt)],
                                        self.idnr[:])
                if h == 0:
                    nc.scalar.copy(lgT[dt][:, _hh(h)], pT[:])
                else:
                    nc.vector.tensor_copy(lgT[dt][:, _hh(h)], pT[:])
        for dt in range(ND):
            # damp: agg += lg_last * csd
            nc.vector.scalar_tensor_tensor(
                aggsl(dt), self.csdt[:, dt * HOR:(dt + 1) * HOR],
                lgl4[:, dt:dt + 1], aggsl(dt), op0=ALU.mult, op1=ALU.add)

        # --- level: grT; scans update xtmid
        grT = wk.tile([TF, N], F32, tag="grT", name="grT")
        for h in range(2):
            pgr = self.bank(6)
            for kt in range(ND):
                nc.tensor.matmul(pgr[0:TF, :], lay["lw"][:, kt * TF:(kt + 1) * TF],
                                 lgT[kt][:, _hh(h)],
                                 start=(kt == 0), stop=(kt == ND - 1))
            # fold level bg bias (lcol col 18) into the psum->sbuf copy
            nc.vector.tensor_scalar_add(grT[:, _hh(h)], pgr[0:TF, :],
                                        lc[0:TF, 18:19])

        xts2 = wk.tile([TF, N], F32, tag="xts", name="xts2")
        if l == 0:
            nc.sync.dma_start(xts2[:], self.d_xT[s * TF:(s + 1) * TF, :])
        else:
            nc.sync.dma_start(xts2[:], self.xtmid[s, :, :])
        v = wk.tile([TF, N], F32, tag="lvv", name="lvv")
        # v = (xts2 - bp) - perT  (fold level bp bias, lcol col 19)
        nc.vector.scalar_tensor_tensor(v[:], xts2[:], lc[0:TF, 19:20], perT[:],
                                       op0=ALU.subtract, op1=ALU.subtract)
        nc.vector.tensor_scalar_mul(v[:], v[:], lc[0:TF, 16:17])
        if OMA_BCAST:
            omlv_ap = lc[0:TF, 17:18].broadcast_to([TF, N])
        else:
            omlv = wk.tile([TF, N], F32, tag="omlv", name="omlv")
            nc.vector.memset(omlv[:], 1.0)
            nc.vector.tensor_scalar_mul(omlv[:], omlv[:], lc[0:TF, 17:18])
            omlv_ap = omlv[:]
        pt = wk.tile([TF, N], F32, tag="lvp", name="lvp")
        nc.vector.tensor_tensor_scan(pt[:], omlv_ap, v[:], 0.0,
                                     op0=ALU.mult, op1=ALU.add)
        gt = wk.tile([TF, N], F32, tag="lvv", name="lvg")
        nc.vector.tensor_tensor_scan(gt[:], omlv_ap, grT[:], 0.0,
                                     op0=ALU.mult, op1=ALU.add)
        xnew = wk.tile([TF, N], F32, tag="grT", name="xnew")
        nc.vector.tensor_add(xnew[:], pt[:], gt[:])
        # issue on Pool: keeps this late-blocking store off the SP DMA queue
        nc.gpsimd.dma_start(self.xtmid[s, :, :], xnew[:])

        # --- FF (layer 0 only); z4 stays in SBUF for l1
        if not last:
            return self._ff(s, z3, wk)
        return None

    # ---------- LN stats ----------
    def _ln_stats(self, zset, wk, tagp):
        nc = self.nc
        st = wk.tile([128, 8 * NT], F32, tag=f"st{tagp}", name=f"st{tagp}")
        mu8 = st[:, 0:NT]
        s28 = st[:, NT:2 * NT]
        for tt in range(NT):
            scr = wk.tile([128, D], F32,
                          tag="lnscr" if tt % 2 == 0 else "lnscr2",
                          name="lnscr")
            nc.vector.tensor_reduce(st[:, tt:tt + 1], zset[tt][:],
                                    mybir.AxisListType.X, op=ALU.add)
            nc.scalar.activation(scr[:], zset[tt][:], AF.Square,
                                 accum_out=st[:, NT + tt:NT + tt + 1])
        mun = st[:, 2 * NT:3 * NT]
        nc.vector.tensor_scalar_mul(mun, mu8, 1.0 / D)
        ex2 = st[:, 3 * NT:4 * NT]
        nc.vector.tensor_scalar_mul(ex2, s28, 1.0 / D)
        musq = st[:, 4 * NT:5 * NT]
        nc.scalar.activation(musq, mun, AF.Square)
        var = st[:, 5 * NT:6 * NT]
        nc.vector.tensor_sub(var, ex2, musq)
        sd = st[:, 6 * NT:7 * NT]
        nc.scalar.activation(sd, var, AF.Sqrt, bias=self.epst[:, 0:1])
        rs = st[:, 7 * NT:8 * NT]
        nc.vector.reciprocal(rs, sd)
        nmurs = st[:, 4 * NT:5 * NT]  # overwrite musq slot
        nc.vector.tensor_mul(nmurs, mun, rs)
        nc.vector.tensor_scalar_mul(nmurs, nmurs, -1.0)
        return rs, nmurs

    # ---------- FF block ----------
    def _ff(self, s, z3, wk):
        nc = self.nc
        ones, idn = self.ones, self.idn
        cpk = self.cpk
        rs, nmurs = self._ln_stats(z3, wk, "pre")
        # h = (z3-mu)*rs, overwrite z3 tiles in place via scratch
        h_ = [wk.tile([128, D], F32, tag=f"B2_{tt}", name=f"h{tt}")
              for tt in range(NT)]
        for tt in range(NT):
            nc.scalar.activation(h_[tt][:], z3[tt][:], AF.Identity,
                                 scale=rs[:, tt:tt + 1], bias=nmurs[:, tt:tt + 1])
        hT = [wk.tile([128, N], F32, tag=f"A2_{dt}", name=f"hT{dt}")
              for dt in range(ND)]
        for h in range(2):
            for dt in range(ND):
                pT = self.bank(dt)
                for q in range(4):
                    nc.tensor.transpose(pT[:, _sl(q)], h_[h * 4 + q][:, _sl(dt)],
                                        idn[:])
                if h == 0:
                    nc.scalar.copy(hT[dt][:, _hh(h)], pT[:])
                else:
                    nc.vector.tensor_copy(hT[dt][:, _hh(h)], pT[:])
        znT = [wk.tile([128, N], F32, tag=f"A1_{dt}", name=f"znT{dt}")
               for dt in range(ND)]
        for h in range(2):
            for dt in range(ND):
                nc.scalar.activation(znT[dt][:, _hh(h)], hT[dt][:, _hh(h)],
                                     AF.Identity, scale=cpk[:, dt:dt + 1],
                                     bias=cpk[:, 4 + dt:5 + dt])

        yT = [wk.tile([128, N], F32, tag=f"A2_{dt}", name=f"yT{dt}")
              for dt in range(ND)]
        for h in range(2):
            # split znT h-half into bf16 hi/lo (cols 0:512 hi, 512:1024 lo)
            znb = [wk.tile([128, 1024], BF16, tag=f"B3_{kt}", name=f"znb{kt}")
                   for kt in range(ND)]
            for kt in range(ND):
                nc.vector.tensor_copy(znb[kt][:, 0:512], znT[kt][:, _hh(h)])
                nc.vector.tensor_sub(znb[kt][:, 512:1024], znT[kt][:, _hh(h)],
                                     znb[kt][:, 0:512])
            pzf = [self.bank(dt) for dt in range(ND)]
            for m in range(NM):
                w1m = wk.tile([128, 2 * ND * 128], BF16,
                              tag=f"w1mh{m % 2}", name="w1m")
                nc.sync.dma_start(w1m[:], self.d_ffw1t[m, :, :])
                ph = self.bank(4 + m % 2)
                for kt in range(ND):
                    nc.tensor.matmul(ph[:], w1m[:, _sl(kt)], znb[kt][:, 0:512],
                                     start=(kt == 0), stop=False)
                    nc.tensor.matmul(ph[:], w1m[:, _sl(kt)], znb[kt][:, 512:1024],
                                     start=False, stop=False)
                    nc.tensor.matmul(ph[:], w1m[:, 512 + 128 * kt:640 + 128 * kt],
                                     znb[kt][:, 0:512],
                                     start=False, stop=(kt == ND - 1))
                sig = wk.tile([128, 512], F32, tag=f"sig{m % 2}", name="sig")
                nc.scalar.activation(sig[:], ph[:], AF.Sigmoid,
                                     bias=cpk[:, 8 + m:9 + m])
                # bf16 hi/lo split of sig (Pool) for 3x 1-cyc/row matmuls
                sighl = wk.tile([128, 1024], BF16,
                                tag="amp2" if m % 2 == 0 else "lnscr",
                                name="sighl")
                nc.vector.tensor_copy(sighl[:, 0:512], sig[:])
                nc.vector.tensor_sub(sighl[:, 512:1024], sig[:],
                                     sighl[:, 0:512])
                w2m = wk.tile([128, 1024], BF16, tag=f"w2m{m % 2}", name="w2m")
                nc.sync.dma_start(w2m[:], self.d_ffw2hl[_sl(m), :])
                for dt in range(ND):
                    nc.tensor.matmul(pzf[dt][:], w2m[:, _sl(dt)],
                                     sighl[:, 0:512],
                                     start=(m == 0), stop=False)
                    nc.tensor.matmul(pzf[dt][:], w2m[:, _sl(dt)],
                                     sighl[:, 512:1024],
                                     start=False, stop=False)
                    nc.tensor.matmul(pzf[dt][:], w2m[:, 512 + dt * 128:
                                                     640 + dt * 128],
                                     sighl[:, 0:512],
                                     start=False, stop=(m == NM - 1))
            for dt in range(ND):
                nc.vector.scalar_tensor_tensor(yT[dt][:, _hh(h)], pzf[dt][:],
                                               cpk[:, 24 + dt:25 + dt],
                                               znT[dt][:, _hh(h)],
                                               op0=ALU.add, op1=ALU.add)

        # fully per-tt post-LN chains: z4[0]/zhl[0] are ready before the last
        # yT transposes finish, so l1's rfft starts with no barrier on the
        # batched stats
        gb, bb = self.gbt, self.bbt
        z4 = [wk.tile([128, D], F32, tag=f"B3_{tt}", name=f"z4_{tt}")
              for tt in range(NT)]
        zhl = [wk.tile([128, 1024], BF16, tag=f"B2_{tt}", name=f"zhl{tt}")
               for tt in range(NT)]
        st = wk.tile([128, 8 * NT], F32, tag="stpost", name="stpost")
        for tt in range(NT):
            pT = self.bank(6 + tt % 2)
            for dt in range(ND):
                nc.tensor.transpose(pT[:, _sl(dt)], yT[dt][:, _sl(tt)], idn[:])
            y_t = wk.tile([128, D], F32, tag=f"B4_{tt}", name=f"y{tt}")
            nc.scalar.copy(y_t[:], pT[:])
            scr = wk.tile([128, D], F32,
                          tag="lnscr" if tt % 2 == 0 else "lnscr2",
                          name="lnscr")
            mu = st[:, tt:tt + 1]
            s2 = st[:, NT + tt:NT + tt + 1]
            nc.vector.tensor_reduce(mu, y_t[:], mybir.AxisListType.X,
                                    op=ALU.add)
            nc.scalar.activation(scr[:], y_t[:], AF.Square, accum_out=s2)
            mun = st[:, 2 * NT + tt:2 * NT + tt + 1]
            nc.vector.tensor_scalar_mul(mun, mu, 1.0 / D)
            musq = st[:, 3 * NT + tt:3 * NT + tt + 1]
            nc.scalar.activation(musq, mun, AF.Square)
            var = st[:, 4 * NT + tt:4 * NT + tt + 1]
            nc.vector.scalar_tensor_tensor(var, s2, 1.0 / D, musq,
                                           op0=ALU.mult, op1=ALU.subtract)
            sd = st[:, 5 * NT + tt:5 * NT + tt + 1]
            nc.scalar.activation(sd, var, AF.Sqrt, bias=self.epst[:, 0:1])
            rs = st[:, 6 * NT + tt:6 * NT + tt + 1]
            nc.vector.reciprocal(rs, sd)
            nmurs = st[:, 7 * NT + tt:7 * NT + tt + 1]
            nc.vector.scalar_tensor_tensor(nmurs, mun, -1.0, rs,
                                           op0=ALU.mult, op1=ALU.mult)
            nc.scalar.activation(scr[:], y_t[:], AF.Identity,
                                 scale=rs, bias=nmurs)
            nc.vector.tensor_mul(z4[tt][:], scr[:], gb[:])
            nc.vector.tensor_add(z4[tt][:], z4[tt][:], bb[:])
            nc.gpsimd.tensor_copy(zhl[tt][:, 0:512], z4[tt][:])
            nc.gpsimd.tensor_sub(zhl[tt][:, 512:1024], z4[tt][:],
                                 zhl[tt][:, 0:512])
        return z4, zhl

    # ---------- output head ----------
    def _output(self, s, wk):
        nc = self.nc
        ones = self.ones
        po = self.bank(7)
        for kt in range(ND):
            nc.tensor.matmul(po[0:TF, 0:HOR], self.outwt[:, kt * TF:(kt + 1) * TF],
                             self.aggt[:, kt * HOR:(kt + 1) * HOR],
                             start=(kt == 0), stop=False)
        nc.tensor.matmul(po[0:TF, 0:HOR], self.outbt[0:1, 0:TF],
                         ones[0:1, 0:HOR], start=False, stop=True)
        xfin = wk.tile([TF, N], F32, tag="lvp", name="xfin")
        nc.gpsimd.dma_start(xfin[:], self.xtmid[s, :, :])
        oT = wk.tile([TF, HOR], F32, tag="lvv", name="oT")
        nc.vector.tensor_scalar_add(oT[:], po[0:TF, 0:HOR], xfin[:, N - 1:N])
        nc.gpsimd.dma_start(self.d_out[s * TF:(s + 1) * TF, :], oT[:])


def _get_nc():
    if "nc" not in _CACHE:
        _CACHE["nc"] = K().build()
    return _CACHE["nc"]


def _common_maps(inputs, w2d, dft, ib, e8):
    return dict(
        w2d=w2d,
        ones1=np.ones((1, N), np.float32),
        dft=dft, ib=ib,
        idn=np.eye(128, dtype=np.float32),
        e8=e8,
        dfthl=np.concatenate([_split_hi(dft), _split_lo(dft)], axis=1),
        ibhl=np.concatenate([_split_hi(ib), _split_lo(ib)], axis=1),
        ibr=ib,
        winr=np.asarray(inputs["mhesa_win"][1], np.float32),
        woutr=np.asarray(inputs["mhesa_wout"][1], np.float32),
        winhl=np.concatenate(
            [_split_hi(np.asarray(inputs["mhesa_win"][0], np.float32)),
             _split_lo(np.asarray(inputs["mhesa_win"][0], np.float32))],
            axis=1),
        wouthl=np.concatenate(
            [_split_hi(np.asarray(inputs["mhesa_wout"][0], np.float32)),
             _split_lo(np.asarray(inputs["mhesa_wout"][0], np.float32))],
            axis=1),
        boutr=np.asarray(inputs["mhesa_bout"], np.float32).reshape(L, 1, D),
        lcolp=_pack_lcol(inputs),
        alpha8=np.asarray(inputs["mhesa_alpha"], np.float32).reshape(L, HEADS, 1),
        ffw1t=_pack_w1(np.asarray(inputs["ff_w1"], np.float32)),
        cpkp=_pack_cpk(inputs),
        ffw2hl=np.concatenate(
            [_split_hi(np.asarray(inputs["ff_w2"], np.float32)),
             _split_lo(np.asarray(inputs["ff_w2"], np.float32))], axis=1),
        gpostr=np.asarray(inputs["ff_post_g"], np.float32).reshape(1, D),
        bpostr=np.asarray(inputs["ff_post_b"], np.float32).reshape(1, D),
        lvwg=np.asarray(inputs["level_wg"], np.float32),
        lvwp=np.asarray(inputs["level_wp"], np.float32),
        lvbg=np.asarray(inputs["level_bg"], np.float32).reshape(L, 1, TF),
        lvbp=np.asarray(inputs["level_bp"], np.float32).reshape(L, 1, TF),
        lvalpha=np.asarray(inputs["level_alpha"], np.float32).reshape(L, 1, 1),
        damp8=np.asarray(inputs["dampen_factor"], np.float32).reshape(HEADS, 1),
        outw=np.asarray(inputs["out_w"], np.float32)
            .reshape(ND, 128, TF).transpose(1, 0, 2).reshape(128, ND * TF)
            .copy(),
        outbr=np.asarray(inputs["out_b"], np.float32).reshape(1, TF),
    )


def _kernel_impl(inputs, runner):
    x = np.asarray(inputs["x"], np.float32)
    assert (x.shape[0], x.shape[1], x.shape[2]) == (32, N, TF)
    assert int(inputs["forecast_horizon"]) == HOR
    dft, ib = _dft_consts()
    conv_w = np.asarray(inputs["conv_w"], np.float32)
    w2d = _build_w2d(conv_w, np.asarray(inputs["conv_b"], np.float32))
    e8 = np.repeat(np.eye(HEADS, dtype=np.float32), DH, axis=1)
    nc = _get_nc()
    common = _common_maps(inputs, w2d, dft, ib, e8)
    in_maps = []
    for c in range(NCORES):
        xs = x[c * S:(c + 1) * S]
        xT = xs.transpose(0, 2, 1).reshape(S * TF, N).copy()
        in_maps.append(dict(common, xT=xT))
    res = runner(nc, in_maps)
    out = np.zeros((x.shape[0], HOR, TF), np.float32)
    for c in range(NCORES):
        oT = res.results[c]["outT"].reshape(S, TF, HOR)
        out[c * S:(c + 1) * S] = oT.transpose(0, 2, 1)
    return out, res


def kernel(**inputs):
    out, _ = _kernel_impl(
        inputs,
        lambda nc, im: run_bass_kernel_spmd(nc, im, list(range(NCORES))))
    return out


def kernel_traced(**inputs):
    """Like kernel() but with NTFF profiling; returns (out, BassKernelResults)."""
    return _kernel_impl(
        inputs,
        lambda nc, im: run_bass_kernel_spmd(nc, im, list(range(NCORES)),
                                            trace=True))



# revision 11
# speedup vs baseline: 1.4632x; 1.4632x over previous
"""ETSFormer forward pass on 8 Trainium2 NeuronCores (Bass/Tile).

Data-parallel over batch: 32 samples -> 8 cores x 4 samples, weights
replicated, no collectives. The reference's FFT machinery is computed
exactly without FFTs:
  - freq_attention: dense DFT matmuls + hardware top-8 (vector.max) mask
  - mhesa / level exponential smoothing: the reference FFT cross-correlation
    is exactly a first-order EMA -> hardware prefix scan (tensor_tensor_scan)
  - fourier_extrapolate: exact slice (Dirichlet kernel identity)

Precision: PREC selects per-GEMM-group dtype. "f32r" = fp32-reduced
(FP22 truncated, 1 cyc/row on PE -- same speed as bf16) vs the fallback
"hl" = bf16 hi/lo 3-term split (~2^-16, 3 cyc/row) / "f32" = true fp32
(4 cyc/row). The top-4 frequency mask is rank-sensitive; flags are
tuned empirically against the end-to-end error gate.
"""
import numpy as np
from contextlib import ExitStack

import concourse.bass as bass
import concourse.bacc as bacc
import concourse.tile as tile
from concourse import mybir
from concourse.bass_utils import run_bass_kernel_spmd

F32 = mybir.dt.float32
F32R = mybir.dt.float32r
BF16 = mybir.dt.bfloat16
AF = mybir.ActivationFunctionType
ALU = mybir.AluOpType

N = 1024
D = 512
TF = 7
HEADS = 8
DH = D // HEADS
L = 2
S = 4
NCORES = 8
HOR = 96
FD = 2048
NT = N // 128   # 8
ND = D // 128   # 4
NM = FD // 128  # 16

_CACHE = {}

# per-stage precision: "f32r" fast path vs baseline "hl" (bf16 3-term)
# / "f32" (true fp32) fallback.
PREC = dict(
    l0head="f32",    # conv z GEMM + low-rank DFT: feeds the layer-0 top-4
                     # ranking, which flips even under 2^-12 weight rounding
                     # (emulation: 52 flips, 2.7e-2 err) -- keep exact fp32
    irfft0="f32r",   # layer-0 irfft (feeds layer-1 ranking path)
    mhesa0="f32r",   # layer-0 win/wout GEMMs
    ff="f32r",       # FF block w1/w2 GEMMs
    rfft1="f32r",    # layer-1 rfft (feeds layer-1 ranking directly)
)


def _rne11(x):
    # round fp32 mantissa to 11 explicit bits (fp22): the PE's f32r mode
    # truncates operands to fp22, so pre-rounded weights pass through
    # losslessly -- halves f32r noise and removes the truncation bias
    xi = np.ascontiguousarray(np.asarray(x, np.float32)).view(np.uint32)
    return ((xi + np.uint32(0x800)) & np.uint32(0xFFFFF000)).view(np.float32)


def _dft_consts():
    if "dft" not in _CACHE:
        t = np.arange(N)
        f = np.arange(513)
        ang = 2.0 * np.pi * np.outer(t, f) / N
        cos = np.cos(ang)
        sin = np.sin(ang)
        dft = np.zeros((N, 1024), np.float64)
        dft[:, 0:512] = cos[:, 0:512]
        dft[:, 512] = cos[:, 512]
        dft[:, 513:1024] = sin[:, 1:512]
        c = np.full(513, 2.0)
        c[0] = 1.0
        c[512] = 1.0
        ib = np.zeros((1024, N), np.float64)
        ib[0:512, :] = (c[0:512, None] / N) * cos[:, 0:512].T
        ib[512, :] = (1.0 / N) * cos[:, 512]
        ib[513:1024, :] = (2.0 / N) * sin[:, 1:512].T
        _CACHE["dft"] = dft.astype(np.float32)
        _CACHE["ib"] = ib.astype(np.float32)
    return _CACHE["dft"], _CACHE["ib"]


def _sl(i, w=128):
    return slice(i * w, (i + 1) * w)


def _split_hi(x):
    import ml_dtypes
    return x.astype(ml_dtypes.bfloat16)


def _split_lo(x):
    import ml_dtypes
    hi = x.astype(ml_dtypes.bfloat16).astype(np.float32)
    return (x - hi).astype(ml_dtypes.bfloat16)


def _pack_w1(w1):
    # bf16 hi|lo tiles for the "hl" fallback FF path
    hi, lo = _split_hi(w1), _split_lo(w1)
    out = np.zeros((NM, 128, 1024), hi.dtype)
    for m in range(NM):
        for kt in range(ND):
            out[m, :, 128 * kt:128 * (kt + 1)] = hi[_sl(kt), _sl(m)]
            out[m, :, 512 + 128 * kt:640 + 128 * kt] = lo[_sl(kt), _sl(m)]
    return out


def _pack_w1r(w1):
    # f32r per-m contiguous [128(k), 4x128(m)] tiles
    out = np.zeros((NM, 128, 512), np.float32)
    for m in range(NM):
        for kt in range(ND):
            out[m, :, 128 * kt:128 * (kt + 1)] = w1[_sl(kt), _sl(m)]
    return out


def _pack_cpk(inputs):
    # cols: gpre(4) | bpre(4) | ffb1(16) | ffb2(4), each D/FD vector folded
    # into [128, k] column blocks -- one DMA instead of 24
    out = np.zeros((128, 28), np.float32)
    out[:, 0:4] = np.asarray(inputs["ff_pre_g"], np.float32).reshape(4, 128).T
    out[:, 4:8] = np.asarray(inputs["ff_pre_b"], np.float32).reshape(4, 128).T
    out[:, 8:24] = np.asarray(inputs["ff_b1"], np.float32).reshape(16, 128).T
    out[:, 24:28] = np.asarray(inputs["ff_b2"], np.float32).reshape(4, 128).T
    return out


def _pack_lcol(inputs):
    # per layer: init(4 cols) | bin(4 cols)
    out = np.zeros((L, 128, 8), np.float32)
    ini = np.asarray(inputs["mhesa_init"], np.float32).reshape(L, D)
    bi = np.asarray(inputs["mhesa_bin"], np.float32)
    for l in range(L):
        out[l, :, 0:4] = ini[l].reshape(4, 128).T
        out[l, :, 4:8] = bi[l].reshape(4, 128).T
    return out


def _build_w2d(conv_w, conv_b):
    # rows 32k+c hold conv_w[:, c, k] (32-aligned partition groups so the
    # on-device shifted copies keep legal base partitions); row 95 is the
    # bias row, paired with an all-ones row 95 of xsh on device.
    w2d = np.zeros((96, D), np.float32)
    for k in range(3):
        for c in range(TF):
            w2d[32 * k + c] = conv_w[:, c, k]
    w2d[95] = conv_b
    return w2d


def _hh(h):
    return slice(h * 512, (h + 1) * 512)


class K:
    def __init__(self):
        nc = bacc.Bacc()
        self.nc = nc
        p = nc.declare_dram_parameter
        self.d_xT = p("xT", [S * TF, N], F32, isOutput=False)
        self.d_w2d = p("w2d", [96, D],
                       F32R if PREC["l0head"] == "f32r" else F32,
                       isOutput=False)
        self.d_ones1 = p("ones1", [1, N], F32, isOutput=False)
        if PREC["l0head"] == "f32":
            self.d_dft = p("dft", [N, 1024], F32, isOutput=False)
        if PREC["l0head"] == "f32r" or PREC["rfft1"] == "f32r":
            self.d_dftr = p("dftr", [N, 1024], F32R, isOutput=False)
        if PREC["rfft1"] == "hl":
            self.d_dfthl = p("dfthl", [N, 2048], BF16, isOutput=False)
        self.d_ibr = p("ibr", [1024, N], F32R, isOutput=False)
        self.d_winr = p("winr", [L, D, D], F32R, isOutput=False)
        self.d_woutr = p("woutr", [L, D, D], F32R, isOutput=False)
        if PREC["irfft0"] == "hl":
            self.d_ibhl = p("ibhl", [1024, 2048], BF16, isOutput=False)
        if PREC["mhesa0"] == "hl":
            self.d_winhl = p("winhl", [D, 2 * D], BF16, isOutput=False)
            self.d_wouthl = p("wouthl", [D, 2 * D], BF16, isOutput=False)
        self.d_idn = p("idn", [128, 128], F32, isOutput=False)
        self.d_e8 = p("e8", [HEADS, D], F32, isOutput=False)
        self.d_bout = p("boutr", [L, 1, D], F32, isOutput=False)
        self.d_lcolp = p("lcolp", [L, 128, 8], F32, isOutput=False)
        self.d_al8 = p("alpha8", [L, HEADS, 1], F32, isOutput=False)
        if PREC["ff"] == "f32r":
            self.d_ffw1r = p("ffw1r", [NM, 128, 512], F32R, isOutput=False)
            self.d_ffw2r = p("ffw2r", [FD, D], F32R, isOutput=False)
        else:
            self.d_ffw1t = p("ffw1t", [NM, 128, 2 * ND * 128], BF16,
                             isOutput=False)
            self.d_ffw2hl = p("ffw2hl", [FD, 2 * D], BF16, isOutput=False)
        self.d_cpkp = p("cpkp", [128, 28], F32, isOutput=False)
        self.d_gpost = p("gpostr", [1, D], F32, isOutput=False)
        self.d_bpost = p("bpostr", [1, D], F32, isOutput=False)
        self.d_wg = p("lvwg", [L, D, TF], F32, isOutput=False)
        self.d_wp = p("lvwp", [L, D, TF], F32, isOutput=False)
        self.d_bg = p("lvbg", [L, 1, TF], F32, isOutput=False)
        self.d_bp = p("lvbp", [L, 1, TF], F32, isOutput=False)
        self.d_alv = p("lvalpha", [L, 1, 1], F32, isOutput=False)
        self.d_damp = p("damp8", [HEADS, 1], F32, isOutput=False)
        self.d_outw = p("outw", [128, ND * TF], F32, isOutput=False)
        self.d_outb = p("outbr", [1, TF], F32, isOutput=False)
        self.d_out = p("outT", [S * TF, HOR], F32, isOutput=True)
        self.xtmid = nc.dram_tensor("xtmid", [S, TF, N], F32)

    # psum bank helper: tag-based reuse of the 8 banks
    def bank(self, i, shape=(128, 512), dtype=F32):
        tl = self.psp.tile(list(shape), dtype, tag=f"bk{i}", name=f"bk{i}")
        return tl

    def build(self):
        nc = self.nc
        with ExitStack() as ctx:
            self.tc = ctx.enter_context(tile.TileContext(nc))
            tc = self.tc
            top = ctx.enter_context(tc.tile_pool(name="top", bufs=1))

            idn = top.tile([128, 128], F32, name="idn")
            nc.sync.dma_start(idn[:], self.d_idn[:])
            idnr = top.tile([128, 128], F32R, name="idnr")
            nc.vector.tensor_copy(idnr[:], idn[:])
            self.idnr = idnr
            ones = top.tile([128, 128], F32, name="ones")
            nc.vector.memset(ones[:], 1.0)
            w2d = top.tile([96, D],
                           F32R if PREC["l0head"] == "f32r" else F32,
                           name="w2d")
            nc.sync.dma_start(w2d[:], self.d_w2d[:])
            outbr = top.tile([1, TF], F32, name="outbr")
            nc.sync.dma_start(outbr[:], self.d_outb[:])
            self.outbt = outbr
            # col pack: gpre(4) | bpre(4)
            cpk = top.tile([128, 28], F32, name="cpk")
            nc.sync.dma_start(cpk[:], self.d_cpkp[:])
            outw = top.tile([128, ND * TF], F32, name="outw")
            nc.sync.dma_start(outw[:], self.d_outw[:])
            eps = top.tile([128, 1], F32, name="eps")
            nc.vector.memset(eps[:], 1e-5)
            self.epst = eps
            gbt = top.tile([128, D], F32, name="gbt")
            bbt = top.tile([128, D], F32, name="bbt")
            self.gbt, self.bbt = gbt, bbt
            agg = top.tile([128, ND * HOR], F32, name="agg")
            csd = top.tile([128, ND * HOR], F32, name="csd")

            self.idn, self.ones, self.cpk = idn, ones, cpk
            self.w2dt_, self.aggt, self.csdt = w2d, agg, csd
            self.outwt = outw

            self.psp = ctx.enter_context(
                tc.tile_pool(name="ps", bufs=1, space="PSUM"))
            # both layers' constants resident; samples run L0->L1 back to
            # back so L1's DVE-heavy tail overlaps the next sample's
            # PE-heavy head, and z4 never round-trips through DRAM
            lay0p = ctx.enter_context(tc.tile_pool(name="lay0", bufs=1))
            lay1p = ctx.enter_context(tc.tile_pool(name="lay1", bufs=1))
            with tc.tile_pool(name="ini", bufs=1) as ini:
                e8 = ini.tile([HEADS, D], F32, name="e8")
                nc.sync.dma_start(e8[:], self.d_e8[:])
                self.e8t = e8
                self._damp_cs(ini, self.psp)
                lay = [self._layer_consts(0, lay0p),
                       self._layer_consts(1, lay1p)]
            wk = ctx.enter_context(tc.tile_pool(name="wk", bufs=1))
            for s in range(S):
                z4 = self._sample(0, s, lay[0], wk)
                self._sample(1, s, lay[1], wk, zin=z4)
                self._output(s, wk)

        nc.compile()
        return nc

    # ---------- dampening cumsum -> csd [128, ND*HOR] ----------
    def _damp_cs(self, ini, inips):
        nc = self.nc
        ones = self.ones
        dcol = ini.tile([HEADS, 1], F32, name="dcol")
        nc.sync.dma_start(dcol[:], self.d_damp[:])
        df = ini.tile([HEADS, 1], F32, name="dfsig")
        nc.scalar.activation(df[:], dcol[:], AF.Sigmoid)
        dfb = ini.tile([HEADS, HOR], F32, name="dfb")
        nc.scalar.activation(dfb[:], ones[0:HEADS, 0:HOR], AF.Identity,
                             scale=df[:, 0:1])
        zer = ini.tile([HEADS, HOR], F32, name="zer8")
        nc.vector.memset(zer[:], 0.0)
        dfp = ini.tile([HEADS, HOR], F32, name="dfp")
        nc.vector.tensor_tensor_scan(dfp[:], dfb[:], zer[:], 1.0,
                                     op0=ALU.mult, op1=ALU.add)
        cs8 = ini.tile([HEADS, HOR], F32, name="cs8")
        nc.vector.tensor_tensor_scan(cs8[:], ones[0:HEADS, 0:HOR], dfp[:], 0.0,
                                     op0=ALU.mult, op1=ALU.add)
        for dt in range(ND):
            pini = inips.tile([128, HOR], F32, tag=f"bk{dt}", name="pini")
            nc.tensor.matmul(pini[:], self.e8t[:, _sl(dt)], cs8[:],
                             start=True, stop=True)
            nc.scalar.copy(self.csdt[:, dt * HOR:(dt + 1) * HOR], pini[:])
        # hoisted FF post-LN gamma/beta broadcasts (layer-invariant)
        rows = ini.tile([1, 1024], F32, name="rows")
        nc.sync.dma_start(rows[0:1, 0:512], self.d_gpost[:])
        nc.sync.dma_start(rows[0:1, 512:1024], self.d_bpost[:])
        pgb = inips.tile([128, D], F32, tag="bk4", name="pgb")
        nc.tensor.matmul(pgb[:], self.ones[0:1, 0:128],
                         rows[0:1, 0:512], start=True, stop=True)
        nc.scalar.copy(self.gbt[:], pgb[:])
        pbb = inips.tile([128, D], F32, tag="bk5", name="pbb")
        nc.tensor.matmul(pbb[:], self.ones[0:1, 0:128],
                         rows[0:1, 512:1024], start=True, stop=True)
        nc.scalar.copy(self.bbt[:], pbb[:])

    # ---------- per-layer constants ----------
    def _layer_consts(self, l, layp):
        nc = self.nc
        ones = self.ones
        last = l == L - 1
        lay = {"l": l, "last": last}

        if last or PREC["mhesa0"] == "f32r":
            win = [layp.tile([128, D], F32R, name=f"win{k}") for k in range(ND)]
            wout = [layp.tile([128, D], F32R, name=f"wout{k}")
                    for k in range(ND)]
            for kt in range(ND):
                nc.scalar.dma_start(win[kt][:], self.d_winr[l, _sl(kt), :])
                nc.scalar.dma_start(wout[kt][:], self.d_woutr[l, _sl(kt), :])
        else:
            # bf16 hi|lo packed (cols 0:512 hi, 512:1024 lo)
            win = [layp.tile([128, 2 * D], BF16, name=f"win{k}")
                   for k in range(ND)]
            wout = [layp.tile([128, 2 * D], BF16, name=f"wout{k}")
                    for k in range(ND)]
            for kt in range(ND):
                nc.scalar.dma_start(win[kt][:], self.d_winhl[_sl(kt), :])
                nc.scalar.dma_start(wout[kt][:], self.d_wouthl[_sl(kt), :])

        # lrows: p0 = bout[512]; p32 = bg[7] then bp at cols 16..23
        lrows = layp.tile([128, 512], F32, name="lrows")
        nc.sync.dma_start(lrows[0:1, 0:D], self.d_bout[l, :, :])
        nc.sync.dma_start(lrows[32:33, 0:TF], self.d_bg[l, :, :])
        nc.sync.dma_start(lrows[32:33, 16:16 + TF], self.d_bp[l, :, :])

        # bout broadcast [128, D] (replaces per-tile bias matmuls)
        boutb = layp.tile([128, D], F32, name="boutb")
        pbo = self.psp.tile([128, D], F32, tag="bk7", name="pbo")
        nc.tensor.matmul(pbo[:], ones[0:1, 0:128], lrows[0:1, 0:D],
                         start=True, stop=True)
        nc.scalar.copy(boutb[:], pbo[:])

        # lcol pack [128, 16]: al(4) oma(4) init(4) bi(4); plus lv cols [7,1]
        # cols 18/19: level bg/bp as [7,1] columns
        lcol = layp.tile([128, 24], F32, name="lcol")
        nc.sync.dma_start(lcol[0:TF, 18:19],
                          self.d_bg[l, :, :].rearrange("a b -> b a"))
        nc.sync.dma_start(lcol[0:TF, 19:20],
                          self.d_bp[l, :, :].rearrange("a b -> b a"))
        al8 = layp.tile([HEADS, 1], F32, tag="al8t", name="al8")
        nc.sync.dma_start(al8[:], self.d_al8[l, :, :])
        al8s = layp.tile([HEADS, 1], F32, tag="al8s", name="al8s")
        nc.scalar.activation(al8s[:], al8[:], AF.Sigmoid)
        for dt in range(ND):
            pal = self.psp.tile([128, 1], F32, tag="bk0", name="pal")
            nc.tensor.matmul(pal[:], self.e8t[:, _sl(dt)], al8s[:],
                             start=True, stop=True)
            nc.scalar.copy(lcol[:, dt:dt + 1], pal[:])
        nc.sync.dma_start(lcol[:, 8:16], self.d_lcolp[l, :, :])
        for dt in range(ND):
            nc.vector.tensor_scalar(lcol[:, 4 + dt:5 + dt], lcol[:, dt:dt + 1],
                                    -1.0, 1.0, op0=ALU.mult, op1=ALU.add)
        nc.vector.tensor_sub(lcol[:, 12:16], lcol[:, 12:16], lcol[:, 8:12])
        # col 20:24 = al*(bi-init) + (1-al)*init -- the scan-initial folded
        # into xd[0] so the scan can run with a 0.0 immediate initial
        bi = layp.tile([128, ND], F32, tag="bitmp", name="bitmp")
        nc.vector.tensor_mul(lcol[:, 20:24], lcol[:, 0:4], lcol[:, 12:16])
        nc.vector.tensor_mul(bi[:], lcol[:, 4:8], lcol[:, 8:12])
        nc.vector.tensor_add(lcol[:, 20:24], lcol[:, 20:24], bi[:])
        # level alpha
        alv = layp.tile([1, 1], F32, tag="alvt", name="alv")
        nc.sync.dma_start(alv[:], self.d_alv[l, :, :])
        alvs = layp.tile([1, 1], F32, tag="alvst", name="alvs")
        nc.scalar.activation(alvs[:], alv[:], AF.Sigmoid)
        pv = self.psp.tile([TF, 1], F32, tag="bk1", name="palv")
        nc.tensor.matmul(pv[:], ones[0:1, 0:TF], alvs[:], start=True, stop=True)
        nc.scalar.copy(lcol[0:TF, 16:17], pv[:])
        nc.vector.tensor_scalar(lcol[0:TF, 17:18], lcol[0:TF, 16:17], -1.0, 1.0,
                                op0=ALU.mult, op1=ALU.add)

        # level weights [128, TF] x4 packed [128, 2*ND*TF], as fp32r
        lwf = layp.tile([128, 2 * ND * TF], F32, tag="lwf", name="lwf")
        for kt in range(ND):
            nc.sync.dma_start(lwf[:, kt * TF:(kt + 1) * TF], self.d_wg[l, _sl(kt), :])
            nc.sync.dma_start(lwf[:, (ND + kt) * TF:(ND + kt + 1) * TF],
                              self.d_wp[l, _sl(kt), :])
        lw = layp.tile([128, 2 * ND * TF], F32R, name="lw")
        nc.vector.tensor_copy(lw[:], lwf[:])

        lay.update(win=win, wout=wout, lrows=lrows, lcol=lcol, lw=lw,
                   boutb=boutb)
        return lay

    # ---------- one sample through one layer ----------
    def _sample(self, l, s, lay, wk, zin=None):
        nc = self.nc
        ones, idn = self.ones, self.idn
        last = lay["last"]
        agg = self.aggt
        irf_r = last or PREC["irfft0"] == "f32r"
        mh_r = last or PREC["mhesa0"] == "f32r"

        def aggsl(dt):
            return self.aggt[:, dt * HOR:(dt + 1) * HOR]

        # --- z input: conv (l0) or handed over in SBUF from l0 (l1)
        if l == 0:
            hr = PREC["l0head"] == "f32r"
            # agg is per-sample now; clear it (waits on prior _output read)
            nc.gpsimd.memset(agg[:], 0.0)
            z = [wk.tile([128, D], F32R, tag=f"B1_{tt}", name=f"z{tt}")
                 for tt in range(NT)]
            # low-rank path: x is rank-7, so z = xsh^T @ w2d (rows 32k+c hold
            # the 3 shifts of the 7 channels; row 95 = ones * conv_b) and
            # DFT(z) = w2d^T @ (xsh^T @ dft) -- the DFT runs in the 96-dim
            # input space instead of the 512-dim channel space.
            xshf = wk.tile([96, N], F32, tag="xsh", name="xshf")
            xts = wk.tile([TF, N], F32, tag="xts", name="xts")
            nc.sync.dma_start(xts[:], self.d_xT[s * TF:(s + 1) * TF, :])
            nc.gpsimd.memset(xshf[:], 0.0)
            nc.gpsimd.tensor_copy(xshf[0:TF, 1:N], xts[:, 0:N - 1])
            nc.gpsimd.tensor_copy(xshf[32:32 + TF, 0:N], xts[:, 0:N])
            nc.gpsimd.tensor_copy(xshf[64:64 + TF, 0:N - 1], xts[:, 1:N])
            nc.sync.dma_start(xshf[95:96, :], self.d_ones1[:])
            if hr:
                # Pool can't touch f32r (ISA); one DVE copy re-tags for PE
                xsh = wk.tile([96, N], F32R, tag="xshr", name="xsh")
                nc.vector.tensor_copy(xsh[:], xshf[:])
            else:
                xsh = xshf
            xshT = [wk.tile([128, 96], F32R if hr else F32,
                            tag=f"xshT{tt}", name=f"xshT{tt}")
                    for tt in range(NT)]
            psF1A = self.bank(2, shape=(96, 512))
            psF1B = self.bank(3, shape=(96, 512))
            tid = self.idnr if hr else idn
            d_dft_src = self.d_dftr if hr else self.d_dft
            for tt in range(NT):
                pz = self.bank(tt % 2)
                nc.tensor.matmul(pz[:], xsh[:, _sl(tt)], self.w2dt_[:],
                                 start=True, stop=True)
                nc.scalar.copy(z[tt][:], pz[:])
                pxT = self.bank(6, shape=(128, 96),
                                dtype=F32R if hr else F32)
                nc.tensor.transpose(pxT[:], xsh[:, _sl(tt)], tid[0:96, 0:96])
                nc.scalar.copy(xshT[tt][:], pxT[:])
                dftk = wk.tile([128, 1024], F32R if hr else F32,
                               tag=f"dftk{tt % 2}", name="dftk")
                nc.sync.dma_start(dftk[:], d_dft_src[_sl(tt), :])
                nc.tensor.matmul(psF1A[:], xshT[tt][:], dftk[:, 0:512],
                                 start=(tt == 0), stop=(tt == NT - 1))
                nc.tensor.matmul(psF1B[:], xshT[tt][:], dftk[:, 512:1024],
                                 start=(tt == 0), stop=(tt == NT - 1))
            F1s = wk.tile([96, 1024], F32R if hr else F32, tag="lvp",
                          name="F1s")
            nc.scalar.copy(F1s[:, 0:512], psF1A[:])
            nc.scalar.copy(F1s[:, 512:1024], psF1B[:])
            psA = [self.bank(ct) for ct in range(ND)]
            psB = [self.bank(4 + ct) for ct in range(ND)]
            for ct in range(ND):
                nc.tensor.matmul(psA[ct][:], self.w2dt_[:, _sl(ct)],
                                 F1s[:, 0:512], start=True, stop=True)
                nc.tensor.matmul(psB[ct][:], self.w2dt_[:, _sl(ct)],
                                 F1s[:, 512:1024], start=True, stop=True)
            ibkpf = []
            if irf_r:
                for pf in range(2):
                    ibkp = wk.tile([128, 1024], F32R, tag=f"dftk{pf % 2}",
                                   name="ibk")
                    nc.sync.dma_start(ibkp[:], self.d_ibr[_sl(pf), :])
                    ibkpf.append(ibkp)
            else:
                for pf in range(2):
                    ibkp = wk.tile([128, 2048], BF16, tag=f"dftk{pf % 2}",
                                   name="ibk")
                    nc.sync.dma_start(ibkp[:], self.d_ibhl[_sl(pf), :])
                    ibkpf.append(ibkp)
        else:
            z, zhl = zin

            psA = [self.bank(ct) for ct in range(ND)]
            psB = [self.bank(4 + ct) for ct in range(ND)]
            if PREC["rfft1"] == "f32r":
                # z tiles are F32R [t, d]; stationary slice [t, c-block]
                for kt in range(NT):
                    dftk = wk.tile([128, 1024], F32R, tag=f"dftk{kt % 2}",
                                   name="dftk")
                    nc.sync.dma_start(dftk[:], self.d_dftr[_sl(kt), :])
                    st0 = kt == 0
                    sp = kt == NT - 1
                    for ct in range(ND):
                        zst = z[kt][:, _sl(ct)]
                        nc.tensor.matmul(psA[ct][:], zst, dftk[:, 0:512],
                                         start=st0, stop=sp)
                        nc.tensor.matmul(psB[ct][:], zst, dftk[:, 512:1024],
                                         start=st0, stop=sp)
            else:
                # rfft via bf16 hi/lo 3-term split (exact to ~2^-17)
                for kt in range(NT):
                    dftk = wk.tile([128, 2048], BF16, tag=f"dftk{kt % 2}",
                                   name="dftk")
                    nc.sync.dma_start(dftk[:], self.d_dfthl[_sl(kt), :])
                    st0 = kt == 0
                    sp = kt == NT - 1
                    for ct in range(ND):
                        zh = zhl[kt][:, _sl(ct)]
                        zl = zhl[kt][:, 512 + 128 * ct:640 + 128 * ct]
                        nc.tensor.matmul(psA[ct][:], zh, dftk[:, 0:512],
                                         start=st0, stop=False)
                        nc.tensor.matmul(psA[ct][:], zh, dftk[:, 1024:1536],
                                         start=False, stop=False)
                        nc.tensor.matmul(psB[ct][:], zh, dftk[:, 512:1024],
                                         start=st0, stop=False)
                        nc.tensor.matmul(psB[ct][:], zh, dftk[:, 1536:2048],
                                         start=False, stop=False)
                        nc.tensor.matmul(psA[ct][:], zl, dftk[:, 0:512],
                                         start=False, stop=sp)
                        nc.tensor.matmul(psB[ct][:], zl, dftk[:, 512:1024],
                                         start=False, stop=sp)
            # prefetch the first two irfft ib stripes while the mask runs
            ibkpf = []
            for pf in range(2):
                ibkp = wk.tile([128, 1024], F32R, tag=f"dftk{pf % 2}",
                               name="ibk")
                nc.sync.dma_start(ibkp[:], self.d_ibr[_sl(pf), :])
                ibkpf.append(ibkp)

        # --- top-4 mask -> filt [ND][128, 1024] ([c, f])
        filt = [wk.tile([128, 1024], F32R if irf_r else F32,
                        tag=f"A1_{ct}", name=f"filt{ct}")
                for ct in range(ND)]
        for ct in range(ND):
            sqA = wk.tile([128, 512], F32,
                          tag="sqA" if ct % 2 == 0 else "lnscr2", name="sqA")
            nc.scalar.activation(sqA[:], psA[ct][:], AF.Square)
            sqB = wk.tile([128, 512], F32,
                          tag="w2m0" if ct % 2 == 0 else "w2m1", name="sqB")
            nc.scalar.activation(sqB[:], psB[ct][:], AF.Square)
            amp2 = wk.tile([128, 513], F32,
                           tag="amp2" if ct % 2 == 0 else "lnscr", name="amp2")
            nc.vector.tensor_add(amp2[:, 1:512], sqA[:, 1:512], sqB[:, 1:512])
            nc.scalar.copy(amp2[:, 0:1], sqA[:, 0:1])
            nc.scalar.copy(amp2[:, 512:513], sqB[:, 0:1])
            top8 = wk.tile([128, 8], F32, tag="top8", name="top8")
            nc.vector.max(top8[:], amp2[:])
            kth = top8[:, 3:4]
            nc.vector.scalar_tensor_tensor(filt[ct][:, 0:512], amp2[:, 0:512],
                                           kth, psA[ct][:],
                                           op0=ALU.is_ge, op1=ALU.mult)
            nc.vector.scalar_tensor_tensor(filt[ct][:, 513:1024], amp2[:, 1:512],
                                           kth, psB[ct][:, 1:512],
                                           op0=ALU.is_ge, op1=ALU.mult)
            nc.vector.scalar_tensor_tensor(filt[ct][:, 512:513], amp2[:, 512:513],
                                           kth, psB[ct][:, 0:1],
                                           op0=ALU.is_ge, op1=ALU.mult)

        # --- transpose filt -> filtT [f, c]; hl splits to bf16 hi|lo
        if irf_r:
            filtT = [wk.tile([128, 512], F32R, tag=f"B2_{ft}",
                             name=f"filtT{ft}") for ft in range(NT)]
            for ft in range(NT):
                pT = self.bank(ft % 4, dtype=F32R)
                for ct in range(ND):
                    nc.tensor.transpose(pT[:, _sl(ct)], filt[ct][:, _sl(ft)],
                                        self.idnr[:])
                if ft % 2 == 0:
                    nc.scalar.copy(filtT[ft][:], pT[:])
                else:
                    nc.vector.tensor_copy(filtT[ft][:], pT[:])
        else:
            filtT = [wk.tile([128, 1024], BF16, tag=f"B2_{ft}",
                             name=f"fthl{ft}") for ft in range(NT)]
            for ft in range(NT):
                pT = self.bank(ft % 4)
                for ct in range(ND):
                    nc.tensor.transpose(pT[:, _sl(ct)], filt[ct][:, _sl(ft)],
                                        idn[:])
                nc.scalar.copy(filtT[ft][:, 0:512], pT[:])
                nc.vector.tensor_sub(filtT[ft][:, 512:1024], pT[:],
                                     filtT[ft][:, 0:512])

        # --- irfft (ib streamed, 8 banks) -> lp, z2
        pl = [self.bank(tt) for tt in range(NT)]
        if irf_r:
            for ft in range(NT):
                if ft < 2:
                    ibk = ibkpf[ft]
                else:
                    ibk = wk.tile([128, 1024], F32R, tag=f"dftk{ft % 2}",
                                  name="ibk")
                    nc.sync.dma_start(ibk[:], self.d_ibr[_sl(ft), :])
                for tt in range(NT):
                    nc.tensor.matmul(pl[tt][:], ibk[:, _sl(tt)], filtT[ft][:],
                                     start=(ft == 0), stop=(ft == NT - 1))
        else:
            for ft in range(NT):
                if ft < 2:
                    ibk = ibkpf[ft]
                else:
                    ibk = wk.tile([128, 2048], BF16, tag=f"dftk{ft % 2}",
                                  name="ibk")
                    nc.sync.dma_start(ibk[:], self.d_ibhl[_sl(ft), :])
                for tt in range(NT):
                    ibh = ibk[:, _sl(tt)]
                    ibl = ibk[:, 1024 + 128 * tt:1152 + 128 * tt]
                    nc.tensor.matmul(pl[tt][:], ibh, filtT[ft][:, 0:512],
                                     start=(ft == 0), stop=False)
                    nc.tensor.matmul(pl[tt][:], ibh, filtT[ft][:, 512:1024],
                                     start=False, stop=False)
                    nc.tensor.matmul(pl[tt][:], ibl, filtT[ft][:, 0:512],
                                     start=False, stop=(ft == NT - 1))
        lp = [wk.tile([128, D], F32R, tag=f"B3_{tt}", name=f"lp{tt}")
              for tt in range(NT)]
        z2 = [wk.tile([128, D], F32R if mh_r else F32,
                      tag=f"B4_{tt}", name=f"z2_{tt}")
              for tt in range(NT)]
        for tt in range(NT):
            # z2 before lp: in l1 the lp tiles reuse z's memory (tag B3)
            nc.vector.tensor_sub(z2[tt][:], z[tt][:], pl[tt][:])
            nc.scalar.copy(lp[tt][:], pl[tt][:])

        # --- lpT [ND][128, N] (tag A2) + extrap + perT; then free
        lpT = [wk.tile([128, N], F32R, tag=f"A2_{dt}", name=f"lpT{dt}")
               for dt in range(ND)]
        for h in range(2):
            for dt in range(ND):
                pT = self.bank(dt, dtype=F32R)
                for q in range(4):
                    nc.tensor.transpose(pT[:, _sl(q)], lp[h * 4 + q][:, _sl(dt)],
                                        self.idnr[:])
                if h == 0:
                    nc.scalar.copy(lpT[dt][:, _hh(h)], pT[:])
                    nc.vector.tensor_add(aggsl(dt), aggsl(dt),
                                         lpT[dt][:, 0:HOR])
                else:
                    nc.vector.tensor_copy(lpT[dt][:, _hh(h)], pT[:])
        perT = wk.tile([TF, N], F32, tag="dftk0", name="perT")
        for h in range(2):
            pp = self.bank(2 + h)
            for kt in range(ND):
                nc.tensor.matmul(pp[0:TF, :], lay["lw"][:, (ND + kt) * TF:(ND + kt + 1) * TF],
                                 lpT[kt][:, _hh(h)],
                                 start=(kt == 0), stop=(kt == ND - 1))
            nc.scalar.copy(perT[:, _hh(h)], pp[0:TF, :])

        # --- z2T (tag A2 reuse after lpT dead); hl packs bf16 hi|lo
        if mh_r:
            z2T = [wk.tile([128, N], F32R, tag=f"A2_{dt}", name=f"z2T{dt}")
                   for dt in range(ND)]
            for h in range(2):
                for dt in range(ND):
                    pT = self.bank(dt, dtype=F32R)
                    for q in range(4):
                        nc.tensor.transpose(pT[:, _sl(q)],
                                            z2[h * 4 + q][:, _sl(dt)],
                                            self.idnr[:])
                    nc.vector.tensor_copy(z2T[dt][:, _hh(h)], pT[:])
        else:
            z2T = [wk.tile([128, 2 * N], BF16, tag=f"A2_{dt}",
                           name=f"z2Thl{dt}") for dt in range(ND)]
            for h in range(2):
                for dt in range(ND):
                    pT = self.bank(dt)
                    for q in range(4):
                        nc.tensor.transpose(pT[:, _sl(q)],
                                            z2[h * 4 + q][:, _sl(dt)], idn[:])
                    nc.scalar.copy(z2T[dt][:, _hh(h)], pT[:])
                    nc.vector.tensor_sub(
                        z2T[dt][:, N + 512 * h:N + 512 * h + 512], pT[:],
                        z2T[dt][:, _hh(h)])

        # --- win GEMM -> xinT (tag A1 reuse: filt dead)
        xinT = [wk.tile([128, N], F32, tag=f"A1_{dt}", name=f"xinT{dt}")
                for dt in range(ND)]
        for h in range(2):
            for dt in range(ND):
                px = self.bank(4 + dt % 2)
                if mh_r:
                    for kt in range(ND):
                        nc.tensor.matmul(px[:], lay["win"][kt][:, _sl(dt)],
                                         z2T[kt][:, _hh(h)],
                                         start=(kt == 0), stop=(kt == ND - 1))
                else:
                    for kt in range(ND):
                        wh = lay["win"][kt][:, _sl(dt)]
                        wl = lay["win"][kt][:, 512 + 128 * dt:640 + 128 * dt]
                        zh = z2T[kt][:, _hh(h)]
                        zl = z2T[kt][:, N + 512 * h:N + 512 * h + 512]
                        nc.tensor.matmul(px[:], wh, zh,
                                         start=(kt == 0), stop=False)
                        nc.tensor.matmul(px[:], wh, zl,
                                         start=False, stop=False)
                        nc.tensor.matmul(px[:], wl, zh,
                                         start=False, stop=(kt == ND - 1))
                # fold the per-head alpha scale into the psum->sbuf copy
                nc.scalar.activation(xinT[dt][:, _hh(h)], px[:], AF.Identity,
                                     scale=lay["lcol"][:, dt:dt + 1])

        # --- xd -> scan -> sT (tag A2 reuse: z2T dead); alternate DVE/Pool
        # by dt parity so the serial scan chain splits across two engines
        lc = lay["lcol"]
        if mh_r:
            sT = [wk.tile([128, N], F32R, tag=f"A2_{dt}", name=f"sT{dt}")
                  for dt in range(ND)]
            sTsc = sT
        else:
            sTsc = [wk.tile([128, N], F32, tag=f"A1_{dt}", name=f"sTf{dt}")
                    for dt in range(ND)]
            sT = [wk.tile([128, 2 * N], BF16, tag=f"A2_{dt}",
                          name=f"sThl{dt}") for dt in range(ND)]
        for dt in range(ND):
            eng = nc.vector if dt % 2 == 0 else nc.gpsimd
            xd = wk.tile([128, N], F32, tag="xsh" if dt % 2 == 0 else "dftk1",
                         name="xd")
            eng.tensor_sub(xd[:, 1:N], xinT[dt][:, 1:N], xinT[dt][:, 0:N - 1])
            # xinT is pre-scaled by alpha; col 20+dt folds in the scan initial
            nc.vector.tensor_scalar_add(xd[:, 0:1], xinT[dt][:, 0:1],
                                        lc[:, 20 + dt:21 + dt])
            omab_ap = lc[:, 4 + dt:5 + dt].broadcast_to([128, N])
            nc.vector.tensor_tensor_scan(sTsc[dt][:], omab_ap, xd[:], 0.0,
                                         op0=ALU.mult, op1=ALU.add)
            if not mh_r:
                eng.tensor_copy(sT[dt][:, 0:N], sTsc[dt][:])
                eng.tensor_sub(sT[dt][:, N:2 * N], sTsc[dt][:],
                               sT[dt][:, 0:N])

        # --- wout GEMM -> lg [t,d] (tag B2 reuse: filtT dead) (+ z3 if l0)
        lg = [wk.tile([128, D], F32R, tag=f"B2_{tt}", name=f"lg{tt}")
              for tt in range(NT)]
        for tt in range(NT):
            pg = self.bank(tt % 2)
            if mh_r:
                for kt in range(ND):
                    nc.tensor.matmul(pg[:], sT[kt][:, _sl(tt)],
                                     lay["wout"][kt][:],
                                     start=(kt == 0), stop=(kt == ND - 1))
            else:
                for kt in range(ND):
                    sh = sT[kt][:, _sl(tt)]
                    sl_ = sT[kt][:, N + 128 * tt:N + 128 * tt + 128]
                    nc.tensor.matmul(pg[:], sh, lay["wout"][kt][:, 0:512],
                                     start=(kt == 0), stop=False)
                    nc.tensor.matmul(pg[:], sh, lay["wout"][kt][:, 512:1024],
                                     start=False, stop=False)
                    nc.tensor.matmul(pg[:], sl_, lay["wout"][kt][:, 0:512],
                                     start=False, stop=(kt == ND - 1))
            nc.vector.tensor_add(lg[tt][:], pg[:], lay["boutb"][:])
            if not last:
                # z3 overwrites z (tag B1): z dead after z2
                nc.vector.tensor_sub(z[tt][:], z2[tt][:], lg[tt][:])
        z3 = z

        # exact last-growth column for damp (avoids f32r transpose truncation,
        # which the dampening cumsum amplifies)
        lglast = wk.tile([1, D], F32, tag="sqA", name="lglast")
        nc.gpsimd.dma_start(lglast[:], lg[NT - 1][127:128, :])
        lgl4 = wk.tile([128, ND], F32, tag="top8", name="lgl4")
        pTl = self.bank(7, shape=(128, ND))
        for dt in range(ND):
            nc.tensor.matmul(pTl[:, dt:dt + 1], lglast[0:1, _sl(dt)],
                             ones[0:1, 0:1], start=True, stop=True)
        nc.scalar.copy(lgl4[:], pTl[:])

        # --- lgT via transposes (tag A1 reuse: xinT dead)
        lgT = [wk.tile([128, N], F32R, tag=f"A1_{dt}", name=f"lgT{dt}")
               for dt in range(ND)]
        for h in range(2):
            for dt in range(ND):
                pT = self.bank(2 + dt % 2, dtype=F32R)
                for q in range(4):
                    nc.tensor.transpose(pT[:, _sl(q)], lg[h * 4 + q][:, _sl(dt)],
                                        self.idnr[:])
                if h == 0:
                    nc.scalar.copy(lgT[dt][:, _hh(h)], pT[:])
                else:
                    nc.vector.tensor_copy(lgT[dt][:, _hh(h)], pT[:])
        for dt in range(ND):
            # damp: agg += lg_last * csd
            nc.vector.scalar_tensor_tensor(
                aggsl(dt), self.csdt[:, dt * HOR:(dt + 1) * HOR],
                lgl4[:, dt:dt + 1], aggsl(dt), op0=ALU.mult, op1=ALU.add)

        # --- level: grT; scans update xtmid
        grT = wk.tile([TF, N], F32, tag="grT", name="grT")
        for h in range(2):
            pgr = self.bank(6)
            for kt in range(ND):
                nc.tensor.matmul(pgr[0:TF, :], lay["lw"][:, kt * TF:(kt + 1) * TF],
                                 lgT[kt][:, _hh(h)],
                                 start=(kt == 0), stop=(kt == ND - 1))
            # fold level bg bias (lcol col 18) into the psum->sbuf copy
            nc.vector.tensor_scalar_add(grT[:, _hh(h)], pgr[0:TF, :],
                                        lc[0:TF, 18:19])

        xts2 = wk.tile([TF, N], F32, tag="xts", name="xts2")
        if l == 0:
            nc.sync.dma_start(xts2[:], self.d_xT[s * TF:(s + 1) * TF, :])
        else:
            nc.sync.dma_start(xts2[:], self.xtmid[s, :, :])
        v = wk.tile([TF, N], F32, tag="lvv", name="lvv")
        # v = (xts2 - bp) - perT  (fold level bp bias, lcol col 19)
        nc.vector.scalar_tensor_tensor(v[:], xts2[:], lc[0:TF, 19:20], perT[:],
                                       op0=ALU.subtract, op1=ALU.subtract)
        nc.vector.tensor_scalar_mul(v[:], v[:], lc[0:TF, 16:17])
        omlv_ap = lc[0:TF, 17:18].broadcast_to([TF, N])
        pt = wk.tile([TF, N], F32, tag="lvp", name="lvp")
        nc.vector.tensor_tensor_scan(pt[:], omlv_ap, v[:], 0.0,
                                     op0=ALU.mult, op1=ALU.add)
        gt = wk.tile([TF, N], F32, tag="lvv", name="lvg")
        nc.vector.tensor_tensor_scan(gt[:], omlv_ap, grT[:], 0.0,
                                     op0=ALU.mult, op1=ALU.add)
        xnew = wk.tile([TF, N], F32, tag="grT", name="xnew")
        nc.vector.tensor_add(xnew[:], pt[:], gt[:])
        # issue on Pool: keeps this late-blocking store off the SP DMA queue
        nc.gpsimd.dma_start(self.xtmid[s, :, :], xnew[:])

        # --- FF (layer 0 only); z4 stays in SBUF for l1
        if not last:
            return self._ff(s, z3, wk)
        return None

    # ---------- LN stats ----------
    def _ln_stats(self, zset, wk, tagp):
        nc = self.nc
        st = wk.tile([128, 8 * NT], F32, tag=f"st{tagp}", name=f"st{tagp}")
        mu8 = st[:, 0:NT]
        s28 = st[:, NT:2 * NT]
        for tt in range(NT):
            scr = wk.tile([128, D], F32,
                          tag="lnscr" if tt % 2 == 0 else "lnscr2",
                          name="lnscr")
            nc.vector.tensor_reduce(st[:, tt:tt + 1], zset[tt][:],
                                    mybir.AxisListType.X, op=ALU.add)
            nc.scalar.activation(scr[:], zset[tt][:], AF.Square,
                                 accum_out=st[:, NT + tt:NT + tt + 1])
        mun = st[:, 2 * NT:3 * NT]
        nc.vector.tensor_scalar_mul(mun, mu8, 1.0 / D)
        ex2 = st[:, 3 * NT:4 * NT]
        nc.vector.tensor_scalar_mul(ex2, s28, 1.0 / D)
        musq = st[:, 4 * NT:5 * NT]
        nc.scalar.activation(musq, mun, AF.Square)
        var = st[:, 5 * NT:6 * NT]
        nc.vector.tensor_sub(var, ex2, musq)
        sd = st[:, 6 * NT:7 * NT]
        nc.scalar.activation(sd, var, AF.Sqrt, bias=self.epst[:, 0:1])
        rs = st[:, 7 * NT:8 * NT]
        nc.vector.reciprocal(rs, sd)
        nmurs = st[:, 4 * NT:5 * NT]  # overwrite musq slot
        nc.vector.tensor_mul(nmurs, mun, rs)
        nc.vector.tensor_scalar_mul(nmurs, nmurs, -1.0)
        return rs, nmurs

    # ---------- FF block ----------
    def _ff(self, s, z3, wk):
        if PREC["ff"] == "f32r":
            return self._ff_f32r(s, z3, wk)
        return self._ff_hl(s, z3, wk)

    def _ff_f32r(self, s, z3, wk):
        nc = self.nc
        cpk = self.cpk
        rs, nmurs = self._ln_stats(z3, wk, "pre")
        h_ = [wk.tile([128, D], F32R, tag=f"B2_{tt}", name=f"h{tt}")
              for tt in range(NT)]
        for tt in range(NT):
            nc.scalar.activation(h_[tt][:], z3[tt][:], AF.Identity,
                                 scale=rs[:, tt:tt + 1], bias=nmurs[:, tt:tt + 1])
        hT = [wk.tile([128, N], F32R, tag=f"A2_{dt}", name=f"hT{dt}")
              for dt in range(ND)]
        for h in range(2):
            for dt in range(ND):
                pT = self.bank(dt, dtype=F32R)
                for q in range(4):
                    nc.tensor.transpose(pT[:, _sl(q)], h_[h * 4 + q][:, _sl(dt)],
                                        self.idnr[:])
                if h == 0:
                    nc.scalar.copy(hT[dt][:, _hh(h)], pT[:])
                else:
                    nc.vector.tensor_copy(hT[dt][:, _hh(h)], pT[:])
        znT = [wk.tile([128, N], F32R, tag=f"A1_{dt}", name=f"znT{dt}")
               for dt in range(ND)]
        for h in range(2):
            for dt in range(ND):
                nc.scalar.activation(znT[dt][:, _hh(h)], hT[dt][:, _hh(h)],
                                     AF.Identity, scale=cpk[:, dt:dt + 1],
                                     bias=cpk[:, 4 + dt:5 + dt])

        yT = [wk.tile([128, N], F32R, tag=f"A2_{dt}", name=f"yT{dt}")
              for dt in range(ND)]
        for h in range(2):
            pzf = [self.bank(dt) for dt in range(ND)]
            for m in range(NM):
                w1m = wk.tile([128, 512], F32R, tag=f"w1mh{m % 2}", name="w1m")
                nc.sync.dma_start(w1m[:], self.d_ffw1r[m, :, :])
                ph = self.bank(4 + m % 2)
                for kt in range(ND):
                    nc.tensor.matmul(ph[:], w1m[:, _sl(kt)],
                                     znT[kt][:, _hh(h)],
                                     start=(kt == 0), stop=(kt == ND - 1))
                sig = wk.tile([128, 512], F32R, tag=f"sig{m % 2}", name="sig")
                nc.scalar.activation(sig[:], ph[:], AF.Sigmoid,
                                     bias=cpk[:, 8 + m:9 + m])
                w2m = wk.tile([128, 512], F32R, tag=f"w2m{m % 2}", name="w2m")
                nc.sync.dma_start(w2m[:], self.d_ffw2r[_sl(m), :])
                for dt in range(ND):
                    nc.tensor.matmul(pzf[dt][:], w2m[:, _sl(dt)], sig[:],
                                     start=(m == 0), stop=(m == NM - 1))
            for dt in range(ND):
                nc.vector.scalar_tensor_tensor(yT[dt][:, _hh(h)], pzf[dt][:],
                                               cpk[:, 24 + dt:25 + dt],
                                               znT[dt][:, _hh(h)],
                                               op0=ALU.add, op1=ALU.add)
        return self._post_ln(s, yT, wk, yr=True)

    def _ff_hl(self, s, z3, wk):
        nc = self.nc
        idn = self.idn
        cpk = self.cpk
        rs, nmurs = self._ln_stats(z3, wk, "pre")
        h_ = [wk.tile([128, D], F32, tag=f"B2_{tt}", name=f"h{tt}")
              for tt in range(NT)]
        for tt in range(NT):
            nc.scalar.activation(h_[tt][:], z3[tt][:], AF.Identity,
                                 scale=rs[:, tt:tt + 1], bias=nmurs[:, tt:tt + 1])
        hT = [wk.tile([128, N], F32, tag=f"A2_{dt}", name=f"hT{dt}")
              for dt in range(ND)]
        for h in range(2):
            for dt in range(ND):
                pT = self.bank(dt)
                for q in range(4):
                    nc.tensor.transpose(pT[:, _sl(q)], h_[h * 4 + q][:, _sl(dt)],
                                        idn[:])
                if h == 0:
                    nc.scalar.copy(hT[dt][:, _hh(h)], pT[:])
                else:
                    nc.vector.tensor_copy(hT[dt][:, _hh(h)], pT[:])
        znT = [wk.tile([128, N], F32, tag=f"A1_{dt}", name=f"znT{dt}")
               for dt in range(ND)]
        for h in range(2):
            for dt in range(ND):
                nc.scalar.activation(znT[dt][:, _hh(h)], hT[dt][:, _hh(h)],
                                     AF.Identity, scale=cpk[:, dt:dt + 1],
                                     bias=cpk[:, 4 + dt:5 + dt])

        yT = [wk.tile([128, N], F32, tag=f"A2_{dt}", name=f"yT{dt}")
              for dt in range(ND)]
        for h in range(2):
            znb = [wk.tile([128, 1024], BF16, tag=f"B3_{kt}", name=f"znb{kt}")
                   for kt in range(ND)]
            for kt in range(ND):
                nc.vector.tensor_copy(znb[kt][:, 0:512], znT[kt][:, _hh(h)])
                nc.vector.tensor_sub(znb[kt][:, 512:1024], znT[kt][:, _hh(h)],
                                     znb[kt][:, 0:512])
            pzf = [self.bank(dt) for dt in range(ND)]
            for m in range(NM):
                w1m = wk.tile([128, 2 * ND * 128], BF16,
                              tag=f"w1mh{m % 2}", name="w1m")
                nc.sync.dma_start(w1m[:], self.d_ffw1t[m, :, :])
                ph = self.bank(4 + m % 2)
                for kt in range(ND):
                    nc.tensor.matmul(ph[:], w1m[:, _sl(kt)], znb[kt][:, 0:512],
                                     start=(kt == 0), stop=False)
                    nc.tensor.matmul(ph[:], w1m[:, _sl(kt)], znb[kt][:, 512:1024],
                                     start=False, stop=False)
                    nc.tensor.matmul(ph[:], w1m[:, 512 + 128 * kt:640 + 128 * kt],
                                     znb[kt][:, 0:512],
                                     start=False, stop=(kt == ND - 1))
                sig = wk.tile([128, 512], F32, tag=f"sig{m % 2}", name="sig")
                nc.scalar.activation(sig[:], ph[:], AF.Sigmoid,
                                     bias=cpk[:, 8 + m:9 + m])
                sighl = wk.tile([128, 1024], BF16,
                                tag="amp2" if m % 2 == 0 else "lnscr",
                                name="sighl")
                nc.vector.tensor_copy(sighl[:, 0:512], sig[:])
                nc.vector.tensor_sub(sighl[:, 512:1024], sig[:],
                                     sighl[:, 0:512])
                w2m = wk.tile([128, 1024], BF16, tag=f"w2m{m % 2}", name="w2m")
                nc.sync.dma_start(w2m[:], self.d_ffw2hl[_sl(m), :])
                for dt in range(ND):
                    nc.tensor.matmul(pzf[dt][:], w2m[:, _sl(dt)],
                                     sighl[:, 0:512],
                                     start=(m == 0), stop=False)
                    nc.tensor.matmul(pzf[dt][:], w2m[:, _sl(dt)],
                                     sighl[:, 512:1024],
                                     start=False, stop=False)
                    nc.tensor.matmul(pzf[dt][:], w2m[:, 512 + dt * 128:
                                                     640 + dt * 128],
                                     sighl[:, 0:512],
                                     start=False, stop=(m == NM - 1))
            for dt in range(ND):
                nc.vector.scalar_tensor_tensor(yT[dt][:, _hh(h)], pzf[dt][:],
                                               cpk[:, 24 + dt:25 + dt],
                                               znT[dt][:, _hh(h)],
                                               op0=ALU.add, op1=ALU.add)
        return self._post_ln(s, yT, wk, yr=False)

    def _post_ln(self, s, yT, wk, yr):
        # fully per-tt post-LN chains: z4[0] is ready before the last yT
        # transposes finish, so l1's rfft starts with no barrier on the
        # batched stats
        nc = self.nc
        idn = self.idn
        rfr = PREC["rfft1"] == "f32r"
        gb, bb = self.gbt, self.bbt
        z4 = [wk.tile([128, D], F32R, tag=f"B3_{tt}", name=f"z4_{tt}")
              for tt in range(NT)]
        if not rfr:
            zhl = [wk.tile([128, 1024], BF16, tag=f"B2_{tt}", name=f"zhl{tt}")
                   for tt in range(NT)]
        st = wk.tile([128, 8 * NT], F32, tag="stpost", name="stpost")
        for tt in range(NT):
            pT = self.bank(6 + tt % 2, dtype=F32R if yr else F32)
            for dt in range(ND):
                nc.tensor.transpose(pT[:, _sl(dt)], yT[dt][:, _sl(tt)],
                                    self.idnr[:] if yr else idn[:])
            y_t = wk.tile([128, D], F32, tag=f"B4_{tt}", name=f"y{tt}")
            nc.scalar.copy(y_t[:], pT[:])
            scr = wk.tile([128, D], F32,
                          tag="lnscr" if tt % 2 == 0 else "lnscr2",
                          name="lnscr")
            mu = st[:, tt:tt + 1]
            s2 = st[:, NT + tt:NT + tt + 1]
            nc.vector.tensor_reduce(mu, y_t[:], mybir.AxisListType.X,
                                    op=ALU.add)
            nc.scalar.activation(scr[:], y_t[:], AF.Square, accum_out=s2)
            mun = st[:, 2 * NT + tt:2 * NT + tt + 1]
            nc.vector.tensor_scalar_mul(mun, mu, 1.0 / D)
            musq = st[:, 3 * NT + tt:3 * NT + tt + 1]
            nc.scalar.activation(musq, mun, AF.Square)
            var = st[:, 4 * NT + tt:4 * NT + tt + 1]
            nc.vector.scalar_tensor_tensor(var, s2, 1.0 / D, musq,
                                           op0=ALU.mult, op1=ALU.subtract)
            sd = st[:, 5 * NT + tt:5 * NT + tt + 1]
            nc.scalar.activation(sd, var, AF.Sqrt, bias=self.epst[:, 0:1])
            rs = st[:, 6 * NT + tt:6 * NT + tt + 1]
            nc.vector.reciprocal(rs, sd)
            nmurs = st[:, 7 * NT + tt:7 * NT + tt + 1]
            nc.vector.scalar_tensor_tensor(nmurs, mun, -1.0, rs,
                                           op0=ALU.mult, op1=ALU.mult)
            nc.scalar.activation(scr[:], y_t[:], AF.Identity,
                                 scale=rs, bias=nmurs)
            nc.vector.tensor_mul(z4[tt][:], scr[:], gb[:])
            nc.vector.tensor_add(z4[tt][:], z4[tt][:], bb[:])
            if not rfr:
                nc.gpsimd.tensor_copy(zhl[tt][:, 0:512], z4[tt][:])
                nc.gpsimd.tensor_sub(zhl[tt][:, 512:1024], z4[tt][:],
                                     zhl[tt][:, 0:512])
        if rfr:
            return z4, None
        return z4, zhl

    # ---------- output head ----------
    def _output(self, s, wk):
        nc = self.nc
        ones = self.ones
        po = self.bank(7)
        for kt in range(ND):
            nc.tensor.matmul(po[0:TF, 0:HOR], self.outwt[:, kt * TF:(kt + 1) * TF],
                             self.aggt[:, kt * HOR:(kt + 1) * HOR],
                             start=(kt == 0), stop=False)
        nc.tensor.matmul(po[0:TF, 0:HOR], self.outbt[0:1, 0:TF],
                         ones[0:1, 0:HOR], start=False, stop=True)
        xfin = wk.tile([TF, N], F32, tag="lvp", name="xfin")
        nc.gpsimd.dma_start(xfin[:], self.xtmid[s, :, :])
        oT = wk.tile([TF, HOR], F32, tag="lvv", name="oT")
        nc.vector.tensor_scalar_add(oT[:], po[0:TF, 0:HOR], xfin[:, N - 1:N])
        nc.gpsimd.dma_start(self.d_out[s * TF:(s + 1) * TF, :], oT[:])


def _get_nc():
    if "nc" not in _CACHE:
        _CACHE["nc"] = K().build()
    return _CACHE["nc"]


def _common_maps(inputs, w2d, dft, ib, e8):
    m = dict(
        w2d=_rne11(w2d) if PREC["l0head"] == "f32r" else w2d,
        ones1=np.ones((1, N), np.float32),
        idn=np.eye(128, dtype=np.float32),
        e8=e8,
        ibr=_rne11(ib),
        winr=_rne11(np.asarray(inputs["mhesa_win"], np.float32)),
        woutr=_rne11(np.asarray(inputs["mhesa_wout"], np.float32)),
        boutr=np.asarray(inputs["mhesa_bout"], np.float32).reshape(L, 1, D),
        lcolp=_pack_lcol(inputs),
        alpha8=np.asarray(inputs["mhesa_alpha"], np.float32).reshape(L, HEADS, 1),
        cpkp=_pack_cpk(inputs),
        gpostr=np.asarray(inputs["ff_post_g"], np.float32).reshape(1, D),
        bpostr=np.asarray(inputs["ff_post_b"], np.float32).reshape(1, D),
        lvwg=np.asarray(inputs["level_wg"], np.float32),
        lvwp=np.asarray(inputs["level_wp"], np.float32),
        lvbg=np.asarray(inputs["level_bg"], np.float32).reshape(L, 1, TF),
        lvbp=np.asarray(inputs["level_bp"], np.float32).reshape(L, 1, TF),
        lvalpha=np.asarray(inputs["level_alpha"], np.float32).reshape(L, 1, 1),
        damp8=np.asarray(inputs["dampen_factor"], np.float32).reshape(HEADS, 1),
        outw=np.asarray(inputs["out_w"], np.float32)
            .reshape(ND, 128, TF).transpose(1, 0, 2).reshape(128, ND * TF)
            .copy(),
        outbr=np.asarray(inputs["out_b"], np.float32).reshape(1, TF),
    )
    if PREC["l0head"] == "f32":
        m["dft"] = dft
    if PREC["l0head"] == "f32r" or PREC["rfft1"] == "f32r":
        m["dftr"] = _rne11(dft)
    if PREC["rfft1"] == "hl":
        m["dfthl"] = np.concatenate([_split_hi(dft), _split_lo(dft)], axis=1)
    if PREC["irfft0"] == "hl":
        m["ibhl"] = np.concatenate([_split_hi(ib), _split_lo(ib)], axis=1)
    if PREC["mhesa0"] == "hl":
        win0 = np.asarray(inputs["mhesa_win"][0], np.float32)
        wout0 = np.asarray(inputs["mhesa_wout"][0], np.float32)
        m["winhl"] = np.concatenate([_split_hi(win0), _split_lo(win0)], axis=1)
        m["wouthl"] = np.concatenate([_split_hi(wout0), _split_lo(wout0)],
                                     axis=1)
    w1 = np.asarray(inputs["ff_w1"], np.float32)
    w2 = np.asarray(inputs["ff_w2"], np.float32)
    if PREC["ff"] == "f32r":
        m["ffw1r"] = _rne11(_pack_w1r(w1))
        m["ffw2r"] = _rne11(w2)
    else:
        m["ffw1t"] = _pack_w1(w1)
        m["ffw2hl"] = np.concatenate([_split_hi(w2), _split_lo(w2)], axis=1)
    return m


def _kernel_impl(inputs, runner):
    x = np.asarray(inputs["x"], np.float32)
    assert (x.shape[0], x.shape[1], x.shape[2]) == (32, N, TF)
    assert int(inputs["forecast_horizon"]) == HOR
    dft, ib = _dft_consts()
    conv_w = np.asarray(inputs["conv_w"], np.float32)
    w2d = _build_w2d(conv_w, np.asarray(inputs["conv_b"], np.float32))
    e8 = np.repeat(np.eye(HEADS, dtype=np.float32), DH, axis=1)
    nc = _get_nc()
    common = _common_maps(inputs, w2d, dft, ib, e8)
    in_maps = []
    for c in range(NCORES):
        xs = x[c * S:(c + 1) * S]
        xT = xs.transpose(0, 2, 1).reshape(S * TF, N).copy()
        in_maps.append(dict(common, xT=xT))
    res = runner(nc, in_maps)
    out = np.zeros((x.shape[0], HOR, TF), np.float32)
    for c in range(NCORES):
        oT = res.results[c]["outT"].reshape(S, TF, HOR)
        out[c * S:(c + 1) * S] = oT.transpose(0, 2, 1)
    return out, res


def kernel(**inputs):
    out, _ = _kernel_impl(
        inputs,
        lambda nc, im: run_bass_kernel_spmd(nc, im, list(range(NCORES))))
    return out


def kernel_traced(**inputs):
    """Like kernel() but with NTFF profiling; returns (out, BassKernelResults)."""
    return _kernel_impl(
        inputs,
        lambda nc, im: run_bass_kernel_spmd(nc, im, list(range(NCORES)),
                                            trace=True))


# revision 26
# speedup vs baseline: 1.5276x; 1.0440x over previous
"""ETSFormer forward pass on 8 Trainium2 NeuronCores (Bass/Tile).

Data-parallel over batch: 32 samples -> 8 cores x 4 samples, weights
replicated, no collectives. The reference's FFT machinery is computed
exactly without FFTs:
  - freq_attention: dense DFT matmuls + hardware top-8 (vector.max) mask
  - mhesa / level exponential smoothing: the reference FFT cross-correlation
    is exactly a first-order EMA -> hardware prefix scan (tensor_tensor_scan)
  - fourier_extrapolate: exact slice (Dirichlet kernel identity)

Precision: PREC selects per-GEMM-group dtype. "f32r" = fp32-reduced
(FP22 truncated, 1 cyc/row on PE -- same speed as bf16) vs the fallback
"hl" = bf16 hi/lo 3-term split (~2^-16, 3 cyc/row) / "f32" = true fp32
(4 cyc/row). The top-4 frequency mask is rank-sensitive; flags are
tuned empirically against the end-to-end error gate.
"""
import numpy as np
from contextlib import ExitStack

import concourse.bass as bass
import concourse.bacc as bacc
import concourse.tile as tile
from concourse import mybir
from concourse.bass_utils import run_bass_kernel_spmd

F32 = mybir.dt.float32
F32R = mybir.dt.float32r
BF16 = mybir.dt.bfloat16
AF = mybir.ActivationFunctionType
ALU = mybir.AluOpType

N = 1024
D = 512
TF = 7
HEADS = 8
DH = D // HEADS
L = 2
S = 4
NCORES = 8
HOR = 96
FD = 2048
NT = N // 128   # 8
ND = D // 128   # 4
NM = FD // 128  # 16

_CACHE = {}

# per-stage precision: "f32r" fast path vs baseline "hl" (bf16 3-term)
# / "f32" (true fp32) fallback.
PREC = dict(
    l0head="f32",    # conv z GEMM + low-rank DFT: feeds the layer-0 top-4
                     # ranking, which flips even under 2^-12 weight rounding
                     # (emulation: 52 flips, 2.7e-2 err) -- keep exact fp32
    irfft0="f32r",   # layer-0 irfft (feeds layer-1 ranking path)
    mhesa0="f32r",   # layer-0 win/wout GEMMs
    ff="f32r",       # FF block w1/w2 GEMMs
    rfft1="f32r",    # layer-1 rfft (feeds layer-1 ranking directly)
)


def _rne11(x):
    # round fp32 mantissa to 11 explicit bits (fp22): the PE's f32r mode
    # truncates operands to fp22, so pre-rounded weights pass through
    # losslessly -- halves f32r noise and removes the truncation bias
    xi = np.ascontiguousarray(np.asarray(x, np.float32)).view(np.uint32)
    return ((xi + np.uint32(0x800)) & np.uint32(0xFFFFF000)).view(np.float32)


def _dft_consts():
    if "dft" not in _CACHE:
        t = np.arange(N)
        f = np.arange(513)
        ang = 2.0 * np.pi * np.outer(t, f) / N
        cos = np.cos(ang)
        sin = np.sin(ang)
        dft = np.zeros((N, 1024), np.float64)
        dft[:, 0:512] = cos[:, 0:512]
        dft[:, 512] = cos[:, 512]
        dft[:, 513:1024] = sin[:, 1:512]
        c = np.full(513, 2.0)
        c[0] = 1.0
        c[512] = 1.0
        ib = np.zeros((1024, N), np.float64)
        ib[0:512, :] = (c[0:512, None] / N) * cos[:, 0:512].T
        ib[512, :] = (1.0 / N) * cos[:, 512]
        ib[513:1024, :] = (2.0 / N) * sin[:, 1:512].T
        _CACHE["dft"] = dft.astype(np.float32)
        _CACHE["ib"] = ib.astype(np.float32)
    return _CACHE["dft"], _CACHE["ib"]


def _sl(i, w=128):
    return slice(i * w, (i + 1) * w)


def _split_hi(x):
    import ml_dtypes
    return x.astype(ml_dtypes.bfloat16)


def _split_lo(x):
    import ml_dtypes
    hi = x.astype(ml_dtypes.bfloat16).astype(np.float32)
    return (x - hi).astype(ml_dtypes.bfloat16)


def _pack_w1(w1):
    # bf16 hi|lo tiles for the "hl" fallback FF path
    hi, lo = _split_hi(w1), _split_lo(w1)
    out = np.zeros((NM, 128, 1024), hi.dtype)
    for m in range(NM):
        for kt in range(ND):
            out[m, :, 128 * kt:128 * (kt + 1)] = hi[_sl(kt), _sl(m)]
            out[m, :, 512 + 128 * kt:640 + 128 * kt] = lo[_sl(kt), _sl(m)]
    return out


def _pack_w1r(w1):
    # f32r per-m contiguous [128(k), 4x128(m)] tiles
    out = np.zeros((NM, 128, 512), np.float32)
    for m in range(NM):
        for kt in range(ND):
            out[m, :, 128 * kt:128 * (kt + 1)] = w1[_sl(kt), _sl(m)]
    return out


def _pack_cpk(inputs):
    # cols: gpre(4) | bpre(4) | ffb1(16) | ffb2(4), each D/FD vector folded
    # into [128, k] column blocks -- one DMA instead of 24
    out = np.zeros((128, 28), np.float32)
    out[:, 0:4] = np.asarray(inputs["ff_pre_g"], np.float32).reshape(4, 128).T
    out[:, 4:8] = np.asarray(inputs["ff_pre_b"], np.float32).reshape(4, 128).T
    out[:, 8:24] = np.asarray(inputs["ff_b1"], np.float32).reshape(16, 128).T
    out[:, 24:28] = np.asarray(inputs["ff_b2"], np.float32).reshape(4, 128).T
    return out


def _pack_lcol(inputs):
    # per layer: init(4 cols) | bin(4 cols)
    out = np.zeros((L, 128, 8), np.float32)
    ini = np.asarray(inputs["mhesa_init"], np.float32).reshape(L, D)
    bi = np.asarray(inputs["mhesa_bin"], np.float32)
    for l in range(L):
        out[l, :, 0:4] = ini[l].reshape(4, 128).T
        out[l, :, 4:8] = bi[l].reshape(4, 128).T
    return out


def _build_w2d(conv_w, conv_b):
    # rows 32k+c hold conv_w[:, c, k] (32-aligned partition groups so the
    # on-device shifted copies keep legal base partitions); row 95 is the
    # bias row, paired with an all-ones row 95 of xsh on device.
    w2d = np.zeros((96, D), np.float32)
    for k in range(3):
        for c in range(TF):
            w2d[32 * k + c] = conv_w[:, c, k]
    w2d[95] = conv_b
    return w2d


def _hh(h):
    return slice(h * 512, (h + 1) * 512)


class K:
    def __init__(self):
        nc = bacc.Bacc()
        self.nc = nc
        p = nc.declare_dram_parameter
        self.d_xT = p("xT", [S * TF, N], F32, isOutput=False)
        self.d_w2d = p("w2d", [96, D],
                       F32R if PREC["l0head"] == "f32r" else F32,
                       isOutput=False)
        self.d_ones1 = p("ones1", [1, N], F32, isOutput=False)
        if PREC["l0head"] == "f32":
            self.d_dft = p("dft", [N, 1024], F32, isOutput=False)
        if PREC["l0head"] == "f32r" or PREC["rfft1"] == "f32r":
            self.d_dftr = p("dftr", [N, 1024], F32R, isOutput=False)
        if PREC["rfft1"] == "hl":
            self.d_dfthl = p("dfthl", [N, 2048], BF16, isOutput=False)
        self.d_ibr = p("ibr", [1024, N], F32R, isOutput=False)
        self.d_winr = p("winr", [L, D, D], F32R, isOutput=False)
        self.d_woutr = p("woutr", [L, D, D], F32R, isOutput=False)
        if PREC["irfft0"] == "hl":
            self.d_ibhl = p("ibhl", [1024, 2048], BF16, isOutput=False)
        if PREC["mhesa0"] == "hl":
            self.d_winhl = p("winhl", [D, 2 * D], BF16, isOutput=False)
            self.d_wouthl = p("wouthl", [D, 2 * D], BF16, isOutput=False)
        self.d_idn = p("idn", [128, 128], F32, isOutput=False)
        self.d_e8 = p("e8", [HEADS, D], F32, isOutput=False)
        self.d_bout = p("boutr", [L, 1, D], F32, isOutput=False)
        self.d_lcolp = p("lcolp", [L, 128, 8], F32, isOutput=False)
        self.d_al8 = p("alpha8", [L, HEADS, 1], F32, isOutput=False)
        if PREC["ff"] == "f32r":
            self.d_ffw1r = p("ffw1r", [NM, 128, 512], F32R, isOutput=False)
            self.d_ffw2r = p("ffw2r", [FD, D], F32R, isOutput=False)
        else:
            self.d_ffw1t = p("ffw1t", [NM, 128, 2 * ND * 128], BF16,
                             isOutput=False)
            self.d_ffw2hl = p("ffw2hl", [FD, 2 * D], BF16, isOutput=False)
        self.d_cpkp = p("cpkp", [128, 28], F32, isOutput=False)
        self.d_gpost = p("gpostr", [1, D], F32, isOutput=False)
        self.d_bpost = p("bpostr", [1, D], F32, isOutput=False)
        self.d_wg = p("lvwg", [L, D, TF], F32, isOutput=False)
        self.d_wp = p("lvwp", [L, D, TF], F32, isOutput=False)
        self.d_bg = p("lvbg", [L, 1, TF], F32, isOutput=False)
        self.d_bp = p("lvbp", [L, 1, TF], F32, isOutput=False)
        self.d_alv = p("lvalpha", [L, 1, 1], F32, isOutput=False)
        self.d_damp = p("damp8", [HEADS, 1], F32, isOutput=False)
        self.d_outw = p("outw", [128, ND * TF], F32, isOutput=False)
        self.d_outb = p("outbr", [1, TF], F32, isOutput=False)
        self.d_out = p("outT", [S * TF, HOR], F32, isOutput=True)
        self.xtmid = nc.dram_tensor("xtmid", [S, TF, N], F32)

    # psum bank helper: tag-based reuse of the 8 banks
    def bank(self, i, shape=(128, 512), dtype=F32):
        tl = self.psp.tile(list(shape), dtype, tag=f"bk{i}", name=f"bk{i}")
        return tl

    def build(self):
        nc = self.nc
        with ExitStack() as ctx:
            self.tc = ctx.enter_context(tile.TileContext(nc))
            tc = self.tc
            top = ctx.enter_context(tc.tile_pool(name="top", bufs=1))

            idn = top.tile([128, 128], F32, name="idn")
            nc.sync.dma_start(idn[:], self.d_idn[:])
            idnr = top.tile([128, 128], F32R, name="idnr")
            nc.vector.tensor_copy(idnr[:], idn[:])
            self.idnr = idnr
            ones = top.tile([128, 128], F32, name="ones")
            nc.vector.memset(ones[:], 1.0)
            w2d = top.tile([96, D],
                           F32R if PREC["l0head"] == "f32r" else F32,
                           name="w2d")
            nc.sync.dma_start(w2d[:], self.d_w2d[:])
            outbr = top.tile([1, TF], F32, name="outbr")
            nc.sync.dma_start(outbr[:], self.d_outb[:])
            self.outbt = outbr
            # col pack: gpre(4) | bpre(4)
            cpk = top.tile([128, 28], F32, name="cpk")
            nc.sync.dma_start(cpk[:], self.d_cpkp[:])
            outw = top.tile([128, ND * TF], F32, name="outw")
            nc.sync.dma_start(outw[:], self.d_outw[:])
            eps = top.tile([128, 1], F32, name="eps")
            nc.vector.memset(eps[:], 1e-5)
            self.epst = eps
            gbt = top.tile([128, D], F32, name="gbt")
            bbt = top.tile([128, D], F32, name="bbt")
            self.gbt, self.bbt = gbt, bbt
            agg = top.tile([128, ND * HOR], F32, name="agg")
            csd = top.tile([128, ND * HOR], F32, name="csd")

            self.idn, self.ones, self.cpk = idn, ones, cpk
            self.w2dt_, self.aggt, self.csdt = w2d, agg, csd
            self.outwt = outw

            self.psp = ctx.enter_context(
                tc.tile_pool(name="ps", bufs=1, space="PSUM"))
            # both layers' constants resident; samples run L0->L1 back to
            # back so L1's DVE-heavy tail overlaps the next sample's
            # PE-heavy head, and z4 never round-trips through DRAM
            lay0p = ctx.enter_context(tc.tile_pool(name="lay0", bufs=1))
            lay1p = ctx.enter_context(tc.tile_pool(name="lay1", bufs=1))
            with tc.tile_pool(name="ini", bufs=1) as ini:
                e8 = ini.tile([HEADS, D], F32, name="e8")
                nc.sync.dma_start(e8[:], self.d_e8[:])
                self.e8t = e8
                self._damp_cs(ini, self.psp)
                lay = [self._layer_consts(0, lay0p),
                       self._layer_consts(1, lay1p)]
            wk = ctx.enter_context(tc.tile_pool(name="wk", bufs=1))
            for s in range(S):
                z4 = self._sample(0, s, lay[0], wk)
                self._sample(1, s, lay[1], wk, zin=z4)
                self._output(s, wk)

        nc.compile()
        return nc

    # ---------- dampening cumsum -> csd [128, ND*HOR] ----------
    def _damp_cs(self, ini, inips):
        nc = self.nc
        ones = self.ones
        dcol = ini.tile([HEADS, 1], F32, name="dcol")
        nc.sync.dma_start(dcol[:], self.d_damp[:])
        df = ini.tile([HEADS, 1], F32, name="dfsig")
        nc.scalar.activation(df[:], dcol[:], AF.Sigmoid)
        dfb = ini.tile([HEADS, HOR], F32, name="dfb")
        nc.scalar.activation(dfb[:], ones[0:HEADS, 0:HOR], AF.Identity,
                             scale=df[:, 0:1])
        zer = ini.tile([HEADS, HOR], F32, name="zer8")
        nc.vector.memset(zer[:], 0.0)
        dfp = ini.tile([HEADS, HOR], F32, name="dfp")
        nc.vector.tensor_tensor_scan(dfp[:], dfb[:], zer[:], 1.0,
                                     op0=ALU.mult, op1=ALU.add)
        cs8 = ini.tile([HEADS, HOR], F32, name="cs8")
        nc.vector.tensor_tensor_scan(cs8[:], ones[0:HEADS, 0:HOR], dfp[:], 0.0,
                                     op0=ALU.mult, op1=ALU.add)
        for dt in range(ND):
            pini = inips.tile([128, HOR], F32, tag=f"bk{dt}", name="pini")
            nc.tensor.matmul(pini[:], self.e8t[:, _sl(dt)], cs8[:],
                             start=True, stop=True)
            nc.scalar.copy(self.csdt[:, dt * HOR:(dt + 1) * HOR], pini[:])
        # hoisted FF post-LN gamma/beta broadcasts (layer-invariant)
        rows = ini.tile([1, 1024], F32, name="rows")
        nc.sync.dma_start(rows[0:1, 0:512], self.d_gpost[:])
        nc.sync.dma_start(rows[0:1, 512:1024], self.d_bpost[:])
        pgb = inips.tile([128, D], F32, tag="bk4", name="pgb")
        nc.tensor.matmul(pgb[:], self.ones[0:1, 0:128],
                         rows[0:1, 0:512], start=True, stop=True)
        nc.scalar.copy(self.gbt[:], pgb[:])
        pbb = inips.tile([128, D], F32, tag="bk5", name="pbb")
        nc.tensor.matmul(pbb[:], self.ones[0:1, 0:128],
                         rows[0:1, 512:1024], start=True, stop=True)
        nc.scalar.copy(self.bbt[:], pbb[:])

    # ---------- per-layer constants ----------
    def _layer_consts(self, l, layp):
        nc = self.nc
        ones = self.ones
        last = l == L - 1
        lay = {"l": l, "last": last}

        if last or PREC["mhesa0"] == "f32r":
            win = [layp.tile([128, D], F32R, name=f"win{k}") for k in range(ND)]
            wout = [layp.tile([128, D], F32R, name=f"wout{k}")
                    for k in range(ND)]
            for kt in range(ND):
                nc.scalar.dma_start(win[kt][:], self.d_winr[l, _sl(kt), :])
                nc.scalar.dma_start(wout[kt][:], self.d_woutr[l, _sl(kt), :])
        else:
            # bf16 hi|lo packed (cols 0:512 hi, 512:1024 lo)
            win = [layp.tile([128, 2 * D], BF16, name=f"win{k}")
                   for k in range(ND)]
            wout = [layp.tile([128, 2 * D], BF16, name=f"wout{k}")
                    for k in range(ND)]
            for kt in range(ND):
                nc.scalar.dma_start(win[kt][:], self.d_winhl[_sl(kt), :])
                nc.scalar.dma_start(wout[kt][:], self.d_wouthl[_sl(kt), :])

        # lrows: p0 = bout[512]; p32 = bg[7] then bp at cols 16..23
        lrows = layp.tile([128, 512], F32, name="lrows")
        nc.sync.dma_start(lrows[0:1, 0:D], self.d_bout[l, :, :])
        nc.sync.dma_start(lrows[32:33, 0:TF], self.d_bg[l, :, :])
        nc.sync.dma_start(lrows[32:33, 16:16 + TF], self.d_bp[l, :, :])

        # bout broadcast [128, D] (replaces per-tile bias matmuls)
        boutb = layp.tile([128, D], F32, name="boutb")
        pbo = self.psp.tile([128, D], F32, tag="bk7", name="pbo")
        nc.tensor.matmul(pbo[:], ones[0:1, 0:128], lrows[0:1, 0:D],
                         start=True, stop=True)
        nc.scalar.copy(boutb[:], pbo[:])

        # lcol pack [128, 16]: al(4) oma(4) init(4) bi(4); plus lv cols [7,1]
        # cols 18/19: level bg/bp as [7,1] columns
        lcol = layp.tile([128, 24], F32, name="lcol")
        nc.sync.dma_start(lcol[0:TF, 18:19],
                          self.d_bg[l, :, :].rearrange("a b -> b a"))
        nc.sync.dma_start(lcol[0:TF, 19:20],
                          self.d_bp[l, :, :].rearrange("a b -> b a"))
        al8 = layp.tile([HEADS, 1], F32, tag="al8t", name="al8")
        nc.sync.dma_start(al8[:], self.d_al8[l, :, :])
        al8s = layp.tile([HEADS, 1], F32, tag="al8s", name="al8s")
        nc.scalar.activation(al8s[:], al8[:], AF.Sigmoid)
        for dt in range(ND):
            pal = self.psp.tile([128, 1], F32, tag="bk0", name="pal")
            nc.tensor.matmul(pal[:], self.e8t[:, _sl(dt)], al8s[:],
                             start=True, stop=True)
            nc.scalar.copy(lcol[:, dt:dt + 1], pal[:])
        nc.sync.dma_start(lcol[:, 8:16], self.d_lcolp[l, :, :])
        for dt in range(ND):
            nc.vector.tensor_scalar(lcol[:, 4 + dt:5 + dt], lcol[:, dt:dt + 1],
                                    -1.0, 1.0, op0=ALU.mult, op1=ALU.add)
        nc.vector.tensor_sub(lcol[:, 12:16], lcol[:, 12:16], lcol[:, 8:12])
        # col 20:24 = al*(bi-init) + (1-al)*init -- the scan-initial folded
        # into xd[0] so the scan can run with a 0.0 immediate initial
        bi = layp.tile([128, ND], F32, tag="bitmp", name="bitmp")
        nc.vector.tensor_mul(lcol[:, 20:24], lcol[:, 0:4], lcol[:, 12:16])
        nc.vector.tensor_mul(bi[:], lcol[:, 4:8], lcol[:, 8:12])
        nc.vector.tensor_add(lcol[:, 20:24], lcol[:, 20:24], bi[:])
        # level alpha
        alv = layp.tile([1, 1], F32, tag="alvt", name="alv")
        nc.sync.dma_start(alv[:], self.d_alv[l, :, :])
        alvs = layp.tile([1, 1], F32, tag="alvst", name="alvs")
        nc.scalar.activation(alvs[:], alv[:], AF.Sigmoid)
        pv = self.psp.tile([TF, 1], F32, tag="bk1", name="palv")
        nc.tensor.matmul(pv[:], ones[0:1, 0:TF], alvs[:], start=True, stop=True)
        nc.scalar.copy(lcol[0:TF, 16:17], pv[:])
        nc.vector.tensor_scalar(lcol[0:TF, 17:18], lcol[0:TF, 16:17], -1.0, 1.0,
                                op0=ALU.mult, op1=ALU.add)

        # level weights [128, TF] x4 packed [128, 2*ND*TF], as fp32r
        lwf = layp.tile([128, 2 * ND * TF], F32, tag="lwf", name="lwf")
        for kt in range(ND):
            nc.sync.dma_start(lwf[:, kt * TF:(kt + 1) * TF], self.d_wg[l, _sl(kt), :])
            nc.sync.dma_start(lwf[:, (ND + kt) * TF:(ND + kt + 1) * TF],
                              self.d_wp[l, _sl(kt), :])
        lw = layp.tile([128, 2 * ND * TF], F32R, name="lw")
        nc.vector.tensor_copy(lw[:], lwf[:])

        lay.update(win=win, wout=wout, lrows=lrows, lcol=lcol, lw=lw,
                   boutb=boutb)
        return lay

    # ---------- one sample through one layer ----------
    def _sample(self, l, s, lay, wk, zin=None):
        nc = self.nc
        ones, idn = self.ones, self.idn
        last = lay["last"]
        agg = self.aggt
        irf_r = last or PREC["irfft0"] == "f32r"
        mh_r = last or PREC["mhesa0"] == "f32r"

        def aggsl(dt):
            return self.aggt[:, dt * HOR:(dt + 1) * HOR]

        # --- z input: conv (l0) or handed over in SBUF from l0 (l1)
        if l == 0:
            hr = PREC["l0head"] == "f32r"
            # agg is per-sample now; clear it (waits on prior _output read)
            nc.gpsimd.memset(agg[:], 0.0)
            z = [wk.tile([128, D], F32R, tag=f"B1_{tt}", name=f"z{tt}")
                 for tt in range(NT)]
            # low-rank path: x is rank-7, so z = xsh^T @ w2d (rows 32k+c hold
            # the 3 shifts of the 7 channels; row 95 = ones * conv_b) and
            # DFT(z) = w2d^T @ (xsh^T @ dft) -- the DFT runs in the 96-dim
            # input space instead of the 512-dim channel space.
            xshf = wk.tile([96, N], F32, tag="xsh", name="xshf")
            xts = wk.tile([TF, N], F32, tag="xts", name="xts")
            nc.sync.dma_start(xts[:], self.d_xT[s * TF:(s + 1) * TF, :])
            nc.gpsimd.memset(xshf[:], 0.0)
            nc.gpsimd.tensor_copy(xshf[0:TF, 1:N], xts[:, 0:N - 1])
            nc.gpsimd.tensor_copy(xshf[32:32 + TF, 0:N], xts[:, 0:N])
            nc.gpsimd.tensor_copy(xshf[64:64 + TF, 0:N - 1], xts[:, 1:N])
            nc.sync.dma_start(xshf[95:96, :], self.d_ones1[:])
            if hr:
                # Pool can't touch f32r (ISA); one DVE copy re-tags for PE
                xsh = wk.tile([96, N], F32R, tag="xshr", name="xsh")
                nc.vector.tensor_copy(xsh[:], xshf[:])
            else:
                xsh = xshf
            xshT = [wk.tile([128, 96], F32R if hr else F32,
                            tag=f"xshT{tt}", name=f"xshT{tt}")
                    for tt in range(NT)]
            psF1A = self.bank(2, shape=(96, 512))
            psF1B = self.bank(3, shape=(96, 512))
            tid = self.idnr if hr else idn
            d_dft_src = self.d_dftr if hr else self.d_dft
            for tt in range(NT):
                pz = self.bank(tt % 2)
                nc.tensor.matmul(pz[:], xsh[:, _sl(tt)], self.w2dt_[:],
                                 start=True, stop=True)
                nc.scalar.copy(z[tt][:], pz[:])
                pxT = self.bank(6, shape=(128, 96),
                                dtype=F32R if hr else F32)
                nc.tensor.transpose(pxT[:], xsh[:, _sl(tt)], tid[0:96, 0:96])
                nc.scalar.copy(xshT[tt][:], pxT[:])
                dftk = wk.tile([128, 1024], F32R if hr else F32,
                               tag=f"dftk{tt % 2}", name="dftk")
                nc.sync.dma_start(dftk[:], d_dft_src[_sl(tt), :])
                nc.tensor.matmul(psF1A[:], xshT[tt][:], dftk[:, 0:512],
                                 start=(tt == 0), stop=(tt == NT - 1))
                nc.tensor.matmul(psF1B[:], xshT[tt][:], dftk[:, 512:1024],
                                 start=(tt == 0), stop=(tt == NT - 1))
            F1s = wk.tile([96, 1024], F32R if hr else F32, tag="lvp",
                          name="F1s")
            nc.scalar.copy(F1s[:, 0:512], psF1A[:])
            nc.scalar.copy(F1s[:, 512:1024], psF1B[:])
            psA = [self.bank(ct) for ct in range(ND)]
            psB = [self.bank(4 + ct) for ct in range(ND)]
            for ct in range(ND):
                nc.tensor.matmul(psA[ct][:], self.w2dt_[:, _sl(ct)],
                                 F1s[:, 0:512], start=True, stop=True)
                nc.tensor.matmul(psB[ct][:], self.w2dt_[:, _sl(ct)],
                                 F1s[:, 512:1024], start=True, stop=True)
            ibkpf = []
            if irf_r:
                for pf in range(2):
                    ibkp = wk.tile([128, 1024], F32R, tag=f"dftk{pf % 2}",
                                   name="ibk")
                    nc.sync.dma_start(ibkp[:], self.d_ibr[_sl(pf), :])
                    ibkpf.append(ibkp)
            else:
                for pf in range(2):
                    ibkp = wk.tile([128, 2048], BF16, tag=f"dftk{pf % 2}",
                                   name="ibk")
                    nc.sync.dma_start(ibkp[:], self.d_ibhl[_sl(pf), :])
                    ibkpf.append(ibkp)
        else:
            z, zhl = zin

            psA = [self.bank(ct) for ct in range(ND)]
            psB = [self.bank(4 + ct) for ct in range(ND)]
            if PREC["rfft1"] == "f32r":
                # z tiles are F32R [t, d]; stationary slice [t, c-block]
                for kt in range(NT):
                    dftk = wk.tile([128, 1024], F32R, tag=f"dftk{kt % 2}",
                                   name="dftk")
                    nc.sync.dma_start(dftk[:], self.d_dftr[_sl(kt), :])
                    st0 = kt == 0
                    sp = kt == NT - 1
                    for ct in range(ND):
                        zst = z[kt][:, _sl(ct)]
                        nc.tensor.matmul(psA[ct][:], zst, dftk[:, 0:512],
                                         start=st0, stop=sp)
                        nc.tensor.matmul(psB[ct][:], zst, dftk[:, 512:1024],
                                         start=st0, stop=sp)
            else:
                # rfft via bf16 hi/lo 3-term split (exact to ~2^-17)
                for kt in range(NT):
                    dftk = wk.tile([128, 2048], BF16, tag=f"dftk{kt % 2}",
                                   name="dftk")
                    nc.sync.dma_start(dftk[:], self.d_dfthl[_sl(kt), :])
                    st0 = kt == 0
                    sp = kt == NT - 1
                    for ct in range(ND):
                        zh = zhl[kt][:, _sl(ct)]
                        zl = zhl[kt][:, 512 + 128 * ct:640 + 128 * ct]
                        nc.tensor.matmul(psA[ct][:], zh, dftk[:, 0:512],
                                         start=st0, stop=False)
                        nc.tensor.matmul(psA[ct][:], zh, dftk[:, 1024:1536],
                                         start=False, stop=False)
                        nc.tensor.matmul(psB[ct][:], zh, dftk[:, 512:1024],
                                         start=st0, stop=False)
                        nc.tensor.matmul(psB[ct][:], zh, dftk[:, 1536:2048],
                                         start=False, stop=False)
                        nc.tensor.matmul(psA[ct][:], zl, dftk[:, 0:512],
                                         start=False, stop=sp)
                        nc.tensor.matmul(psB[ct][:], zl, dftk[:, 512:1024],
                                         start=False, stop=sp)
            # prefetch the first two irfft ib stripes while the mask runs
            ibkpf = []
            for pf in range(2):
                ibkp = wk.tile([128, 1024], F32R, tag=f"dftk{pf % 2}",
                               name="ibk")
                nc.sync.dma_start(ibkp[:], self.d_ibr[_sl(pf), :])
                ibkpf.append(ibkp)

        # --- top-4 mask -> filt [ND][128, 1024] ([c, f])
        # Pool has no PSUM port: psA/psB land in SBUF once (ACT), then the
        # whole chain (squares, add, is_ge mask) runs on the idle Pool
        # engine; only the top-8 max needs DVE.
        filt = [wk.tile([128, 1024], F32R if irf_r else F32,
                        tag=f"A1_{ct}", name=f"filt{ct}")
                for ct in range(ND)]
        for ct in range(ND):
            sqA = wk.tile([128, 512], F32,
                          tag="sqA" if ct % 2 == 0 else "lnscr2", name="sqA")
            nc.scalar.activation(sqA[:], psA[ct][:], AF.Square)
            sqB = wk.tile([128, 512], F32,
                          tag="w2m0" if ct % 2 == 0 else "w2m1", name="sqB")
            nc.scalar.activation(sqB[:], psB[ct][:], AF.Square)
            amp2 = wk.tile([128, 513], F32,
                           tag="amp2" if ct % 2 == 0 else "lnscr", name="amp2")
            nc.vector.tensor_add(amp2[:, 1:512], sqA[:, 1:512], sqB[:, 1:512])
            nc.scalar.copy(amp2[:, 0:1], sqA[:, 0:1])
            nc.scalar.copy(amp2[:, 512:513], sqB[:, 0:1])
            top8 = wk.tile([128, 8], F32, tag="top8", name="top8")
            nc.vector.max(top8[:], amp2[:])
            kth = top8[:, 3:4]
            nc.vector.scalar_tensor_tensor(filt[ct][:, 0:512], amp2[:, 0:512],
                                           kth, psA[ct][:],
                                           op0=ALU.is_ge, op1=ALU.mult)
            nc.vector.scalar_tensor_tensor(filt[ct][:, 513:1024], amp2[:, 1:512],
                                           kth, psB[ct][:, 1:512],
                                           op0=ALU.is_ge, op1=ALU.mult)
            nc.vector.scalar_tensor_tensor(filt[ct][:, 512:513], amp2[:, 512:513],
                                           kth, psB[ct][:, 0:1],
                                           op0=ALU.is_ge, op1=ALU.mult)

        # --- transpose filt -> filtT [f, c]; hl splits to bf16 hi|lo
        if irf_r:
            filtT = [wk.tile([128, 512], F32R, tag=f"B2_{ft}",
                             name=f"filtT{ft}") for ft in range(NT)]
            for ft in range(NT):
                pT = self.bank(ft % 4, dtype=F32R)
                for ct in range(ND):
                    nc.tensor.transpose(pT[:, _sl(ct)], filt[ct][:, _sl(ft)],
                                        self.idnr[:])
                if ft % 2 == 0:
                    nc.scalar.copy(filtT[ft][:], pT[:])
                else:
                    nc.vector.tensor_copy(filtT[ft][:], pT[:])
        else:
            filtT = [wk.tile([128, 1024], BF16, tag=f"B2_{ft}",
                             name=f"fthl{ft}") for ft in range(NT)]
            for ft in range(NT):
                pT = self.bank(ft % 4)
                for ct in range(ND):
                    nc.tensor.transpose(pT[:, _sl(ct)], filt[ct][:, _sl(ft)],
                                        idn[:])
                nc.scalar.copy(filtT[ft][:, 0:512], pT[:])
                nc.vector.tensor_sub(filtT[ft][:, 512:1024], pT[:],
                                     filtT[ft][:, 0:512])

        # --- irfft (ib streamed, 8 banks) -> lp, z2
        pl = [self.bank(tt) for tt in range(NT)]
        if irf_r:
            for ft in range(NT):
                if ft < 2:
                    ibk = ibkpf[ft]
                else:
                    ibk = wk.tile([128, 1024], F32R, tag=f"dftk{ft % 2}",
                                  name="ibk")
                    nc.sync.dma_start(ibk[:], self.d_ibr[_sl(ft), :])
                for tt in range(NT):
                    nc.tensor.matmul(pl[tt][:], ibk[:, _sl(tt)], filtT[ft][:],
                                     start=(ft == 0), stop=(ft == NT - 1))
        else:
            for ft in range(NT):
                if ft < 2:
                    ibk = ibkpf[ft]
                else:
                    ibk = wk.tile([128, 2048], BF16, tag=f"dftk{ft % 2}",
                                  name="ibk")
                    nc.sync.dma_start(ibk[:], self.d_ibhl[_sl(ft), :])
                for tt in range(NT):
                    ibh = ibk[:, _sl(tt)]
                    ibl = ibk[:, 1024 + 128 * tt:1152 + 128 * tt]
                    nc.tensor.matmul(pl[tt][:], ibh, filtT[ft][:, 0:512],
                                     start=(ft == 0), stop=False)
                    nc.tensor.matmul(pl[tt][:], ibh, filtT[ft][:, 512:1024],
                                     start=False, stop=False)
                    nc.tensor.matmul(pl[tt][:], ibl, filtT[ft][:, 0:512],
                                     start=False, stop=(ft == NT - 1))
        lp = [wk.tile([128, D], F32R, tag=f"B3_{tt}", name=f"lp{tt}")
              for tt in range(NT)]
        z2 = [wk.tile([128, D], F32R if mh_r else F32,
                      tag=f"B4_{tt}", name=f"z2_{tt}")
              for tt in range(NT)]
        for tt in range(NT):
            # z2 before lp: in l1 the lp tiles reuse z's memory (tag B3)
            nc.vector.tensor_sub(z2[tt][:], z[tt][:], pl[tt][:])
            nc.scalar.copy(lp[tt][:], pl[tt][:])

        # --- lpT [ND][128, N] (tag A2) + extrap + perT; then free
        lpT = [wk.tile([128, N], F32R, tag=f"A2_{dt}", name=f"lpT{dt}")
               for dt in range(ND)]
        for h in range(2):
            for dt in range(ND):
                pT = self.bank(dt, dtype=F32R)
                for q in range(4):
                    nc.tensor.transpose(pT[:, _sl(q)], lp[h * 4 + q][:, _sl(dt)],
                                        self.idnr[:])
                if h == 0:
                    nc.vector.tensor_copy(lpT[dt][:, _hh(h)], pT[:])
                    nc.vector.tensor_add(aggsl(dt), aggsl(dt),
                                         lpT[dt][:, 0:HOR])
                else:
                    nc.vector.tensor_copy(lpT[dt][:, _hh(h)], pT[:])
        perT = wk.tile([TF, N], F32, tag="dftk0", name="perT")
        for h in range(2):
            pp = self.bank(2 + h)
            for kt in range(ND):
                nc.tensor.matmul(pp[0:TF, :], lay["lw"][:, (ND + kt) * TF:(ND + kt + 1) * TF],
                                 lpT[kt][:, _hh(h)],
                                 start=(kt == 0), stop=(kt == ND - 1))
            nc.scalar.copy(perT[:, _hh(h)], pp[0:TF, :])

        # --- z2T (tag A2 reuse after lpT dead); hl packs bf16 hi|lo
        if mh_r:
            z2T = [wk.tile([128, N], F32R, tag=f"A2_{dt}", name=f"z2T{dt}")
                   for dt in range(ND)]
            for h in range(2):
                for dt in range(ND):
                    pT = self.bank(dt, dtype=F32R)
                    for q in range(4):
                        nc.tensor.transpose(pT[:, _sl(q)],
                                            z2[h * 4 + q][:, _sl(dt)],
                                            self.idnr[:])
                    nc.vector.tensor_copy(z2T[dt][:, _hh(h)], pT[:])
        else:
            z2T = [wk.tile([128, 2 * N], BF16, tag=f"A2_{dt}",
                           name=f"z2Thl{dt}") for dt in range(ND)]
            for h in range(2):
                for dt in range(ND):
                    pT = self.bank(dt)
                    for q in range(4):
                        nc.tensor.transpose(pT[:, _sl(q)],
                                            z2[h * 4 + q][:, _sl(dt)], idn[:])
                    nc.scalar.copy(z2T[dt][:, _hh(h)], pT[:])
                    nc.vector.tensor_sub(
                        z2T[dt][:, N + 512 * h:N + 512 * h + 512], pT[:],
                        z2T[dt][:, _hh(h)])

        # --- win GEMM -> xinT -> xd -> scan, interleaved per dt so the
        # serial DVE scan chain overlaps the next dt's win GEMMs on PE
        xinT = [wk.tile([128, N], F32, tag=f"A1_{dt}", name=f"xinT{dt}")
                for dt in range(ND)]
        lc = lay["lcol"]
        if mh_r:
            sT = [wk.tile([128, N], F32R, tag=f"A2_{dt}", name=f"sT{dt}")
                  for dt in range(ND)]
            sTsc = sT
            for dt in range(ND):
                for h in range(2):
                    px = self.bank(4 + h)
                    for kt in range(ND):
                        nc.tensor.matmul(px[:], lay["win"][kt][:, _sl(dt)],
                                         z2T[kt][:, _hh(h)],
                                         start=(kt == 0), stop=(kt == ND - 1))
                    # fold the per-head alpha scale into the psum->sbuf copy
                    nc.scalar.activation(xinT[dt][:, _hh(h)], px[:],
                                         AF.Identity,
                                         scale=lay["lcol"][:, dt:dt + 1])
                eng = nc.vector if dt % 2 == 0 else nc.gpsimd
                xd = wk.tile([128, N], F32,
                             tag="xdsc0" if dt % 2 == 0 else "xdsc1",
                             name="xd")
                eng.tensor_sub(xd[:, 1:N], xinT[dt][:, 1:N],
                               xinT[dt][:, 0:N - 1])
                # xinT is pre-scaled by alpha; col 20+dt folds the initial
                nc.vector.tensor_scalar_add(xd[:, 0:1], xinT[dt][:, 0:1],
                                            lc[:, 20 + dt:21 + dt])
                omab_ap = lc[:, 4 + dt:5 + dt].broadcast_to([128, N])
                nc.vector.tensor_tensor_scan(sTsc[dt][:], omab_ap, xd[:], 0.0,
                                             op0=ALU.mult, op1=ALU.add)
        else:
            for h in range(2):
                for dt in range(ND):
                    px = self.bank(4 + dt % 2)
                    for kt in range(ND):
                        wh = lay["win"][kt][:, _sl(dt)]
                        wl = lay["win"][kt][:, 512 + 128 * dt:640 + 128 * dt]
                        zh = z2T[kt][:, _hh(h)]
                        zl = z2T[kt][:, N + 512 * h:N + 512 * h + 512]
                        nc.tensor.matmul(px[:], wh, zh,
                                         start=(kt == 0), stop=False)
                        nc.tensor.matmul(px[:], wh, zl,
                                         start=False, stop=False)
                        nc.tensor.matmul(px[:], wl, zh,
                                         start=False, stop=(kt == ND - 1))
                    nc.scalar.activation(xinT[dt][:, _hh(h)], px[:],
                                         AF.Identity,
                                         scale=lay["lcol"][:, dt:dt + 1])
            sTsc = [wk.tile([128, N], F32, tag=f"A1_{dt}", name=f"sTf{dt}")
                    for dt in range(ND)]
            sT = [wk.tile([128, 2 * N], BF16, tag=f"A2_{dt}",
                          name=f"sThl{dt}") for dt in range(ND)]
            for dt in range(ND):
                eng = nc.vector if dt % 2 == 0 else nc.gpsimd
                xd = wk.tile([128, N], F32,
                             tag="xdsc0" if dt % 2 == 0 else "xdsc1",
                             name="xd")
                eng.tensor_sub(xd[:, 1:N], xinT[dt][:, 1:N],
                               xinT[dt][:, 0:N - 1])
                nc.vector.tensor_scalar_add(xd[:, 0:1], xinT[dt][:, 0:1],
                                            lc[:, 20 + dt:21 + dt])
                omab_ap = lc[:, 4 + dt:5 + dt].broadcast_to([128, N])
                nc.vector.tensor_tensor_scan(sTsc[dt][:], omab_ap, xd[:], 0.0,
                                             op0=ALU.mult, op1=ALU.add)
                eng.tensor_copy(sT[dt][:, 0:N], sTsc[dt][:])
                eng.tensor_sub(sT[dt][:, N:2 * N], sTsc[dt][:],
                               sT[dt][:, 0:N])

        # --- wout GEMM -> lg [t,d] (tag B2 reuse: filtT dead) (+ z3 if l0)
        lg = [wk.tile([128, D], F32R, tag=f"B2_{tt}", name=f"lg{tt}")
              for tt in range(NT)]
        for tt in range(NT):
            pg = self.bank(tt % 2)
            if mh_r:
                for kt in range(ND):
                    nc.tensor.matmul(pg[:], sT[kt][:, _sl(tt)],
                                     lay["wout"][kt][:],
                                     start=(kt == 0), stop=(kt == ND - 1))
            else:
                for kt in range(ND):
                    sh = sT[kt][:, _sl(tt)]
                    sl_ = sT[kt][:, N + 128 * tt:N + 128 * tt + 128]
                    nc.tensor.matmul(pg[:], sh, lay["wout"][kt][:, 0:512],
                                     start=(kt == 0), stop=False)
                    nc.tensor.matmul(pg[:], sh, lay["wout"][kt][:, 512:1024],
                                     start=False, stop=False)
                    nc.tensor.matmul(pg[:], sl_, lay["wout"][kt][:, 0:512],
                                     start=False, stop=(kt == ND - 1))
            nc.vector.tensor_add(lg[tt][:], pg[:], lay["boutb"][:])
            if not last:
                # z3 overwrites z (tag B1): z dead after z2
                nc.vector.tensor_sub(z[tt][:], z2[tt][:], lg[tt][:])
        z3 = z

        # exact last-growth column for damp (avoids f32r transpose truncation,
        # which the dampening cumsum amplifies)
        lglast = wk.tile([1, D], F32, tag="sqA", name="lglast")
        nc.gpsimd.dma_start(lglast[:], lg[NT - 1][127:128, :])
        lgl4 = wk.tile([128, ND], F32, tag="top8", name="lgl4")
        pTl = self.bank(7, shape=(128, ND))
        for dt in range(ND):
            nc.tensor.matmul(pTl[:, dt:dt + 1], lglast[0:1, _sl(dt)],
                             ones[0:1, 0:1], start=True, stop=True)
        nc.scalar.copy(lgl4[:], pTl[:])

        # --- lgT via transposes (tag A1 reuse: xinT dead)
        lgT = [wk.tile([128, N], F32R, tag=f"A1_{dt}", name=f"lgT{dt}")
               for dt in range(ND)]
        for h in range(2):
            for dt in range(ND):
                pT = self.bank(2 + dt % 2, dtype=F32R)
                for q in range(4):
                    nc.tensor.transpose(pT[:, _sl(q)], lg[h * 4 + q][:, _sl(dt)],
                                        self.idnr[:])
                if h == 0:
                    nc.scalar.copy(lgT[dt][:, _hh(h)], pT[:])
                else:
                    nc.vector.tensor_copy(lgT[dt][:, _hh(h)], pT[:])
        for dt in range(ND):
            # damp: agg += lg_last * csd
            nc.vector.scalar_tensor_tensor(
                aggsl(dt), self.csdt[:, dt * HOR:(dt + 1) * HOR],
                lgl4[:, dt:dt + 1], aggsl(dt), op0=ALU.mult, op1=ALU.add)

        # --- level: grT; scans update xtmid
        grT = wk.tile([TF, N], F32, tag="grT", name="grT")
        for h in range(2):
            pgr = self.bank(6)
            for kt in range(ND):
                nc.tensor.matmul(pgr[0:TF, :], lay["lw"][:, kt * TF:(kt + 1) * TF],
                                 lgT[kt][:, _hh(h)],
                                 start=(kt == 0), stop=(kt == ND - 1))
            # fold level bg bias (lcol col 18) into the psum->sbuf copy
            nc.vector.tensor_scalar_add(grT[:, _hh(h)], pgr[0:TF, :],
                                        lc[0:TF, 18:19])

        xts2 = wk.tile([TF, N], F32, tag="xts", name="xts2")
        if l == 0:
            nc.sync.dma_start(xts2[:], self.d_xT[s * TF:(s + 1) * TF, :])
        else:
            nc.sync.dma_start(xts2[:], self.xtmid[s, :, :])
        v = wk.tile([TF, N], F32, tag="lvv", name="lvv")
        # v = (xts2 - bp) - perT (DVE: Pool has no TensorScalarPtr opcode);
        # the long scans still run on Pool to keep DVE free
        nc.vector.scalar_tensor_tensor(v[:], xts2[:], lc[0:TF, 19:20], perT[:],
                                       op0=ALU.subtract, op1=ALU.subtract)
        nc.vector.tensor_scalar_mul(v[:], v[:], lc[0:TF, 16:17])
        omlv_ap = lc[0:TF, 17:18].broadcast_to([TF, N])
        pt = wk.tile([TF, N], F32, tag="lvp", name="lvp")
        nc.vector.tensor_tensor_scan(pt[:], omlv_ap, v[:], 0.0,
                                     op0=ALU.mult, op1=ALU.add)
        gt = wk.tile([TF, N], F32, tag="lvv", name="lvg")
        nc.vector.tensor_tensor_scan(gt[:], omlv_ap, grT[:], 0.0,
                                     op0=ALU.mult, op1=ALU.add)
        xnew = wk.tile([TF, N], F32, tag="grT", name="xnew")
        nc.gpsimd.tensor_add(xnew[:], pt[:], gt[:])
        # issue on Pool: keeps this late-blocking store off the SP DMA queue
        nc.gpsimd.dma_start(self.xtmid[s, :, :], xnew[:])

        # --- FF (layer 0 only); z4 stays in SBUF for l1
        if not last:
            return self._ff(s, z3, wk)
        return None

    # ---------- LN stats ----------
    def _ln_stats(self, zset, wk, tagp):
        nc = self.nc
        st = wk.tile([128, 8 * NT], F32, tag=f"st{tagp}", name=f"st{tagp}")
        mu8 = st[:, 0:NT]
        s28 = st[:, NT:2 * NT]
        for tt in range(NT):
            scr = wk.tile([128, D], F32,
                          tag="lnscr" if tt % 2 == 0 else "lnscr2",
                          name="lnscr")
            nc.vector.tensor_reduce(st[:, tt:tt + 1], zset[tt][:],
                                    mybir.AxisListType.X, op=ALU.add)
            nc.scalar.activation(scr[:], zset[tt][:], AF.Square,
                                 accum_out=st[:, NT + tt:NT + tt + 1])
        mun = st[:, 2 * NT:3 * NT]
        nc.vector.tensor_scalar_mul(mun, mu8, 1.0 / D)
        ex2 = st[:, 3 * NT:4 * NT]
        nc.vector.tensor_scalar_mul(ex2, s28, 1.0 / D)
        musq = st[:, 4 * NT:5 * NT]
        nc.scalar.activation(musq, mun, AF.Square)
        var = st[:, 5 * NT:6 * NT]
        nc.vector.tensor_sub(var, ex2, musq)
        sd = st[:, 6 * NT:7 * NT]
        nc.scalar.activation(sd, var, AF.Sqrt, bias=self.epst[:, 0:1])
        rs = st[:, 7 * NT:8 * NT]
        nc.vector.reciprocal(rs, sd)
        nmurs = st[:, 4 * NT:5 * NT]  # overwrite musq slot
        nc.vector.tensor_mul(nmurs, mun, rs)
        nc.vector.tensor_scalar_mul(nmurs, nmurs, -1.0)
        return rs, nmurs

    # ---------- FF block ----------
    def _ff(self, s, z3, wk):
        if PREC["ff"] == "f32r":
            return self._ff_f32r(s, z3, wk)
        return self._ff_hl(s, z3, wk)

    def _ff_f32r(self, s, z3, wk):
        nc = self.nc
        cpk = self.cpk
        # fully per-tt pre-LN chains (no batch barrier): h_[0..3] are ready
        # while wout still writes the last z3 tiles, so the hT transposes
        # start without a 9us PE stall
        st = wk.tile([128, 8 * NT], F32, tag="stpre", name="stpre")
        h_ = [wk.tile([128, D], F32R, tag=f"B2_{tt}", name=f"h{tt}")
              for tt in range(NT)]
        for tt in range(NT):
            scr = wk.tile([128, D], F32,
                          tag="lnscr" if tt % 2 == 0 else "lnscr2",
                          name="lnscr")
            mu = st[:, tt:tt + 1]
            s2 = st[:, NT + tt:NT + tt + 1]
            nc.vector.tensor_reduce(mu, z3[tt][:], mybir.AxisListType.X,
                                    op=ALU.add)
            nc.scalar.activation(scr[:], z3[tt][:], AF.Square, accum_out=s2)
            mun = st[:, 2 * NT + tt:2 * NT + tt + 1]
            nc.vector.tensor_scalar_mul(mun, mu, 1.0 / D)
            musq = st[:, 3 * NT + tt:3 * NT + tt + 1]
            nc.scalar.activation(musq, mun, AF.Square)
            var = st[:, 4 * NT + tt:4 * NT + tt + 1]
            nc.vector.scalar_tensor_tensor(var, s2, 1.0 / D, musq,
                                           op0=ALU.mult, op1=ALU.subtract)
            sd = st[:, 5 * NT + tt:5 * NT + tt + 1]
            nc.scalar.activation(sd, var, AF.Sqrt, bias=self.epst[:, 0:1])
            rs = st[:, 6 * NT + tt:6 * NT + tt + 1]
            nc.vector.reciprocal(rs, sd)
            nmurs = st[:, 7 * NT + tt:7 * NT + tt + 1]
            nc.vector.scalar_tensor_tensor(nmurs, mun, -1.0, rs,
                                           op0=ALU.mult, op1=ALU.mult)
            nc.scalar.activation(h_[tt][:], z3[tt][:], AF.Identity,
                                 scale=rs, bias=nmurs)
        hT = [wk.tile([128, N], F32R, tag=f"A2_{dt}", name=f"hT{dt}")
              for dt in range(ND)]
        for h in range(2):
            for dt in range(ND):
                pT = self.bank(dt, dtype=F32R)
                for q in range(4):
                    nc.tensor.transpose(pT[:, _sl(q)], h_[h * 4 + q][:, _sl(dt)],
                                        self.idnr[:])
                if h == 0:
                    nc.scalar.copy(hT[dt][:, _hh(h)], pT[:])
                else:
                    nc.vector.tensor_copy(hT[dt][:, _hh(h)], pT[:])
        znT = [wk.tile([128, N], F32R, tag=f"A1_{dt}", name=f"znT{dt}")
               for dt in range(ND)]
        for h in range(2):
            for dt in range(ND):
                nc.vector.tensor_scalar(znT[dt][:, _hh(h)], hT[dt][:, _hh(h)],
                                        cpk[:, dt:dt + 1],
                                        cpk[:, 4 + dt:5 + dt],
                                        op0=ALU.mult, op1=ALU.add)

        yT = [wk.tile([128, N], F32R, tag=f"A2_{dt}", name=f"yT{dt}")
              for dt in range(ND)]
        for h in range(2):
            pzf = [self.bank(dt) for dt in range(ND)]
            # software-pipelined: w2(m-1) is emitted AFTER w1(m), so the PE
            # never sits head-of-line waiting on sig(m-1)'s ACT latency
            sigs = [None, None]
            w2ms = [None, None]

            def w2_stage(m):
                for dt in range(ND):
                    nc.tensor.matmul(pzf[dt][:], w2ms[m % 2][:, _sl(dt)],
                                     sigs[m % 2][:],
                                     start=(m == 0), stop=(m == NM - 1))

            for m in range(NM):
                w1m = wk.tile([128, 512], F32R, tag=f"w1mh{m % 2}", name="w1m")
                nc.sync.dma_start(w1m[:], self.d_ffw1r[m, :, :])
                ph = self.bank(4 + m % 2)
                for kt in range(ND):
                    nc.tensor.matmul(ph[:], w1m[:, _sl(kt)],
                                     znT[kt][:, _hh(h)],
                                     start=(kt == 0), stop=(kt == ND - 1))
                if m > 0:
                    w2_stage(m - 1)
                sig = wk.tile([128, 512], F32R, tag=f"sig{m % 2}", name="sig")
                nc.scalar.activation(sig[:], ph[:], AF.Sigmoid,
                                     bias=cpk[:, 8 + m:9 + m])
                sigs[m % 2] = sig
                w2m = wk.tile([128, 512], F32R, tag=f"w2m{m % 2}", name="w2m")
                nc.sync.dma_start(w2m[:], self.d_ffw2r[_sl(m), :])
                w2ms[m % 2] = w2m
            w2_stage(NM - 1)
            for dt in range(ND):
                nc.vector.scalar_tensor_tensor(yT[dt][:, _hh(h)], pzf[dt][:],
                                               cpk[:, 24 + dt:25 + dt],
                                               znT[dt][:, _hh(h)],
                                               op0=ALU.add, op1=ALU.add)
        return self._post_ln(s, yT, wk, yr=True)

    def _ff_hl(self, s, z3, wk):
        nc = self.nc
        idn = self.idn
        cpk = self.cpk
        rs, nmurs = self._ln_stats(z3, wk, "pre")
        h_ = [wk.tile([128, D], F32, tag=f"B2_{tt}", name=f"h{tt}")
              for tt in range(NT)]
        for tt in range(NT):
            nc.scalar.activation(h_[tt][:], z3[tt][:], AF.Identity,
                                 scale=rs[:, tt:tt + 1], bias=nmurs[:, tt:tt + 1])
        hT = [wk.tile([128, N], F32, tag=f"A2_{dt}", name=f"hT{dt}")
              for dt in range(ND)]
        for h in range(2):
            for dt in range(ND):
                pT = self.bank(dt)
                for q in range(4):
                    nc.tensor.transpose(pT[:, _sl(q)], h_[h * 4 + q][:, _sl(dt)],
                                        idn[:])
                if h == 0:
                    nc.scalar.copy(hT[dt][:, _hh(h)], pT[:])
                else:
                    nc.vector.tensor_copy(hT[dt][:, _hh(h)], pT[:])
        znT = [wk.tile([128, N], F32, tag=f"A1_{dt}", name=f"znT{dt}")
               for dt in range(ND)]
        for h in range(2):
            for dt in range(ND):
                nc.vector.tensor_scalar(znT[dt][:, _hh(h)], hT[dt][:, _hh(h)],
                                        cpk[:, dt:dt + 1],
                                        cpk[:, 4 + dt:5 + dt],
                                        op0=ALU.mult, op1=ALU.add)

        yT = [wk.tile([128, N], F32, tag=f"A2_{dt}", name=f"yT{dt}")
              for dt in range(ND)]
        for h in range(2):
            znb = [wk.tile([128, 1024], BF16, tag=f"B3_{kt}", name=f"znb{kt}")
                   for kt in range(ND)]
            for kt in range(ND):
                nc.vector.tensor_copy(znb[kt][:, 0:512], znT[kt][:, _hh(h)])
                nc.vector.tensor_sub(znb[kt][:, 512:1024], znT[kt][:, _hh(h)],
                                     znb[kt][:, 0:512])
            pzf = [self.bank(dt) for dt in range(ND)]
            for m in range(NM):
                w1m = wk.tile([128, 2 * ND * 128], BF16,
                              tag=f"w1mh{m % 2}", name="w1m")
                nc.sync.dma_start(w1m[:], self.d_ffw1t[m, :, :])
                ph = self.bank(4 + m % 2)
                for kt in range(ND):
                    nc.tensor.matmul(ph[:], w1m[:, _sl(kt)], znb[kt][:, 0:512],
                                     start=(kt == 0), stop=False)
                    nc.tensor.matmul(ph[:], w1m[:, _sl(kt)], znb[kt][:, 512:1024],
                                     start=False, stop=False)
                    nc.tensor.matmul(ph[:], w1m[:, 512 + 128 * kt:640 + 128 * kt],
                                     znb[kt][:, 0:512],
                                     start=False, stop=(kt == ND - 1))
                sig = wk.tile([128, 512], F32, tag=f"sig{m % 2}", name="sig")
                nc.scalar.activation(sig[:], ph[:], AF.Sigmoid,
                                     bias=cpk[:, 8 + m:9 + m])
                sighl = wk.tile([128, 1024], BF16,
                                tag="amp2" if m % 2 == 0 else "lnscr",
                                name="sighl")
                nc.vector.tensor_copy(sighl[:, 0:512], sig[:])
                nc.vector.tensor_sub(sighl[:, 512:1024], sig[:],
                                     sighl[:, 0:512])
                w2m = wk.tile([128, 1024], BF16, tag=f"w2m{m % 2}", name="w2m")
                nc.sync.dma_start(w2m[:], self.d_ffw2hl[_sl(m), :])
                for dt in range(ND):
                    nc.tensor.matmul(pzf[dt][:], w2m[:, _sl(dt)],
                                     sighl[:, 0:512],
                                     start=(m == 0), stop=False)
                    nc.tensor.matmul(pzf[dt][:], w2m[:, _sl(dt)],
                                     sighl[:, 512:1024],
                                     start=False, stop=False)
                    nc.tensor.matmul(pzf[dt][:], w2m[:, 512 + dt * 128:
                                                     640 + dt * 128],
                                     sighl[:, 0:512],
                                     start=False, stop=(m == NM - 1))
            for dt in range(ND):
                nc.vector.scalar_tensor_tensor(yT[dt][:, _hh(h)], pzf[dt][:],
                                               cpk[:, 24 + dt:25 + dt],
                                               znT[dt][:, _hh(h)],
                                               op0=ALU.add, op1=ALU.add)
        return self._post_ln(s, yT, wk, yr=False)

    def _post_ln(self, s, yT, wk, yr):
        # fully per-tt post-LN chains: z4[0] is ready before the last yT
        # transposes finish, so l1's rfft starts with no barrier on the
        # batched stats
        nc = self.nc
        idn = self.idn
        rfr = PREC["rfft1"] == "f32r"
        gb, bb = self.gbt, self.bbt
        z4 = [wk.tile([128, D], F32R, tag=f"B3_{tt}", name=f"z4_{tt}")
              for tt in range(NT)]
        if not rfr:
            zhl = [wk.tile([128, 1024], BF16, tag=f"B2_{tt}", name=f"zhl{tt}")
                   for tt in range(NT)]
        st = wk.tile([128, 8 * NT], F32, tag="stpost", name="stpost")
        for tt in range(NT):
            pT = self.bank(6 + tt % 2, dtype=F32R if yr else F32)
            for dt in range(ND):
                nc.tensor.transpose(pT[:, _sl(dt)], yT[dt][:, _sl(tt)],
                                    self.idnr[:] if yr else idn[:])
            y_t = wk.tile([128, D], F32, tag=f"B4_{tt}", name=f"y{tt}")
            nc.scalar.copy(y_t[:], pT[:])
            scr = wk.tile([128, D], F32,
                          tag="lnscr" if tt % 2 == 0 else "lnscr2",
                          name="lnscr")
            mu = st[:, tt:tt + 1]
            s2 = st[:, NT + tt:NT + tt + 1]
            nc.vector.tensor_reduce(mu, y_t[:], mybir.AxisListType.X,
                                    op=ALU.add)
            nc.scalar.activation(scr[:], y_t[:], AF.Square, accum_out=s2)
            mun = st[:, 2 * NT + tt:2 * NT + tt + 1]
            nc.vector.tensor_scalar_mul(mun, mu, 1.0 / D)
            musq = st[:, 3 * NT + tt:3 * NT + tt + 1]
            nc.scalar.activation(musq, mun, AF.Square)
            var = st[:, 4 * NT + tt:4 * NT + tt + 1]
            nc.vector.scalar_tensor_tensor(var, s2, 1.0 / D, musq,
                                           op0=ALU.mult, op1=ALU.subtract)
            sd = st[:, 5 * NT + tt:5 * NT + tt + 1]
            nc.scalar.activation(sd, var, AF.Sqrt, bias=self.epst[:, 0:1])
            rs = st[:, 6 * NT + tt:6 * NT + tt + 1]
            nc.vector.reciprocal(rs, sd)
            nmurs = st[:, 7 * NT + tt:7 * NT + tt + 1]
            nc.vector.scalar_tensor_tensor(nmurs, mun, -1.0, rs,
                                           op0=ALU.mult, op1=ALU.mult)
            nc.scalar.activation(scr[:], y_t[:], AF.Identity,
                                 scale=rs, bias=nmurs)
            nc.vector.tensor_mul(z4[tt][:], scr[:], gb[:])
            nc.vector.tensor_add(z4[tt][:], z4[tt][:], bb[:])
            if not rfr:
                nc.gpsimd.tensor_copy(zhl[tt][:, 0:512], z4[tt][:])
                nc.gpsimd.tensor_sub(zhl[tt][:, 512:1024], z4[tt][:],
                                     zhl[tt][:, 0:512])
        if rfr:
            return z4, None
        return z4, zhl

    # ---------- output head ----------
    def _output(self, s, wk):
        nc = self.nc
        ones = self.ones
        po = self.bank(7)
        for kt in range(ND):
            nc.tensor.matmul(po[0:TF, 0:HOR], self.outwt[:, kt * TF:(kt + 1) * TF],
                             self.aggt[:, kt * HOR:(kt + 1) * HOR],
                             start=(kt == 0), stop=False)
        nc.tensor.matmul(po[0:TF, 0:HOR], self.outbt[0:1, 0:TF],
                         ones[0:1, 0:HOR], start=False, stop=True)
        xfin = wk.tile([TF, N], F32, tag="lvp", name="xfin")
        nc.gpsimd.dma_start(xfin[:], self.xtmid[s, :, :])
        oT = wk.tile([TF, HOR], F32, tag="lvv", name="oT")
        nc.vector.tensor_scalar_add(oT[:], po[0:TF, 0:HOR], xfin[:, N - 1:N])
        nc.gpsimd.dma_start(self.d_out[s * TF:(s + 1) * TF, :], oT[:])


def _get_nc():
    if "nc" not in _CACHE:
        _CACHE["nc"] = K().build()
    return _CACHE["nc"]


def _common_maps(inputs, w2d, dft, ib, e8):
    m = dict(
        w2d=_rne11(w2d) if PREC["l0head"] == "f32r" else w2d,
        ones1=np.ones((1, N), np.float32),
        idn=np.eye(128, dtype=np.float32),
        e8=e8,
        ibr=_rne11(ib),
        winr=_rne11(np.asarray(inputs["mhesa_win"], np.float32)),
        woutr=_rne11(np.asarray(inputs["mhesa_wout"], np.float32)),
        boutr=np.asarray(inputs["mhesa_bout"], np.float32).reshape(L, 1, D),
        lcolp=_pack_lcol(inputs),
        alpha8=np.asarray(inputs["mhesa_alpha"], np.float32).reshape(L, HEADS, 1),
        cpkp=_pack_cpk(inputs),
        gpostr=np.asarray(inputs["ff_post_g"], np.float32).reshape(1, D),
        bpostr=np.asarray(inputs["ff_post_b"], np.float32).reshape(1, D),
        lvwg=np.asarray(inputs["level_wg"], np.float32),
        lvwp=np.asarray(inputs["level_wp"], np.float32),
        lvbg=np.asarray(inputs["level_bg"], np.float32).reshape(L, 1, TF),
        lvbp=np.asarray(inputs["level_bp"], np.float32).reshape(L, 1, TF),
        lvalpha=np.asarray(inputs["level_alpha"], np.float32).reshape(L, 1, 1),
        damp8=np.asarray(inputs["dampen_factor"], np.float32).reshape(HEADS, 1),
        outw=np.asarray(inputs["out_w"], np.float32)
            .reshape(ND, 128, TF).transpose(1, 0, 2).reshape(128, ND * TF)
            .copy(),
        outbr=np.asarray(inputs["out_b"], np.float32).reshape(1, TF),
    )
    if PREC["l0head"] == "f32":
        m["dft"] = dft
    if PREC["l0head"] == "f32r" or PREC["rfft1"] == "f32r":
        m["dftr"] = _rne11(dft)
    if PREC["rfft1"] == "hl":
        m["dfthl"] = np.concatenate([_split_hi(dft), _split_lo(dft)], axis=1)
    if PREC["irfft0"] == "hl":
        m["ibhl"] = np.concatenate([_split_hi(ib), _split_lo(ib)], axis=1)
    if PREC["mhesa0"] == "hl":
        win0 = np.asarray(inputs["mhesa_win"][0], np.float32)
        wout0 = np.asarray(inputs["mhesa_wout"][0], np.float32)
        m["winhl"] = np.concatenate([_split_hi(win0), _split_lo(win0)], axis=1)
        m["wouthl"] = np.concatenate([_split_hi(wout0), _split_lo(wout0)],
                                     axis=1)
    w1 = np.asarray(inputs["ff_w1"], np.float32)
    w2 = np.asarray(inputs["ff_w2"], np.float32)
    if PREC["ff"] == "f32r":
        m["ffw1r"] = _rne11(_pack_w1r(w1))
        m["ffw2r"] = _rne11(w2)
    else:
        m["ffw1t"] = _pack_w1(w1)
        m["ffw2hl"] = np.concatenate([_split_hi(w2), _split_lo(w2)], axis=1)
    return m


def _kernel_impl(inputs, runner):
    x = np.asarray(inputs["x"], np.float32)
    assert (x.shape[0], x.shape[1], x.shape[2]) == (32, N, TF)
    assert int(inputs["forecast_horizon"]) == HOR
    dft, ib = _dft_consts()
    conv_w = np.asarray(inputs["conv_w"], np.float32)
    w2d = _build_w2d(conv_w, np.asarray(inputs["conv_b"], np.float32))
    e8 = np.repeat(np.eye(HEADS, dtype=np.float32), DH, axis=1)
    nc = _get_nc()
    common = _common_maps(inputs, w2d, dft, ib, e8)
    in_maps = []
    for c in range(NCORES):
        xs = x[c * S:(c + 1) * S]
        xT = xs.transpose(0, 2, 1).reshape(S * TF, N).copy()
        in_maps.append(dict(common, xT=xT))
    res = runner(nc, in_maps)
    out = np.zeros((x.shape[0], HOR, TF), np.float32)
    for c in range(NCORES):
        oT = res.results[c]["outT"].reshape(S, TF, HOR)
        out[c * S:(c + 1) * S] = oT.transpose(0, 2, 1)
    return out, res


def kernel(**inputs):
    out, _ = _kernel_impl(
        inputs,
        lambda nc, im: run_bass_kernel_spmd(nc, im, list(range(NCORES))))
    return out


def kernel_traced(**inputs):
    """Like kernel() but with NTFF profiling; returns (out, BassKernelResults)."""
    return _kernel_impl(
        inputs,
        lambda nc, im: run_bass_kernel_spmd(nc, im, list(range(NCORES)),
                                            trace=True))


# revision 36
# speedup vs baseline: 1.5515x; 1.0157x over previous
"""ETSFormer forward pass on 8 Trainium2 NeuronCores (Bass/Tile).

Data-parallel over batch: 32 samples -> 8 cores x 4 samples, weights
replicated, no collectives. The reference's FFT machinery is computed
exactly without FFTs:
  - freq_attention: dense DFT matmuls + hardware top-8 (vector.max) mask
  - mhesa / level exponential smoothing: the reference FFT cross-correlation
    is exactly a first-order EMA -> hardware prefix scan (tensor_tensor_scan)
  - fourier_extrapolate: exact slice (Dirichlet kernel identity)

Precision: PREC selects per-GEMM-group dtype. "f32r" = fp32-reduced
(FP22 truncated, 1 cyc/row on PE -- same speed as bf16) vs the fallback
"hl" = bf16 hi/lo 3-term split (~2^-16, 3 cyc/row) / "f32" = true fp32
(4 cyc/row). The top-4 frequency mask is rank-sensitive; flags are
tuned empirically against the end-to-end error gate.
"""
import numpy as np
from contextlib import ExitStack

import concourse.bass as bass
import concourse.bacc as bacc
import concourse.tile as tile
from concourse import mybir
from concourse.bass_utils import run_bass_kernel_spmd

F32 = mybir.dt.float32
F32R = mybir.dt.float32r
BF16 = mybir.dt.bfloat16
AF = mybir.ActivationFunctionType
ALU = mybir.AluOpType

N = 1024
D = 512
TF = 7
HEADS = 8
DH = D // HEADS
L = 2
S = 4
NCORES = 8
HOR = 96
FD = 2048
NT = N // 128   # 8
ND = D // 128   # 4
NM = FD // 128  # 16

_CACHE = {}

# per-stage precision: "f32r" fast path vs baseline "hl" (bf16 3-term)
# / "f32" (true fp32) fallback.
PREC = dict(
    l0head="f32",    # conv z GEMM + low-rank DFT: feeds the layer-0 top-4
                     # ranking, which flips even under 2^-12 weight rounding
                     # (emulation: 52 flips, 2.7e-2 err) -- keep exact fp32
    irfft0="f32r",   # layer-0 irfft (feeds layer-1 ranking path)
    mhesa0="f32r",   # layer-0 win/wout GEMMs
    ff="f32r",       # FF block w1/w2 GEMMs
    rfft1="f32r",    # layer-1 rfft (feeds layer-1 ranking directly)
)


def _rne11(x):
    # round fp32 mantissa to 11 explicit bits (fp22): the PE's f32r mode
    # truncates operands to fp22, so pre-rounded weights pass through
    # losslessly -- halves f32r noise and removes the truncation bias
    xi = np.ascontiguousarray(np.asarray(x, np.float32)).view(np.uint32)
    return ((xi + np.uint32(0x800)) & np.uint32(0xFFFFF000)).view(np.float32)


def _dft_consts():
    if "dft" not in _CACHE:
        t = np.arange(N)
        f = np.arange(513)
        ang = 2.0 * np.pi * np.outer(t, f) / N
        cos = np.cos(ang)
        sin = np.sin(ang)
        # B-half layout [sin(1..511), cos(512)] (cos512 moved to the END):
        # then amp2[f] = sqA[f] + sqB[f-1] for f=1..512 is a single shifted
        # add, and the f>=512 mask is ONE scalar_tensor_tensor over psB.
        dft = np.zeros((N, 1024), np.float64)
        dft[:, 0:512] = cos[:, 0:512]
        dft[:, 512:1023] = sin[:, 1:512]
        dft[:, 1023] = cos[:, 512]
        c = np.full(513, 2.0)
        c[0] = 1.0
        c[512] = 1.0
        ib = np.zeros((1024, N), np.float64)
        ib[0:512, :] = (c[0:512, None] / N) * cos[:, 0:512].T
        ib[512:1023, :] = (2.0 / N) * sin[:, 1:512].T
        ib[1023, :] = (1.0 / N) * cos[:, 512]
        _CACHE["dft"] = dft.astype(np.float32)
        _CACHE["ib"] = ib.astype(np.float32)
    return _CACHE["dft"], _CACHE["ib"]


def _sl(i, w=128):
    return slice(i * w, (i + 1) * w)


def _split_hi(x):
    import ml_dtypes
    return x.astype(ml_dtypes.bfloat16)


def _split_lo(x):
    import ml_dtypes
    hi = x.astype(ml_dtypes.bfloat16).astype(np.float32)
    return (x - hi).astype(ml_dtypes.bfloat16)


def _pack_w1(w1):
    # bf16 hi|lo tiles for the "hl" fallback FF path
    hi, lo = _split_hi(w1), _split_lo(w1)
    out = np.zeros((NM, 128, 1024), hi.dtype)
    for m in range(NM):
        for kt in range(ND):
            out[m, :, 128 * kt:128 * (kt + 1)] = hi[_sl(kt), _sl(m)]
            out[m, :, 512 + 128 * kt:640 + 128 * kt] = lo[_sl(kt), _sl(m)]
    return out


def _pack_w1r(w1):
    # f32r per-m contiguous [128(k), 4x128(m)] tiles
    out = np.zeros((NM, 128, 512), np.float32)
    for m in range(NM):
        for kt in range(ND):
            out[m, :, 128 * kt:128 * (kt + 1)] = w1[_sl(kt), _sl(m)]
    return out


def _pack_cpk(inputs):
    # cols: gpre(4) | bpre(4) | ffb1(16) | ffb2(4), each D/FD vector folded
    # into [128, k] column blocks -- one DMA instead of 24
    out = np.zeros((128, 28), np.float32)
    out[:, 0:4] = np.asarray(inputs["ff_pre_g"], np.float32).reshape(4, 128).T
    out[:, 4:8] = np.asarray(inputs["ff_pre_b"], np.float32).reshape(4, 128).T
    out[:, 8:24] = np.asarray(inputs["ff_b1"], np.float32).reshape(16, 128).T
    out[:, 24:28] = np.asarray(inputs["ff_b2"], np.float32).reshape(4, 128).T
    return out


def _pack_lcol(inputs):
    # per layer: init(4 cols) | bin(4 cols)
    out = np.zeros((L, 128, 8), np.float32)
    ini = np.asarray(inputs["mhesa_init"], np.float32).reshape(L, D)
    bi = np.asarray(inputs["mhesa_bin"], np.float32)
    for l in range(L):
        out[l, :, 0:4] = ini[l].reshape(4, 128).T
        out[l, :, 4:8] = bi[l].reshape(4, 128).T
    return out


def _build_w2d(conv_w, conv_b):
    # rows 32k+c hold conv_w[:, c, k] (32-aligned partition groups so the
    # on-device shifted copies keep legal base partitions); row 95 is the
    # bias row, paired with an all-ones row 95 of xsh on device.
    w2d = np.zeros((96, D), np.float32)
    for k in range(3):
        for c in range(TF):
            w2d[32 * k + c] = conv_w[:, c, k]
    w2d[95] = conv_b
    return w2d


def _hh(h):
    return slice(h * 512, (h + 1) * 512)


class K:
    def __init__(self):
        nc = bacc.Bacc()
        self.nc = nc
        p = nc.declare_dram_parameter
        self.d_xT = p("xT", [S * TF, N], F32, isOutput=False)
        self.d_w2d = p("w2d", [96, D],
                       F32R if PREC["l0head"] == "f32r" else F32,
                       isOutput=False)
        self.d_ones1 = p("ones1", [1, N], F32, isOutput=False)
        if PREC["l0head"] == "f32":
            self.d_dft = p("dft", [N, 1024], F32, isOutput=False)
        if PREC["l0head"] == "f32r" or PREC["rfft1"] == "f32r":
            self.d_dftr = p("dftr", [N, 1024], F32R, isOutput=False)
        if PREC["rfft1"] == "hl":
            self.d_dfthl = p("dfthl", [N, 2048], BF16, isOutput=False)
        self.d_ibr = p("ibr", [1024, N], F32R, isOutput=False)
        self.d_winr = p("winr", [L, D, D], F32R, isOutput=False)
        self.d_woutr = p("woutr", [L, D, D], F32R, isOutput=False)
        if PREC["irfft0"] == "hl":
            self.d_ibhl = p("ibhl", [1024, 2048], BF16, isOutput=False)
        if PREC["mhesa0"] == "hl":
            self.d_winhl = p("winhl", [D, 2 * D], BF16, isOutput=False)
            self.d_wouthl = p("wouthl", [D, 2 * D], BF16, isOutput=False)
        self.d_idn = p("idn", [128, 128], F32, isOutput=False)
        self.d_e8 = p("e8", [HEADS, D], F32, isOutput=False)
        self.d_bout = p("boutr", [L, 1, D], F32, isOutput=False)
        self.d_lcolp = p("lcolp", [L, 128, 8], F32, isOutput=False)
        self.d_al8 = p("alpha8", [L, HEADS, 1], F32, isOutput=False)
        if PREC["ff"] == "f32r":
            self.d_ffw1r = p("ffw1r", [NM, 128, 512], F32R, isOutput=False)
            self.d_ffw2r = p("ffw2r", [FD, D], F32R, isOutput=False)
        else:
            self.d_ffw1t = p("ffw1t", [NM, 128, 2 * ND * 128], BF16,
                             isOutput=False)
            self.d_ffw2hl = p("ffw2hl", [FD, 2 * D], BF16, isOutput=False)
        self.d_cpkp = p("cpkp", [128, 28], F32, isOutput=False)
        self.d_gpost = p("gpostr", [1, D], F32, isOutput=False)
        self.d_bpost = p("bpostr", [1, D], F32, isOutput=False)
        self.d_wg = p("lvwg", [L, D, TF], F32, isOutput=False)
        self.d_wp = p("lvwp", [L, D, TF], F32, isOutput=False)
        self.d_bg = p("lvbg", [L, 1, TF], F32, isOutput=False)
        self.d_bp = p("lvbp", [L, 1, TF], F32, isOutput=False)
        self.d_alv = p("lvalpha", [L, 1, 1], F32, isOutput=False)
        self.d_damp = p("damp8", [HEADS, 1], F32, isOutput=False)
        self.d_outw = p("outw", [128, ND * TF], F32, isOutput=False)
        self.d_outb = p("outbr", [1, TF], F32, isOutput=False)
        self.d_out = p("outT", [S * TF, HOR], F32, isOutput=True)
        self.xtmid = nc.dram_tensor("xtmid", [S, TF, N], F32)

    # psum bank helper: tag-based reuse of the 8 banks
    def bank(self, i, shape=(128, 512), dtype=F32):
        tl = self.psp.tile(list(shape), dtype, tag=f"bk{i}", name=f"bk{i}")
        return tl

    def build(self):
        nc = self.nc
        with ExitStack() as ctx:
            self.tc = ctx.enter_context(tile.TileContext(nc))
            tc = self.tc
            top = ctx.enter_context(tc.tile_pool(name="top", bufs=1))

            idn = top.tile([128, 128], F32, name="idn")
            nc.sync.dma_start(idn[:], self.d_idn[:])
            idnr = top.tile([128, 128], F32R, name="idnr")
            nc.vector.tensor_copy(idnr[:], idn[:])
            self.idnr = idnr
            ones = top.tile([128, 128], F32, name="ones")
            nc.vector.memset(ones[:], 1.0)
            w2d = top.tile([96, D],
                           F32R if PREC["l0head"] == "f32r" else F32,
                           name="w2d")
            nc.sync.dma_start(w2d[:], self.d_w2d[:])
            outbr = top.tile([1, TF], F32, name="outbr")
            nc.sync.dma_start(outbr[:], self.d_outb[:])
            self.outbt = outbr
            # col pack: gpre(4) | bpre(4)
            cpk = top.tile([128, 28], F32, name="cpk")
            nc.sync.dma_start(cpk[:], self.d_cpkp[:])
            outw = top.tile([128, ND * TF], F32, name="outw")
            nc.sync.dma_start(outw[:], self.d_outw[:])
            eps = top.tile([128, 1], F32, name="eps")
            nc.vector.memset(eps[:], 1e-5)
            self.epst = eps
            gbt = top.tile([128, D], F32, name="gbt")
            bbt = top.tile([128, D], F32, name="bbt")
            self.gbt, self.bbt = gbt, bbt
            agg = top.tile([128, ND * HOR], F32, name="agg")
            csd = top.tile([128, ND * HOR], F32, name="csd")

            self.idn, self.ones, self.cpk = idn, ones, cpk
            self.w2dt_, self.aggt, self.csdt = w2d, agg, csd
            self.outwt = outw

            self.psp = ctx.enter_context(
                tc.tile_pool(name="ps", bufs=1, space="PSUM"))
            # both layers' constants resident; samples run L0->L1 back to
            # back so L1's DVE-heavy tail overlaps the next sample's
            # PE-heavy head, and z4 never round-trips through DRAM
            lay0p = ctx.enter_context(tc.tile_pool(name="lay0", bufs=1))
            lay1p = ctx.enter_context(tc.tile_pool(name="lay1", bufs=1))
            with tc.tile_pool(name="ini", bufs=1) as ini:
                e8 = ini.tile([HEADS, D], F32, name="e8")
                nc.sync.dma_start(e8[:], self.d_e8[:])
                self.e8t = e8
                self._damp_cs(ini, self.psp)
                lay = [self._layer_consts(0, lay0p),
                       self._layer_consts(1, lay1p)]
            wk = ctx.enter_context(tc.tile_pool(name="wk", bufs=1))
            for s in range(S):
                z4 = self._sample(0, s, lay[0], wk)
                self._sample(1, s, lay[1], wk, zin=z4)
                self._output(s, wk)

        nc.compile()
        return nc

    # ---------- dampening cumsum -> csd [128, ND*HOR] ----------
    def _damp_cs(self, ini, inips):
        nc = self.nc
        ones = self.ones
        dcol = ini.tile([HEADS, 1], F32, name="dcol")
        nc.sync.dma_start(dcol[:], self.d_damp[:])
        df = ini.tile([HEADS, 1], F32, name="dfsig")
        nc.scalar.activation(df[:], dcol[:], AF.Sigmoid)
        dfb = ini.tile([HEADS, HOR], F32, name="dfb")
        nc.scalar.activation(dfb[:], ones[0:HEADS, 0:HOR], AF.Identity,
                             scale=df[:, 0:1])
        zer = ini.tile([HEADS, HOR], F32, name="zer8")
        nc.vector.memset(zer[:], 0.0)
        dfp = ini.tile([HEADS, HOR], F32, name="dfp")
        nc.vector.tensor_tensor_scan(dfp[:], dfb[:], zer[:], 1.0,
                                     op0=ALU.mult, op1=ALU.add)
        cs8 = ini.tile([HEADS, HOR], F32, name="cs8")
        nc.vector.tensor_tensor_scan(cs8[:], ones[0:HEADS, 0:HOR], dfp[:], 0.0,
                                     op0=ALU.mult, op1=ALU.add)
        for dt in range(ND):
            pini = inips.tile([128, HOR], F32, tag=f"bk{dt}", name="pini")
            nc.tensor.matmul(pini[:], self.e8t[:, _sl(dt)], cs8[:],
                             start=True, stop=True)
            nc.scalar.copy(self.csdt[:, dt * HOR:(dt + 1) * HOR], pini[:])
        # hoisted FF post-LN gamma/beta broadcasts (layer-invariant)
        rows = ini.tile([1, 1024], F32, name="rows")
        nc.sync.dma_start(rows[0:1, 0:512], self.d_gpost[:])
        nc.sync.dma_start(rows[0:1, 512:1024], self.d_bpost[:])
        pgb = inips.tile([128, D], F32, tag="bk4", name="pgb")
        nc.tensor.matmul(pgb[:], self.ones[0:1, 0:128],
                         rows[0:1, 0:512], start=True, stop=True)
        nc.scalar.copy(self.gbt[:], pgb[:])
        pbb = inips.tile([128, D], F32, tag="bk5", name="pbb")
        nc.tensor.matmul(pbb[:], self.ones[0:1, 0:128],
                         rows[0:1, 512:1024], start=True, stop=True)
        nc.scalar.copy(self.bbt[:], pbb[:])

    # ---------- per-layer constants ----------
    def _layer_consts(self, l, layp):
        nc = self.nc
        ones = self.ones
        last = l == L - 1
        lay = {"l": l, "last": last}

        if last or PREC["mhesa0"] == "f32r":
            win = [layp.tile([128, D], F32R, name=f"win{k}") for k in range(ND)]
            wout = [layp.tile([128, D], F32R, name=f"wout{k}")
                    for k in range(ND)]
            for kt in range(ND):
                nc.scalar.dma_start(win[kt][:], self.d_winr[l, _sl(kt), :])
                nc.scalar.dma_start(wout[kt][:], self.d_woutr[l, _sl(kt), :])
        else:
            # bf16 hi|lo packed (cols 0:512 hi, 512:1024 lo)
            win = [layp.tile([128, 2 * D], BF16, name=f"win{k}")
                   for k in range(ND)]
            wout = [layp.tile([128, 2 * D], BF16, name=f"wout{k}")
                    for k in range(ND)]
            for kt in range(ND):
                nc.scalar.dma_start(win[kt][:], self.d_winhl[_sl(kt), :])
                nc.scalar.dma_start(wout[kt][:], self.d_wouthl[_sl(kt), :])

        # lrows: p0 = bout[512]; p32 = bg[7] then bp at cols 16..23
        lrows = layp.tile([128, 512], F32, name="lrows")
        nc.sync.dma_start(lrows[0:1, 0:D], self.d_bout[l, :, :])
        nc.sync.dma_start(lrows[32:33, 0:TF], self.d_bg[l, :, :])
        nc.sync.dma_start(lrows[32:33, 16:16 + TF], self.d_bp[l, :, :])

        # bout broadcast [128, D] (replaces per-tile bias matmuls)
        boutb = layp.tile([128, D], F32, name="boutb")
        pbo = self.psp.tile([128, D], F32, tag="bk7", name="pbo")
        nc.tensor.matmul(pbo[:], ones[0:1, 0:128], lrows[0:1, 0:D],
                         start=True, stop=True)
        nc.scalar.copy(boutb[:], pbo[:])

        # lcol pack [128, 16]: al(4) oma(4) init(4) bi(4); plus lv cols [7,1]
        # cols 18/19: level bg/bp as [7,1] columns
        lcol = layp.tile([128, 24], F32, name="lcol")
        nc.sync.dma_start(lcol[0:TF, 18:19],
                          self.d_bg[l, :, :].rearrange("a b -> b a"))
        nc.sync.dma_start(lcol[0:TF, 19:20],
                          self.d_bp[l, :, :].rearrange("a b -> b a"))
        al8 = layp.tile([HEADS, 1], F32, tag="al8t", name="al8")
        nc.sync.dma_start(al8[:], self.d_al8[l, :, :])
        al8s = layp.tile([HEADS, 1], F32, tag="al8s", name="al8s")
        nc.scalar.activation(al8s[:], al8[:], AF.Sigmoid)
        for dt in range(ND):
            pal = self.psp.tile([128, 1], F32, tag="bk0", name="pal")
            nc.tensor.matmul(pal[:], self.e8t[:, _sl(dt)], al8s[:],
                             start=True, stop=True)
            nc.scalar.copy(lcol[:, dt:dt + 1], pal[:])
        nc.sync.dma_start(lcol[:, 8:16], self.d_lcolp[l, :, :])
        for dt in range(ND):
            nc.vector.tensor_scalar(lcol[:, 4 + dt:5 + dt], lcol[:, dt:dt + 1],
                                    -1.0, 1.0, op0=ALU.mult, op1=ALU.add)
        nc.vector.tensor_sub(lcol[:, 12:16], lcol[:, 12:16], lcol[:, 8:12])
        # col 20:24 = al*(bi-init) + (1-al)*init -- the scan-initial folded
        # into xd[0] so the scan can run with a 0.0 immediate initial
        bi = layp.tile([128, ND], F32, tag="bitmp", name="bitmp")
        nc.vector.tensor_mul(lcol[:, 20:24], lcol[:, 0:4], lcol[:, 12:16])
        nc.vector.tensor_mul(bi[:], lcol[:, 4:8], lcol[:, 8:12])
        nc.vector.tensor_add(lcol[:, 20:24], lcol[:, 20:24], bi[:])
        # level alpha
        alv = layp.tile([1, 1], F32, tag="alvt", name="alv")
        nc.sync.dma_start(alv[:], self.d_alv[l, :, :])
        alvs = layp.tile([1, 1], F32, tag="alvst", name="alvs")
        nc.scalar.activation(alvs[:], alv[:], AF.Sigmoid)
        pv = self.psp.tile([TF, 1], F32, tag="bk1", name="palv")
        nc.tensor.matmul(pv[:], ones[0:1, 0:TF], alvs[:], start=True, stop=True)
        nc.scalar.copy(lcol[0:TF, 16:17], pv[:])
        nc.vector.tensor_scalar(lcol[0:TF, 17:18], lcol[0:TF, 16:17], -1.0, 1.0,
                                op0=ALU.mult, op1=ALU.add)

        # level weights [128, TF] x4 packed [128, 2*ND*TF], as fp32r
        lwf = layp.tile([128, 2 * ND * TF], F32, tag="lwf", name="lwf")
        for kt in range(ND):
            nc.sync.dma_start(lwf[:, kt * TF:(kt + 1) * TF], self.d_wg[l, _sl(kt), :])
            nc.sync.dma_start(lwf[:, (ND + kt) * TF:(ND + kt + 1) * TF],
                              self.d_wp[l, _sl(kt), :])
        lw = layp.tile([128, 2 * ND * TF], F32R, name="lw")
        nc.vector.tensor_copy(lw[:], lwf[:])

        lay.update(win=win, wout=wout, lrows=lrows, lcol=lcol, lw=lw,
                   boutb=boutb)
        return lay

    # ---------- one sample through one layer ----------
    def _sample(self, l, s, lay, wk, zin=None):
        nc = self.nc
        ones, idn = self.ones, self.idn
        last = lay["last"]
        agg = self.aggt
        irf_r = last or PREC["irfft0"] == "f32r"
        mh_r = last or PREC["mhesa0"] == "f32r"

        def aggsl(dt):
            return self.aggt[:, dt * HOR:(dt + 1) * HOR]

        # --- z input: conv (l0) or handed over in SBUF from l0 (l1)
        if l == 0:
            hr = PREC["l0head"] == "f32r"
            # agg is per-sample now; clear it (waits on prior _output read)
            nc.gpsimd.memset(agg[:], 0.0)
            z = [wk.tile([128, D], F32R, tag=f"B1_{tt}", name=f"z{tt}")
                 for tt in range(NT)]
            # low-rank path: x is rank-7, so z = xsh^T @ w2d (rows 32k+c hold
            # the 3 shifts of the 7 channels; row 95 = ones * conv_b) and
            # DFT(z) = w2d^T @ (xsh^T @ dft) -- the DFT runs in the 96-dim
            # input space instead of the 512-dim channel space.
            xshf = wk.tile([96, N], F32, tag="xsh", name="xshf")
            xts = wk.tile([TF, N], F32, tag="xts", name="xts")
            nc.sync.dma_start(xts[:], self.d_xT[s * TF:(s + 1) * TF, :])
            nc.gpsimd.memset(xshf[:], 0.0)
            nc.gpsimd.tensor_copy(xshf[0:TF, 1:N], xts[:, 0:N - 1])
            nc.gpsimd.tensor_copy(xshf[32:32 + TF, 0:N], xts[:, 0:N])
            nc.gpsimd.tensor_copy(xshf[64:64 + TF, 0:N - 1], xts[:, 1:N])
            nc.sync.dma_start(xshf[95:96, :], self.d_ones1[:])
            if hr:
                # Pool can't touch f32r (ISA); one DVE copy re-tags for PE
                xsh = wk.tile([96, N], F32R, tag="xshr", name="xsh")
                nc.vector.tensor_copy(xsh[:], xshf[:])
            else:
                xsh = xshf
            xshT = [wk.tile([128, 96], F32R if hr else F32,
                            tag=f"xshT{tt}", name=f"xshT{tt}")
                    for tt in range(NT)]
            psF1A = self.bank(2, shape=(96, 512))
            psF1B = self.bank(3, shape=(96, 512))
            tid = self.idnr if hr else idn
            d_dft_src = self.d_dftr if hr else self.d_dft
            for tt in range(NT):
                pz = self.bank(tt % 2)
                nc.tensor.matmul(pz[:], xsh[:, _sl(tt)], self.w2dt_[:],
                                 start=True, stop=True)
                nc.scalar.copy(z[tt][:], pz[:])
                pxT = self.bank(6, shape=(128, 96),
                                dtype=F32R if hr else F32)
                nc.tensor.transpose(pxT[:], xsh[:, _sl(tt)], tid[0:96, 0:96])
                nc.scalar.copy(xshT[tt][:], pxT[:])
                dftk = wk.tile([128, 1024], F32R if hr else F32,
                               tag=f"dftk{tt % 2}", name="dftk")
                nc.sync.dma_start(dftk[:], d_dft_src[_sl(tt), :])
                nc.tensor.matmul(psF1A[:], xshT[tt][:], dftk[:, 0:512],
                                 start=(tt == 0), stop=(tt == NT - 1))
                nc.tensor.matmul(psF1B[:], xshT[tt][:], dftk[:, 512:1024],
                                 start=(tt == 0), stop=(tt == NT - 1))
            F1s = wk.tile([96, 1024], F32R if hr else F32, tag="lvp",
                          name="F1s")
            nc.scalar.copy(F1s[:, 0:512], psF1A[:])
            nc.scalar.copy(F1s[:, 512:1024], psF1B[:])
            psA = [self.bank(ct) for ct in range(ND)]
            psB = [self.bank(4 + ct) for ct in range(ND)]
            for ct in range(ND):
                nc.tensor.matmul(psA[ct][:], self.w2dt_[:, _sl(ct)],
                                 F1s[:, 0:512], start=True, stop=True)
                nc.tensor.matmul(psB[ct][:], self.w2dt_[:, _sl(ct)],
                                 F1s[:, 512:1024], start=True, stop=True)
            ibkpf = []
            if irf_r:
                for pf in range(2):
                    ibkp = wk.tile([128, 1024], F32R, tag=f"dftk{pf % 2}",
                                   name="ibk")
                    nc.sync.dma_start(ibkp[:], self.d_ibr[_sl(pf), :])
                    ibkpf.append(ibkp)
            else:
                for pf in range(2):
                    ibkp = wk.tile([128, 2048], BF16, tag=f"dftk{pf % 2}",
                                   name="ibk")
                    nc.sync.dma_start(ibkp[:], self.d_ibhl[_sl(pf), :])
                    ibkpf.append(ibkp)
        else:
            z, zhl = zin

            psA = [self.bank(ct) for ct in range(ND)]
            psB = [self.bank(4 + ct) for ct in range(ND)]
            if PREC["rfft1"] == "f32r":
                # z tiles are F32R [t, d]; stationary slice [t, c-block]
                pfs = getattr(self, "_dftk_pf", None)
                self._dftk_pf = None
                for kt in range(NT):
                    if pfs is not None and kt < 2:
                        dftk = pfs[kt]
                    else:
                        dftk = wk.tile([128, 1024], F32R, tag=f"dftk{kt % 2}",
                                       name="dftk")
                        nc.sync.dma_start(dftk[:], self.d_dftr[_sl(kt), :])
                    st0 = kt == 0
                    sp = kt == NT - 1
                    for ct in range(ND):
                        zst = z[kt][:, _sl(ct)]
                        nc.tensor.matmul(psA[ct][:], zst, dftk[:, 0:512],
                                         start=st0, stop=sp)
                        nc.tensor.matmul(psB[ct][:], zst, dftk[:, 512:1024],
                                         start=st0, stop=sp)
            else:
                # rfft via bf16 hi/lo 3-term split (exact to ~2^-17)
                for kt in range(NT):
                    dftk = wk.tile([128, 2048], BF16, tag=f"dftk{kt % 2}",
                                   name="dftk")
                    nc.sync.dma_start(dftk[:], self.d_dfthl[_sl(kt), :])
                    st0 = kt == 0
                    sp = kt == NT - 1
                    for ct in range(ND):
                        zh = zhl[kt][:, _sl(ct)]
                        zl = zhl[kt][:, 512 + 128 * ct:640 + 128 * ct]
                        nc.tensor.matmul(psA[ct][:], zh, dftk[:, 0:512],
                                         start=st0, stop=False)
                        nc.tensor.matmul(psA[ct][:], zh, dftk[:, 1024:1536],
                                         start=False, stop=False)
                        nc.tensor.matmul(psB[ct][:], zh, dftk[:, 512:1024],
                                         start=st0, stop=False)
                        nc.tensor.matmul(psB[ct][:], zh, dftk[:, 1536:2048],
                                         start=False, stop=False)
                        nc.tensor.matmul(psA[ct][:], zl, dftk[:, 0:512],
                                         start=False, stop=sp)
                        nc.tensor.matmul(psB[ct][:], zl, dftk[:, 512:1024],
                                         start=False, stop=sp)
            # prefetch the first two irfft ib stripes while the mask runs
            ibkpf = []
            for pf in range(2):
                ibkp = wk.tile([128, 1024], F32R, tag=f"dftk{pf % 2}",
                               name="ibk")
                nc.sync.dma_start(ibkp[:], self.d_ibr[_sl(pf), :])
                ibkpf.append(ibkp)

        # --- top-4 mask -> filt [ND][128, 1024] ([c, f])
        # Pool has no PSUM port: psA/psB land in SBUF once (ACT), then the
        # whole chain (squares, add, is_ge mask) runs on the idle Pool
        # engine; only the top-8 max needs DVE.
        filt = [wk.tile([128, 1024], F32R if irf_r else F32,
                        tag=f"A1_{ct}", name=f"filt{ct}")
                for ct in range(ND)]
        for ct in range(ND):
            amp2 = wk.tile([128, 513], F32,
                           tag="amp2" if ct % 2 == 0 else "lnscr", name="amp2")
            nc.scalar.activation(amp2[:, 0:512], psA[ct][:], AF.Square)
            sqB = wk.tile([128, 512], F32,
                          tag="w2m0" if ct % 2 == 0 else "w2m1", name="sqB")
            nc.scalar.activation(sqB[:], psB[ct][:], AF.Square)
            # permuted B-half: amp2[f] = sqA[f] + sqB[f-1], amp2[512]=sqB[511]
            nc.gpsimd.tensor_add(amp2[:, 1:512], amp2[:, 1:512], sqB[:, 0:511])
            nc.gpsimd.tensor_copy(amp2[:, 512:513], sqB[:, 511:512])
            top8 = wk.tile([128, 8], F32, tag="top8", name="top8")
            nc.vector.max(top8[:], amp2[:])
            kth = top8[:, 3:4]
            nc.vector.scalar_tensor_tensor(filt[ct][:, 0:512], amp2[:, 0:512],
                                           kth, psA[ct][:],
                                           op0=ALU.is_ge, op1=ALU.mult)
            nc.vector.scalar_tensor_tensor(filt[ct][:, 512:1024], amp2[:, 1:513],
                                           kth, psB[ct][:],
                                           op0=ALU.is_ge, op1=ALU.mult)

        # --- transpose filt -> filtT [f, c]; hl splits to bf16 hi|lo
        if irf_r:
            filtT = [wk.tile([128, 512], F32R, tag=f"B2_{ft}",
                             name=f"filtT{ft}") for ft in range(NT)]
            for ft in range(NT):
                pT = self.bank(ft % 4, dtype=F32R)
                for ct in range(ND):
                    nc.tensor.transpose(pT[:, _sl(ct)], filt[ct][:, _sl(ft)],
                                        self.idnr[:])
                if ft % 2 == 0:
                    nc.scalar.copy(filtT[ft][:], pT[:])
                else:
                    nc.vector.tensor_copy(filtT[ft][:], pT[:])
        else:
            filtT = [wk.tile([128, 1024], BF16, tag=f"B2_{ft}",
                             name=f"fthl{ft}") for ft in range(NT)]
            for ft in range(NT):
                pT = self.bank(ft % 4)
                for ct in range(ND):
                    nc.tensor.transpose(pT[:, _sl(ct)], filt[ct][:, _sl(ft)],
                                        idn[:])
                nc.scalar.copy(filtT[ft][:, 0:512], pT[:])
                nc.vector.tensor_sub(filtT[ft][:, 512:1024], pT[:],
                                     filtT[ft][:, 0:512])

        # --- irfft (ib streamed, 8 banks) -> lp, z2
        pl = [self.bank(tt) for tt in range(NT)]
        if irf_r:
            for ft in range(NT):
                if ft < 2:
                    ibk = ibkpf[ft]
                else:
                    ibk = wk.tile([128, 1024], F32R, tag=f"dftk{ft % 2}",
                                  name="ibk")
                    nc.sync.dma_start(ibk[:], self.d_ibr[_sl(ft), :])
                for tt in range(NT):
                    nc.tensor.matmul(pl[tt][:], ibk[:, _sl(tt)], filtT[ft][:],
                                     start=(ft == 0), stop=(ft == NT - 1))
        else:
            for ft in range(NT):
                if ft < 2:
                    ibk = ibkpf[ft]
                else:
                    ibk = wk.tile([128, 2048], BF16, tag=f"dftk{ft % 2}",
                                  name="ibk")
                    nc.sync.dma_start(ibk[:], self.d_ibhl[_sl(ft), :])
                for tt in range(NT):
                    ibh = ibk[:, _sl(tt)]
                    ibl = ibk[:, 1024 + 128 * tt:1152 + 128 * tt]
                    nc.tensor.matmul(pl[tt][:], ibh, filtT[ft][:, 0:512],
                                     start=(ft == 0), stop=False)
                    nc.tensor.matmul(pl[tt][:], ibh, filtT[ft][:, 512:1024],
                                     start=False, stop=False)
                    nc.tensor.matmul(pl[tt][:], ibl, filtT[ft][:, 0:512],
                                     start=False, stop=(ft == NT - 1))
        lp = [wk.tile([128, D], F32R, tag=f"B3_{tt}", name=f"lp{tt}")
              for tt in range(NT)]
        z2 = [wk.tile([128, D], F32R if mh_r else F32,
                      tag=f"B4_{tt}", name=f"z2_{tt}")
              for tt in range(NT)]
        for tt in range(NT):
            # z2 before lp: in l1 the lp tiles reuse z's memory (tag B3)
            nc.vector.tensor_sub(z2[tt][:], z[tt][:], pl[tt][:])
            nc.scalar.copy(lp[tt][:], pl[tt][:])

        # --- lpT [ND][128, N] (tag A2) + extrap + perT; then free
        lpT = [wk.tile([128, N], F32R, tag=f"A2_{dt}", name=f"lpT{dt}")
               for dt in range(ND)]
        perT = wk.tile([TF, N], F32, tag="dftk0", name="perT")
        for h in range(2):
            for dt in range(ND):
                pT = self.bank(dt, dtype=F32R)
                for q in range(4):
                    nc.tensor.transpose(pT[:, _sl(q)], lp[h * 4 + q][:, _sl(dt)],
                                        self.idnr[:])
                if h == 0:
                    nc.vector.tensor_copy(lpT[dt][:, _hh(h)], pT[:])
                    nc.vector.tensor_add(aggsl(dt), aggsl(dt),
                                         lpT[dt][:, 0:HOR])
                else:
                    nc.vector.tensor_copy(lpT[dt][:, _hh(h)], pT[:])
            # perT for this half right away: fills the PE wait on the next
            # half's lp copies
            pp = self.bank(4 + h)
            for kt in range(ND):
                nc.tensor.matmul(pp[0:TF, :], lay["lw"][:, (ND + kt) * TF:(ND + kt + 1) * TF],
                                 lpT[kt][:, _hh(h)],
                                 start=(kt == 0), stop=(kt == ND - 1))
            nc.scalar.copy(perT[:, _hh(h)], pp[0:TF, :])

        # --- z2T (tag A2 reuse after lpT dead); hl packs bf16 hi|lo
        if mh_r:
            z2T = [wk.tile([128, N], F32R, tag=f"A2_{dt}", name=f"z2T{dt}")
                   for dt in range(ND)]
            for h in range(2):
                for dt in range(ND):
                    pT = self.bank(dt, dtype=F32R)
                    for q in range(4):
                        nc.tensor.transpose(pT[:, _sl(q)],
                                            z2[h * 4 + q][:, _sl(dt)],
                                            self.idnr[:])
                    nc.vector.tensor_copy(z2T[dt][:, _hh(h)], pT[:])
        else:
            z2T = [wk.tile([128, 2 * N], BF16, tag=f"A2_{dt}",
                           name=f"z2Thl{dt}") for dt in range(ND)]
            for h in range(2):
                for dt in range(ND):
                    pT = self.bank(dt)
                    for q in range(4):
                        nc.tensor.transpose(pT[:, _sl(q)],
                                            z2[h * 4 + q][:, _sl(dt)], idn[:])
                    nc.scalar.copy(z2T[dt][:, _hh(h)], pT[:])
                    nc.vector.tensor_sub(
                        z2T[dt][:, N + 512 * h:N + 512 * h + 512], pT[:],
                        z2T[dt][:, _hh(h)])

        # --- win GEMM -> xinT -> xd -> scan, interleaved per dt so the
        # serial DVE scan chain overlaps the next dt's win GEMMs on PE
        xinT = [wk.tile([128, N], F32, tag=f"A1_{dt}", name=f"xinT{dt}")
                for dt in range(ND)]
        lc = lay["lcol"]
        if mh_r:
            sT = [wk.tile([128, N], F32R, tag=f"A2_{dt}", name=f"sT{dt}")
                  for dt in range(ND)]
            sTsc = sT
            for dt in range(ND):
                for h in range(2):
                    px = self.bank(4 + h)
                    for kt in range(ND):
                        nc.tensor.matmul(px[:], lay["win"][kt][:, _sl(dt)],
                                         z2T[kt][:, _hh(h)],
                                         start=(kt == 0), stop=(kt == ND - 1))
                    # fold the per-head alpha scale into the psum->sbuf copy
                    nc.scalar.activation(xinT[dt][:, _hh(h)], px[:],
                                         AF.Identity,
                                         scale=lay["lcol"][:, dt:dt + 1])
                eng = nc.vector if dt % 2 == 0 else nc.gpsimd
                xd = wk.tile([128, N], F32,
                             tag="xdsc0" if dt % 2 == 0 else "xdsc1",
                             name="xd")
                eng.tensor_sub(xd[:, 1:N], xinT[dt][:, 1:N],
                               xinT[dt][:, 0:N - 1])
                # xinT is pre-scaled by alpha; col 20+dt folds the initial
                nc.vector.tensor_scalar_add(xd[:, 0:1], xinT[dt][:, 0:1],
                                            lc[:, 20 + dt:21 + dt])
                omab_ap = lc[:, 4 + dt:5 + dt].broadcast_to([128, N])
                nc.vector.tensor_tensor_scan(sTsc[dt][:], omab_ap, xd[:], 0.0,
                                             op0=ALU.mult, op1=ALU.add)
        else:
            for h in range(2):
                for dt in range(ND):
                    px = self.bank(4 + dt % 2)
                    for kt in range(ND):
                        wh = lay["win"][kt][:, _sl(dt)]
                        wl = lay["win"][kt][:, 512 + 128 * dt:640 + 128 * dt]
                        zh = z2T[kt][:, _hh(h)]
                        zl = z2T[kt][:, N + 512 * h:N + 512 * h + 512]
                        nc.tensor.matmul(px[:], wh, zh,
                                         start=(kt == 0), stop=False)
                        nc.tensor.matmul(px[:], wh, zl,
                                         start=False, stop=False)
                        nc.tensor.matmul(px[:], wl, zh,
                                         start=False, stop=(kt == ND - 1))
                    nc.scalar.activation(xinT[dt][:, _hh(h)], px[:],
                                         AF.Identity,
                                         scale=lay["lcol"][:, dt:dt + 1])
            sTsc = [wk.tile([128, N], F32, tag=f"A1_{dt}", name=f"sTf{dt}")
                    for dt in range(ND)]
            sT = [wk.tile([128, 2 * N], BF16, tag=f"A2_{dt}",
                          name=f"sThl{dt}") for dt in range(ND)]
            for dt in range(ND):
                eng = nc.vector if dt % 2 == 0 else nc.gpsimd
                xd = wk.tile([128, N], F32,
                             tag="xdsc0" if dt % 2 == 0 else "xdsc1",
                             name="xd")
                eng.tensor_sub(xd[:, 1:N], xinT[dt][:, 1:N],
                               xinT[dt][:, 0:N - 1])
                nc.vector.tensor_scalar_add(xd[:, 0:1], xinT[dt][:, 0:1],
                                            lc[:, 20 + dt:21 + dt])
                omab_ap = lc[:, 4 + dt:5 + dt].broadcast_to([128, N])
                nc.vector.tensor_tensor_scan(sTsc[dt][:], omab_ap, xd[:], 0.0,
                                             op0=ALU.mult, op1=ALU.add)
                eng.tensor_copy(sT[dt][:, 0:N], sTsc[dt][:])
                eng.tensor_sub(sT[dt][:, N:2 * N], sTsc[dt][:],
                               sT[dt][:, 0:N])

        # --- wout GEMM -> lg [t,d] (tag B2 reuse: filtT dead) (+ z3 if l0)
        # pre-LN stats chains interleave per tt right behind the z3 subs so
        # DVE starts them 8 tiles earlier than a post-wout batch would
        prep = None
        if not last and PREC["ff"] == "f32r":
            stpre = wk.tile([128, 8 * NT], F32, tag="stpre", name="stpre")
            h_ = [wk.tile([128, D], F32R, tag=f"B4_{tt}", name=f"h{tt}")
                  for tt in range(NT)]

            def prep(tt):
                scr = wk.tile([128, D], F32,
                              tag="lnscr" if tt % 2 == 0 else "lnscr2",
                              name="lnscr")
                st = stpre
                mu = st[:, tt:tt + 1]
                s2 = st[:, NT + tt:NT + tt + 1]
                nc.vector.tensor_reduce(mu, z[tt][:], mybir.AxisListType.X,
                                        op=ALU.add)
                nc.scalar.activation(scr[:], z[tt][:], AF.Square, accum_out=s2)
                mun = st[:, 2 * NT + tt:2 * NT + tt + 1]
                nc.vector.tensor_scalar_mul(mun, mu, 1.0 / D)
                musq = st[:, 3 * NT + tt:3 * NT + tt + 1]
                nc.scalar.activation(musq, mun, AF.Square)
                var = st[:, 4 * NT + tt:4 * NT + tt + 1]
                nc.vector.scalar_tensor_tensor(var, s2, 1.0 / D, musq,
                                               op0=ALU.mult, op1=ALU.subtract)
                sd = st[:, 5 * NT + tt:5 * NT + tt + 1]
                nc.scalar.activation(sd, var, AF.Sqrt, bias=self.epst[:, 0:1])
                rs = st[:, 6 * NT + tt:6 * NT + tt + 1]
                nc.vector.reciprocal(rs, sd)
                nmurs = st[:, 7 * NT + tt:7 * NT + tt + 1]
                nc.vector.scalar_tensor_tensor(nmurs, mun, -1.0, rs,
                                               op0=ALU.mult, op1=ALU.mult)
                nc.scalar.activation(h_[tt][:], z[tt][:], AF.Identity,
                                     scale=rs, bias=nmurs)
        lg = [wk.tile([128, D], F32R, tag=f"B2_{tt}", name=f"lg{tt}")
              for tt in range(NT)]
        for tt in range(NT):
            pg = self.bank(tt % 2)
            if mh_r:
                for kt in range(ND):
                    nc.tensor.matmul(pg[:], sT[kt][:, _sl(tt)],
                                     lay["wout"][kt][:],
                                     start=(kt == 0), stop=(kt == ND - 1))
            else:
                for kt in range(ND):
                    sh = sT[kt][:, _sl(tt)]
                    sl_ = sT[kt][:, N + 128 * tt:N + 128 * tt + 128]
                    nc.tensor.matmul(pg[:], sh, lay["wout"][kt][:, 0:512],
                                     start=(kt == 0), stop=False)
                    nc.tensor.matmul(pg[:], sh, lay["wout"][kt][:, 512:1024],
                                     start=False, stop=False)
                    nc.tensor.matmul(pg[:], sl_, lay["wout"][kt][:, 0:512],
                                     start=False, stop=(kt == ND - 1))
            nc.vector.tensor_add(lg[tt][:], pg[:], lay["boutb"][:])
            if not last:
                # z3 overwrites z (tag B1): z dead after z2
                nc.vector.tensor_sub(z[tt][:], z2[tt][:], lg[tt][:])
                if prep is not None:
                    prep(tt)
        z3 = z

        def emit_tail():
            # lglast/lgT/grT/damp/level-step. For l0 this is DEFERRED into
            # the FF (emitted after the h0 GEMM loop) so its PE work (lgT,
            # grT on banks 2/3) and DVE work overlap the FF GEMMs instead
            # of stalling the pre-LN stats chain.
            lglast = wk.tile([1, D], F32, tag="sqA", name="lglast")
            nc.gpsimd.dma_start(lglast[:], lg[NT - 1][127:128, :])
            lgl4 = wk.tile([128, ND], F32, tag="top8", name="lgl4")
            pTl = self.bank(2, shape=(128, ND))
            for dt in range(ND):
                nc.tensor.matmul(pTl[:, dt:dt + 1], lglast[0:1, _sl(dt)],
                                 ones[0:1, 0:1], start=True, stop=True)
            nc.scalar.copy(lgl4[:], pTl[:])

            # lgT via transposes (tag A1 reuse: xinT dead)
            lgT = [wk.tile([128, N], F32R, tag=f"A1_{dt}", name=f"lgT{dt}")
                   for dt in range(ND)]
            for h in range(2):
                for dt in range(ND):
                    pT = self.bank(2 + dt % 2, dtype=F32R)
                    for q in range(4):
                        nc.tensor.transpose(pT[:, _sl(q)],
                                            lg[h * 4 + q][:, _sl(dt)],
                                            self.idnr[:])
                    if h == 0:
                        nc.scalar.copy(lgT[dt][:, _hh(h)], pT[:])
                    else:
                        nc.vector.tensor_copy(lgT[dt][:, _hh(h)], pT[:])
            for dt in range(ND):
                # damp: agg += lg_last * csd
                nc.vector.scalar_tensor_tensor(
                    aggsl(dt), self.csdt[:, dt * HOR:(dt + 1) * HOR],
                    lgl4[:, dt:dt + 1], aggsl(dt), op0=ALU.mult, op1=ALU.add)

            # level: grT; scans update xtmid
            grT = wk.tile([TF, N], F32, tag="grT", name="grT")
            for h in range(2):
                pgr = self.bank(2 + h)
                for kt in range(ND):
                    nc.tensor.matmul(pgr[0:TF, :],
                                     lay["lw"][:, kt * TF:(kt + 1) * TF],
                                     lgT[kt][:, _hh(h)],
                                     start=(kt == 0), stop=(kt == ND - 1))
                # fold level bg bias (lcol col 18) into the psum->sbuf copy
                nc.vector.tensor_scalar_add(grT[:, _hh(h)], pgr[0:TF, :],
                                            lc[0:TF, 18:19])

            xts2 = wk.tile([TF, N], F32, tag="xts", name="xts2")
            if l == 0:
                nc.sync.dma_start(xts2[:], self.d_xT[s * TF:(s + 1) * TF, :])
            else:
                nc.sync.dma_start(xts2[:], self.xtmid[s, :, :])
            v = wk.tile([TF, N], F32, tag="lvv", name="lvv")
            # v = (xts2 - bp) - perT (DVE: Pool has no TensorScalarPtr)
            nc.vector.scalar_tensor_tensor(v[:], xts2[:], lc[0:TF, 19:20],
                                           perT[:],
                                           op0=ALU.subtract, op1=ALU.subtract)
            nc.vector.tensor_scalar_mul(v[:], v[:], lc[0:TF, 16:17])
            omlv_ap = lc[0:TF, 17:18].broadcast_to([TF, N])
            pt = wk.tile([TF, N], F32, tag="lvp", name="lvp")
            nc.vector.tensor_tensor_scan(pt[:], omlv_ap, v[:], 0.0,
                                         op0=ALU.mult, op1=ALU.add)
            gt = wk.tile([TF, N], F32, tag="lvv", name="lvg")
            nc.vector.tensor_tensor_scan(gt[:], omlv_ap, grT[:], 0.0,
                                         op0=ALU.mult, op1=ALU.add)
            xnew = wk.tile([TF, N], F32, tag="grT", name="xnew")
            nc.gpsimd.tensor_add(xnew[:], pt[:], gt[:])
            # on Pool: keeps this late-blocking store off the DMA queues
            nc.gpsimd.dma_start(self.xtmid[s, :, :], xnew[:])
            if l == 0 and PREC["rfft1"] == "f32r":
                # prefetch l1's first two rfft dft stripes on the ACT hwdge
                # queue: the SP queue is still draining FF w1/w2 triggers
                pfs = []
                for i in range(2):
                    t = wk.tile([128, 1024], F32R, tag=f"dftk{i}",
                                name="dftkpf")
                    nc.scalar.dma_start(t[:], self.d_dftr[_sl(i), :])
                    pfs.append(t)
                self._dftk_pf = pfs

        # --- FF (layer 0 only); z4 stays in SBUF for l1
        if not last:
            return self._ff(s, z3, wk, emit_tail,
                            h_ if prep is not None else None)
        emit_tail()
        return None

    # ---------- LN stats ----------
    def _ln_stats(self, zset, wk, tagp):
        nc = self.nc
        st = wk.tile([128, 8 * NT], F32, tag=f"st{tagp}", name=f"st{tagp}")
        mu8 = st[:, 0:NT]
        s28 = st[:, NT:2 * NT]
        for tt in range(NT):
            scr = wk.tile([128, D], F32,
                          tag="lnscr" if tt % 2 == 0 else "lnscr2",
                          name="lnscr")
            nc.vector.tensor_reduce(st[:, tt:tt + 1], zset[tt][:],
                                    mybir.AxisListType.X, op=ALU.add)
            nc.scalar.activation(scr[:], zset[tt][:], AF.Square,
                                 accum_out=st[:, NT + tt:NT + tt + 1])
        mun = st[:, 2 * NT:3 * NT]
        nc.vector.tensor_scalar_mul(mun, mu8, 1.0 / D)
        ex2 = st[:, 3 * NT:4 * NT]
        nc.vector.tensor_scalar_mul(ex2, s28, 1.0 / D)
        musq = st[:, 4 * NT:5 * NT]
        nc.scalar.activation(musq, mun, AF.Square)
        var = st[:, 5 * NT:6 * NT]
        nc.vector.tensor_sub(var, ex2, musq)
        sd = st[:, 6 * NT:7 * NT]
        nc.scalar.activation(sd, var, AF.Sqrt, bias=self.epst[:, 0:1])
        rs = st[:, 7 * NT:8 * NT]
        nc.vector.reciprocal(rs, sd)
        nmurs = st[:, 4 * NT:5 * NT]  # overwrite musq slot
        nc.vector.tensor_mul(nmurs, mun, rs)
        nc.vector.tensor_scalar_mul(nmurs, nmurs, -1.0)
        return rs, nmurs

    # ---------- FF block ----------
    def _ff(self, s, z3, wk, tail, h_=None):
        if PREC["ff"] == "f32r":
            return self._ff_f32r(s, z3, wk, tail, h_)
        return self._ff_hl(s, z3, wk, tail)

    def _ff_f32r(self, s, z3, wk, tail, h_):
        nc = self.nc
        cpk = self.cpk
        # h_ (pre-LN normalized tiles) were produced per-tt inside the wout
        # loop by _sample's prep closure
        hT = [wk.tile([128, N], F32R, tag=f"A2_{dt}", name=f"hT{dt}")
              for dt in range(ND)]
        znT = [wk.tile([128, N], F32R, tag=f"A1_{dt}", name=f"znT{dt}")
               for dt in range(ND)]
        for h in range(2):
            for dt in range(ND):
                pT = self.bank(dt, dtype=F32R)
                for q in range(4):
                    nc.tensor.transpose(pT[:, _sl(q)], h_[h * 4 + q][:, _sl(dt)],
                                        self.idnr[:])
                if h == 0:
                    nc.scalar.copy(hT[dt][:, _hh(h)], pT[:])
                else:
                    nc.vector.tensor_copy(hT[dt][:, _hh(h)], pT[:])
                # znT per (h, dt) immediately: the first w1 matmul only
                # needs the four h0 halves
                nc.vector.tensor_scalar(znT[dt][:, _hh(h)], hT[dt][:, _hh(h)],
                                        cpk[:, dt:dt + 1],
                                        cpk[:, 4 + dt:5 + dt],
                                        op0=ALU.mult, op1=ALU.add)

        yT = [wk.tile([128, N], F32R, tag=f"A2_{dt}", name=f"yT{dt}")
              for dt in range(ND)]
        for h in range(2):
            pzf = [self.bank(b) for b in (0, 1, 6, 7)]
            # software-pipelined: w2(m-1) is emitted AFTER w1(m), so the PE
            # never sits head-of-line waiting on sig(m-1)'s ACT latency
            sigs = [None, None]
            w2ms = [None, None]

            def w2_stage(m):
                for dt in range(ND):
                    nc.tensor.matmul(pzf[dt][:], w2ms[m % 2][:, _sl(dt)],
                                     sigs[m % 2][:],
                                     start=(m == 0), stop=(m == NM - 1))

            for m in range(NM):
                w1m = wk.tile([128, 512], F32R, tag=f"w1mh{m % 2}", name="w1m")
                nc.sync.dma_start(w1m[:], self.d_ffw1r[m, :, :])
                ph = self.bank(4 + m % 2)
                for kt in range(ND):
                    nc.tensor.matmul(ph[:], w1m[:, _sl(kt)],
                                     znT[kt][:, _hh(h)],
                                     start=(kt == 0), stop=(kt == ND - 1))
                if m > 0:
                    w2_stage(m - 1)
                sig = wk.tile([128, 512], F32R, tag=f"sig{m % 2}", name="sig")
                nc.scalar.activation(sig[:], ph[:], AF.Sigmoid,
                                     bias=cpk[:, 8 + m:9 + m])
                sigs[m % 2] = sig
                w2m = wk.tile([128, 512], F32R, tag=f"w2m{m % 2}", name="w2m")
                nc.sync.dma_start(w2m[:], self.d_ffw2r[_sl(m), :])
                w2ms[m % 2] = w2m
            w2_stage(NM - 1)
            for dt in range(ND):
                nc.vector.scalar_tensor_tensor(yT[dt][:, _hh(h)], pzf[dt][:],
                                               cpk[:, 24 + dt:25 + dt],
                                               znT[dt][:, _hh(h)],
                                               op0=ALU.add, op1=ALU.add)
            if h == 0:
                tail()
        return self._post_ln(s, yT, wk, yr=True)

    def _ff_hl(self, s, z3, wk, tail):
        nc = self.nc
        idn = self.idn
        cpk = self.cpk
        tail()
        rs, nmurs = self._ln_stats(z3, wk, "pre")
        h_ = [wk.tile([128, D], F32, tag=f"B2_{tt}", name=f"h{tt}")
              for tt in range(NT)]
        for tt in range(NT):
            nc.scalar.activation(h_[tt][:], z3[tt][:], AF.Identity,
                                 scale=rs[:, tt:tt + 1], bias=nmurs[:, tt:tt + 1])
        hT = [wk.tile([128, N], F32, tag=f"A2_{dt}", name=f"hT{dt}")
              for dt in range(ND)]
        for h in range(2):
            for dt in range(ND):
                pT = self.bank(dt)
                for q in range(4):
                    nc.tensor.transpose(pT[:, _sl(q)], h_[h * 4 + q][:, _sl(dt)],
                                        idn[:])
                if h == 0:
                    nc.scalar.copy(hT[dt][:, _hh(h)], pT[:])
                else:
                    nc.vector.tensor_copy(hT[dt][:, _hh(h)], pT[:])
        znT = [wk.tile([128, N], F32, tag=f"A1_{dt}", name=f"znT{dt}")
               for dt in range(ND)]
        for h in range(2):
            for dt in range(ND):
                nc.vector.tensor_scalar(znT[dt][:, _hh(h)], hT[dt][:, _hh(h)],
                                        cpk[:, dt:dt + 1],
                                        cpk[:, 4 + dt:5 + dt],
                                        op0=ALU.mult, op1=ALU.add)

        yT = [wk.tile([128, N], F32, tag=f"A2_{dt}", name=f"yT{dt}")
              for dt in range(ND)]
        for h in range(2):
            znb = [wk.tile([128, 1024], BF16, tag=f"B3_{kt}", name=f"znb{kt}")
                   for kt in range(ND)]
            for kt in range(ND):
                nc.vector.tensor_copy(znb[kt][:, 0:512], znT[kt][:, _hh(h)])
                nc.vector.tensor_sub(znb[kt][:, 512:1024], znT[kt][:, _hh(h)],
                                     znb[kt][:, 0:512])
            pzf = [self.bank(dt) for dt in range(ND)]
            for m in range(NM):
                w1m = wk.tile([128, 2 * ND * 128], BF16,
                              tag=f"w1mh{m % 2}", name="w1m")
                nc.sync.dma_start(w1m[:], self.d_ffw1t[m, :, :])
                ph = self.bank(4 + m % 2)
                for kt in range(ND):
                    nc.tensor.matmul(ph[:], w1m[:, _sl(kt)], znb[kt][:, 0:512],
                                     start=(kt == 0), stop=False)
                    nc.tensor.matmul(ph[:], w1m[:, _sl(kt)], znb[kt][:, 512:1024],
                                     start=False, stop=False)
                    nc.tensor.matmul(ph[:], w1m[:, 512 + 128 * kt:640 + 128 * kt],
                                     znb[kt][:, 0:512],
                                     start=False, stop=(kt == ND - 1))
                sig = wk.tile([128, 512], F32, tag=f"sig{m % 2}", name="sig")
                nc.scalar.activation(sig[:], ph[:], AF.Sigmoid,
                                     bias=cpk[:, 8 + m:9 + m])
                sighl = wk.tile([128, 1024], BF16,
                                tag="amp2" if m % 2 == 0 else "lnscr",
                                name="sighl")
                nc.vector.tensor_copy(sighl[:, 0:512], sig[:])
                nc.vector.tensor_sub(sighl[:, 512:1024], sig[:],
                                     sighl[:, 0:512])
                w2m = wk.tile([128, 1024], BF16, tag=f"w2m{m % 2}", name="w2m")
                nc.sync.dma_start(w2m[:], self.d_ffw2hl[_sl(m), :])
                for dt in range(ND):
                    nc.tensor.matmul(pzf[dt][:], w2m[:, _sl(dt)],
                                     sighl[:, 0:512],
                                     start=(m == 0), stop=False)
                    nc.tensor.matmul(pzf[dt][:], w2m[:, _sl(dt)],
                                     sighl[:, 512:1024],
                                     start=False, stop=False)
                    nc.tensor.matmul(pzf[dt][:], w2m[:, 512 + dt * 128:
                                                     640 + dt * 128],
                                     sighl[:, 0:512],
                                     start=False, stop=(m == NM - 1))
            for dt in range(ND):
                nc.vector.scalar_tensor_tensor(yT[dt][:, _hh(h)], pzf[dt][:],
                                               cpk[:, 24 + dt:25 + dt],
                                               znT[dt][:, _hh(h)],
                                               op0=ALU.add, op1=ALU.add)
        return self._post_ln(s, yT, wk, yr=False)

    def _post_ln(self, s, yT, wk, yr):
        # fully per-tt post-LN chains: z4[0] is ready before the last yT
        # transposes finish, so l1's rfft starts with no barrier on the
        # batched stats
        nc = self.nc
        idn = self.idn
        rfr = PREC["rfft1"] == "f32r"
        gb, bb = self.gbt, self.bbt
        z4 = [wk.tile([128, D], F32R, tag=f"B3_{tt}", name=f"z4_{tt}")
              for tt in range(NT)]
        if not rfr:
            zhl = [wk.tile([128, 1024], BF16, tag=f"B2_{tt}", name=f"zhl{tt}")
                   for tt in range(NT)]
        st = wk.tile([128, 8 * NT], F32, tag="stpost", name="stpost")
        for tt in range(NT):
            pT = self.bank(6 + tt % 2, dtype=F32R if yr else F32)
            for dt in range(ND):
                nc.tensor.transpose(pT[:, _sl(dt)], yT[dt][:, _sl(tt)],
                                    self.idnr[:] if yr else idn[:])
            y_t = wk.tile([128, D], F32, tag=f"B4_{tt}", name=f"y{tt}")
            nc.scalar.copy(y_t[:], pT[:])
            scr = wk.tile([128, D], F32,
                          tag="lnscr" if tt % 2 == 0 else "lnscr2",
                          name="lnscr")
            mu = st[:, tt:tt + 1]
            s2 = st[:, NT + tt:NT + tt + 1]
            nc.vector.tensor_reduce(mu, y_t[:], mybir.AxisListType.X,
                                    op=ALU.add)
            nc.scalar.activation(scr[:], y_t[:], AF.Square, accum_out=s2)
            mun = st[:, 2 * NT + tt:2 * NT + tt + 1]
            nc.vector.tensor_scalar_mul(mun, mu, 1.0 / D)
            musq = st[:, 3 * NT + tt:3 * NT + tt + 1]
            nc.scalar.activation(musq, mun, AF.Square)
            var = st[:, 4 * NT + tt:4 * NT + tt + 1]
            nc.vector.scalar_tensor_tensor(var, s2, 1.0 / D, musq,
                                           op0=ALU.mult, op1=ALU.subtract)
            sd = st[:, 5 * NT + tt:5 * NT + tt + 1]
            nc.scalar.activation(sd, var, AF.Sqrt, bias=self.epst[:, 0:1])
            rs = st[:, 6 * NT + tt:6 * NT + tt + 1]
            nc.vector.reciprocal(rs, sd)
            nmurs = st[:, 7 * NT + tt:7 * NT + tt + 1]
            nc.vector.scalar_tensor_tensor(nmurs, mun, -1.0, rs,
                                           op0=ALU.mult, op1=ALU.mult)
            nc.scalar.activation(scr[:], y_t[:], AF.Identity,
                                 scale=rs, bias=nmurs)
            nc.vector.tensor_mul(z4[tt][:], scr[:], gb[:])
            nc.vector.tensor_add(z4[tt][:], z4[tt][:], bb[:])
            if not rfr:
                nc.gpsimd.tensor_copy(zhl[tt][:, 0:512], z4[tt][:])
                nc.gpsimd.tensor_sub(zhl[tt][:, 512:1024], z4[tt][:],
                                     zhl[tt][:, 0:512])
        if rfr:
            return z4, None
        return z4, zhl

    # ---------- output head ----------
    def _output(self, s, wk):
        nc = self.nc
        ones = self.ones
        po = self.bank(7)
        for kt in range(ND):
            nc.tensor.matmul(po[0:TF, 0:HOR], self.outwt[:, kt * TF:(kt + 1) * TF],
                             self.aggt[:, kt * HOR:(kt + 1) * HOR],
                             start=(kt == 0), stop=False)
        nc.tensor.matmul(po[0:TF, 0:HOR], self.outbt[0:1, 0:TF],
                         ones[0:1, 0:HOR], start=False, stop=True)
        xfin = wk.tile([TF, N], F32, tag="lvp", name="xfin")
        nc.gpsimd.dma_start(xfin[:], self.xtmid[s, :, :])
        oT = wk.tile([TF, HOR], F32, tag="lvv", name="oT")
        nc.vector.tensor_scalar_add(oT[:], po[0:TF, 0:HOR], xfin[:, N - 1:N])
        nc.gpsimd.dma_start(self.d_out[s * TF:(s + 1) * TF, :], oT[:])


def _get_nc():
    if "nc" not in _CACHE:
        _CACHE["nc"] = K().build()
    return _CACHE["nc"]


def _common_maps(inputs, w2d, dft, ib, e8):
    m = dict(
        w2d=_rne11(w2d) if PREC["l0head"] == "f32r" else w2d,
        ones1=np.ones((1, N), np.float32),
        idn=np.eye(128, dtype=np.float32),
        e8=e8,
        ibr=_rne11(ib),
        winr=_rne11(np.asarray(inputs["mhesa_win"], np.float32)),
        woutr=_rne11(np.asarray(inputs["mhesa_wout"], np.float32)),
        boutr=np.asarray(inputs["mhesa_bout"], np.float32).reshape(L, 1, D),
        lcolp=_pack_lcol(inputs),
        alpha8=np.asarray(inputs["mhesa_alpha"], np.float32).reshape(L, HEADS, 1),
        cpkp=_pack_cpk(inputs),
        gpostr=np.asarray(inputs["ff_post_g"], np.float32).reshape(1, D),
        bpostr=np.asarray(inputs["ff_post_b"], np.float32).reshape(1, D),
        lvwg=np.asarray(inputs["level_wg"], np.float32),
        lvwp=np.asarray(inputs["level_wp"], np.float32),
        lvbg=np.asarray(inputs["level_bg"], np.float32).reshape(L, 1, TF),
        lvbp=np.asarray(inputs["level_bp"], np.float32).reshape(L, 1, TF),
        lvalpha=np.asarray(inputs["level_alpha"], np.float32).reshape(L, 1, 1),
        damp8=np.asarray(inputs["dampen_factor"], np.float32).reshape(HEADS, 1),
        outw=np.asarray(inputs["out_w"], np.float32)
            .reshape(ND, 128, TF).transpose(1, 0, 2).reshape(128, ND * TF)
            .copy(),
        outbr=np.asarray(inputs["out_b"], np.float32).reshape(1, TF),
    )
    if PREC["l0head"] == "f32":
        m["dft"] = dft
    if PREC["l0head"] == "f32r" or PREC["rfft1"] == "f32r":
        m["dftr"] = _rne11(dft)
    if PREC["rfft1"] == "hl":
        m["dfthl"] = np.concatenate([_split_hi(dft), _split_lo(dft)], axis=1)
    if PREC["irfft0"] == "hl":
        m["ibhl"] = np.concatenate([_split_hi(ib), _split_lo(ib)], axis=1)
    if PREC["mhesa0"] == "hl":
        win0 = np.asarray(inputs["mhesa_win"][0], np.float32)
        wout0 = np.asarray(inputs["mhesa_wout"][0], np.float32)
        m["winhl"] = np.concatenate([_split_hi(win0), _split_lo(win0)], axis=1)
        m["wouthl"] = np.concatenate([_split_hi(wout0), _split_lo(wout0)],
                                     axis=1)
    w1 = np.asarray(inputs["ff_w1"], np.float32)
    w2 = np.asarray(inputs["ff_w2"], np.float32)
    if PREC["ff"] == "f32r":
        m["ffw1r"] = _rne11(_pack_w1r(w1))
        m["ffw2r"] = _rne11(w2)
    else:
        m["ffw1t"] = _pack_w1(w1)
        m["ffw2hl"] = np.concatenate([_split_hi(w2), _split_lo(w2)], axis=1)
    return m


def _kernel_impl(inputs, runner):
    x = np.asarray(inputs["x"], np.float32)
    assert (x.shape[0], x.shape[1], x.shape[2]) == (32, N, TF)
    assert int(inputs["forecast_horizon"]) == HOR
    dft, ib = _dft_consts()
    conv_w = np.asarray(inputs["conv_w"], np.float32)
    w2d = _build_w2d(conv_w, np.asarray(inputs["conv_b"], np.float32))
    e8 = np.repeat(np.eye(HEADS, dtype=np.float32), DH, axis=1)
    nc = _get_nc()
    common = _common_maps(inputs, w2d, dft, ib, e8)
    in_maps = []
    for c in range(NCORES):
        xs = x[c * S:(c + 1) * S]
        xT = xs.transpose(0, 2, 1).reshape(S * TF, N).copy()
        in_maps.append(dict(common, xT=xT))
    res = runner(nc, in_maps)
    out = np.zeros((x.shape[0], HOR, TF), np.float32)
    for c in range(NCORES):
        oT = res.results[c]["outT"].reshape(S, TF, HOR)
        out[c * S:(c + 1) * S] = oT.transpose(0, 2, 1)
    return out, res


def kernel(**inputs):
    out, _ = _kernel_impl(
        inputs,
        lambda nc, im: run_bass_kernel_spmd(nc, im, list(range(NCORES))))
    return out


def kernel_traced(**inputs):
    """Like kernel() but with NTFF profiling; returns (out, BassKernelResults)."""
    return _kernel_impl(
        inputs,
        lambda nc, im: run_bass_kernel_spmd(nc, im, list(range(NCORES)),
                                            trace=True))


# revision 42
# speedup vs baseline: 1.5587x; 1.0046x over previous
"""ETSFormer forward pass on 8 Trainium2 NeuronCores (Bass/Tile).

Data-parallel over batch: 32 samples -> 8 cores x 4 samples, weights
replicated, no collectives. The reference's FFT machinery is computed
exactly without FFTs:
  - freq_attention: dense DFT matmuls + hardware top-8 (vector.max) mask
  - mhesa / level exponential smoothing: the reference FFT cross-correlation
    is exactly a first-order EMA -> hardware prefix scan (tensor_tensor_scan)
  - fourier_extrapolate: exact slice (Dirichlet kernel identity)

Precision: PREC selects per-GEMM-group dtype. "f32r" = fp32-reduced
(FP22 truncated, 1 cyc/row on PE -- same speed as bf16) vs the fallback
"hl" = bf16 hi/lo 3-term split (~2^-16, 3 cyc/row) / "f32" = true fp32
(4 cyc/row). The top-4 frequency mask is rank-sensitive; flags are
tuned empirically against the end-to-end error gate.
"""
import numpy as np
from contextlib import ExitStack

import concourse.bass as bass
import concourse.bacc as bacc
import concourse.tile as tile
from concourse import mybir
from concourse.bass_utils import run_bass_kernel_spmd

F32 = mybir.dt.float32
F32R = mybir.dt.float32r
BF16 = mybir.dt.bfloat16
AF = mybir.ActivationFunctionType
ALU = mybir.AluOpType

N = 1024
D = 512
TF = 7
HEADS = 8
DH = D // HEADS
L = 2
S = 4
NCORES = 8
HOR = 96
FD = 2048
NT = N // 128   # 8
ND = D // 128   # 4
NM = FD // 128  # 16

_CACHE = {}

# per-stage precision: "f32r" fast path vs baseline "hl" (bf16 3-term)
# / "f32" (true fp32) fallback.
PREC = dict(
    l0head="f32",    # conv z GEMM + low-rank DFT: feeds the layer-0 top-4
                     # ranking, which flips even under 2^-12 weight rounding
                     # (emulation: 52 flips, 2.7e-2 err) -- keep exact fp32
    irfft0="f32r",   # layer-0 irfft (feeds layer-1 ranking path)
    mhesa0="f32r",   # layer-0 win/wout GEMMs
    ff="f32r",       # FF block w1/w2 GEMMs
    rfft1="f32r",    # layer-1 rfft (feeds layer-1 ranking directly)
)


def _rne11(x):
    # round fp32 mantissa to 11 explicit bits (fp22): the PE's f32r mode
    # truncates operands to fp22, so pre-rounded weights pass through
    # losslessly -- halves f32r noise and removes the truncation bias
    xi = np.ascontiguousarray(np.asarray(x, np.float32)).view(np.uint32)
    return ((xi + np.uint32(0x800)) & np.uint32(0xFFFFF000)).view(np.float32)


def _dft_consts():
    if "dft" not in _CACHE:
        t = np.arange(N)
        f = np.arange(513)
        ang = 2.0 * np.pi * np.outer(t, f) / N
        cos = np.cos(ang)
        sin = np.sin(ang)
        # B-half layout [sin(1..511), cos(512)] (cos512 moved to the END):
        # then amp2[f] = sqA[f] + sqB[f-1] for f=1..512 is a single shifted
        # add, and the f>=512 mask is ONE scalar_tensor_tensor over psB.
        dft = np.zeros((N, 1024), np.float64)
        dft[:, 0:512] = cos[:, 0:512]
        dft[:, 512:1023] = sin[:, 1:512]
        dft[:, 1023] = cos[:, 512]
        c = np.full(513, 2.0)
        c[0] = 1.0
        c[512] = 1.0
        ib = np.zeros((1024, N), np.float64)
        ib[0:512, :] = (c[0:512, None] / N) * cos[:, 0:512].T
        ib[512:1023, :] = (2.0 / N) * sin[:, 1:512].T
        ib[1023, :] = (1.0 / N) * cos[:, 512]
        _CACHE["dft"] = dft.astype(np.float32)
        _CACHE["ib"] = ib.astype(np.float32)
    return _CACHE["dft"], _CACHE["ib"]


def _sl(i, w=128):
    return slice(i * w, (i + 1) * w)


def _split_hi(x):
    import ml_dtypes
    return x.astype(ml_dtypes.bfloat16)


def _split_lo(x):
    import ml_dtypes
    hi = x.astype(ml_dtypes.bfloat16).astype(np.float32)
    return (x - hi).astype(ml_dtypes.bfloat16)


def _pack_w1(w1):
    # bf16 hi|lo tiles for the "hl" fallback FF path
    hi, lo = _split_hi(w1), _split_lo(w1)
    out = np.zeros((NM, 128, 1024), hi.dtype)
    for m in range(NM):
        for kt in range(ND):
            out[m, :, 128 * kt:128 * (kt + 1)] = hi[_sl(kt), _sl(m)]
            out[m, :, 512 + 128 * kt:640 + 128 * kt] = lo[_sl(kt), _sl(m)]
    return out


def _pack_w1r(w1):
    # f32r per-m contiguous [128(k), 4x128(m)] tiles
    out = np.zeros((NM, 128, 512), np.float32)
    for m in range(NM):
        for kt in range(ND):
            out[m, :, 128 * kt:128 * (kt + 1)] = w1[_sl(kt), _sl(m)]
    return out


def _pack_cpk(inputs):
    # cols: gpre(4) | bpre(4) | ffb1(16) | ffb2(4), each D/FD vector folded
    # into [128, k] column blocks -- one DMA instead of 24
    out = np.zeros((128, 28), np.float32)
    out[:, 0:4] = np.asarray(inputs["ff_pre_g"], np.float32).reshape(4, 128).T
    out[:, 4:8] = np.asarray(inputs["ff_pre_b"], np.float32).reshape(4, 128).T
    out[:, 8:24] = np.asarray(inputs["ff_b1"], np.float32).reshape(16, 128).T
    out[:, 24:28] = np.asarray(inputs["ff_b2"], np.float32).reshape(4, 128).T
    return out


def _pack_lcol(inputs):
    # per layer: init(4 cols) | bin(4 cols)
    out = np.zeros((L, 128, 8), np.float32)
    ini = np.asarray(inputs["mhesa_init"], np.float32).reshape(L, D)
    bi = np.asarray(inputs["mhesa_bin"], np.float32)
    for l in range(L):
        out[l, :, 0:4] = ini[l].reshape(4, 128).T
        out[l, :, 4:8] = bi[l].reshape(4, 128).T
    return out


def _build_w2d(conv_w, conv_b):
    # rows 32k+c hold conv_w[:, c, k] (32-aligned partition groups so the
    # on-device shifted copies keep legal base partitions); row 95 is the
    # bias row, paired with an all-ones row 95 of xsh on device.
    w2d = np.zeros((96, D), np.float32)
    for k in range(3):
        for c in range(TF):
            w2d[32 * k + c] = conv_w[:, c, k]
    w2d[95] = conv_b
    return w2d


def _hh(h):
    return slice(h * 512, (h + 1) * 512)


class K:
    def __init__(self):
        nc = bacc.Bacc()
        self.nc = nc
        p = nc.declare_dram_parameter
        self.d_xT = p("xT", [S * TF, N], F32, isOutput=False)
        self.d_w2d = p("w2d", [96, D],
                       F32R if PREC["l0head"] == "f32r" else F32,
                       isOutput=False)
        self.d_ones1 = p("ones1", [1, N], F32, isOutput=False)
        if PREC["l0head"] == "f32":
            self.d_dft = p("dft", [N, 1024], F32, isOutput=False)
        if PREC["l0head"] == "f32r" or PREC["rfft1"] == "f32r":
            self.d_dftr = p("dftr", [N, 1024], F32R, isOutput=False)
        if PREC["rfft1"] == "hl":
            self.d_dfthl = p("dfthl", [N, 2048], BF16, isOutput=False)
        self.d_ibr = p("ibr", [1024, N], F32R, isOutput=False)
        self.d_winr = p("winr", [L, D, D], F32R, isOutput=False)
        self.d_woutr = p("woutr", [L, D, D], F32R, isOutput=False)
        if PREC["irfft0"] == "hl":
            self.d_ibhl = p("ibhl", [1024, 2048], BF16, isOutput=False)
        if PREC["mhesa0"] == "hl":
            self.d_winhl = p("winhl", [D, 2 * D], BF16, isOutput=False)
            self.d_wouthl = p("wouthl", [D, 2 * D], BF16, isOutput=False)
        self.d_idn = p("idn", [128, 128], F32, isOutput=False)
        self.d_e8 = p("e8", [HEADS, D], F32, isOutput=False)
        self.d_bout = p("boutr", [L, 1, D], F32, isOutput=False)
        self.d_lcolp = p("lcolp", [L, 128, 8], F32, isOutput=False)
        self.d_al8 = p("alpha8", [L, HEADS, 1], F32, isOutput=False)
        if PREC["ff"] == "f32r":
            self.d_ffw1r = p("ffw1r", [NM, 128, 512], F32R, isOutput=False)
            self.d_ffw2r = p("ffw2r", [FD, D], F32R, isOutput=False)
        else:
            self.d_ffw1t = p("ffw1t", [NM, 128, 2 * ND * 128], BF16,
                             isOutput=False)
            self.d_ffw2hl = p("ffw2hl", [FD, 2 * D], BF16, isOutput=False)
        self.d_cpkp = p("cpkp", [128, 28], F32, isOutput=False)
        self.d_gpost = p("gpostr", [1, D], F32, isOutput=False)
        self.d_bpost = p("bpostr", [1, D], F32, isOutput=False)
        self.d_wg = p("lvwg", [L, D, TF], F32, isOutput=False)
        self.d_wp = p("lvwp", [L, D, TF], F32, isOutput=False)
        self.d_bg = p("lvbg", [L, 1, TF], F32, isOutput=False)
        self.d_bp = p("lvbp", [L, 1, TF], F32, isOutput=False)
        self.d_alv = p("lvalpha", [L, 1, 1], F32, isOutput=False)
        self.d_damp = p("damp8", [HEADS, 1], F32, isOutput=False)
        self.d_outw = p("outw", [128, ND * TF], F32, isOutput=False)
        self.d_outb = p("outbr", [1, TF], F32, isOutput=False)
        self.d_out = p("outT", [S * TF, HOR], F32, isOutput=True)
        self.xtmid = nc.dram_tensor("xtmid", [S, TF, N], F32)

    # psum bank helper: tag-based reuse of the 8 banks
    def bank(self, i, shape=(128, 512), dtype=F32):
        tl = self.psp.tile(list(shape), dtype, tag=f"bk{i}", name=f"bk{i}")
        return tl

    def build(self):
        nc = self.nc
        with ExitStack() as ctx:
            self.tc = ctx.enter_context(tile.TileContext(nc))
            tc = self.tc
            top = ctx.enter_context(tc.tile_pool(name="top", bufs=1))

            idn = top.tile([128, 128], F32, name="idn")
            nc.sync.dma_start(idn[:], self.d_idn[:])
            idnr = top.tile([128, 128], F32R, name="idnr")
            nc.vector.tensor_copy(idnr[:], idn[:])
            self.idnr = idnr
            ones = top.tile([128, 128], F32, name="ones")
            nc.vector.memset(ones[:], 1.0)
            w2d = top.tile([96, D],
                           F32R if PREC["l0head"] == "f32r" else F32,
                           name="w2d")
            nc.sync.dma_start(w2d[:], self.d_w2d[:])
            outbr = top.tile([1, TF], F32, name="outbr")
            nc.sync.dma_start(outbr[:], self.d_outb[:])
            self.outbt = outbr
            # col pack: gpre(4) | bpre(4)
            cpk = top.tile([128, 28], F32, name="cpk")
            nc.sync.dma_start(cpk[:], self.d_cpkp[:])
            outw = top.tile([128, ND * TF], F32, name="outw")
            nc.sync.dma_start(outw[:], self.d_outw[:])
            eps = top.tile([128, 1], F32, name="eps")
            nc.vector.memset(eps[:], 1e-5)
            self.epst = eps
            gbt = top.tile([128, D], F32, name="gbt")
            bbt = top.tile([128, D], F32, name="bbt")
            self.gbt, self.bbt = gbt, bbt
            agg = top.tile([128, ND * HOR], F32, name="agg")
            csd = top.tile([128, ND * HOR], F32, name="csd")

            self.idn, self.ones, self.cpk = idn, ones, cpk
            self.w2dt_, self.aggt, self.csdt = w2d, agg, csd
            self.outwt = outw

            self.psp = ctx.enter_context(
                tc.tile_pool(name="ps", bufs=1, space="PSUM"))
            # both layers' constants resident; samples run L0->L1 back to
            # back so L1's DVE-heavy tail overlaps the next sample's
            # PE-heavy head, and z4 never round-trips through DRAM
            lay0p = ctx.enter_context(tc.tile_pool(name="lay0", bufs=1))
            lay1p = ctx.enter_context(tc.tile_pool(name="lay1", bufs=1))
            with tc.tile_pool(name="ini", bufs=1) as ini:
                e8 = ini.tile([HEADS, D], F32, name="e8")
                nc.sync.dma_start(e8[:], self.d_e8[:])
                self.e8t = e8
                self._damp_cs(ini, self.psp)
                lay = [self._layer_consts(0, lay0p),
                       self._layer_consts(1, lay1p)]
            wk = ctx.enter_context(tc.tile_pool(name="wk", bufs=1))
            for s in range(S):
                z4 = self._sample(0, s, lay[0], wk)
                self._sample(1, s, lay[1], wk, zin=z4)
                self._output(s, wk)

        nc.compile()
        return nc

    # ---------- dampening cumsum -> csd [128, ND*HOR] ----------
    def _damp_cs(self, ini, inips):
        nc = self.nc
        ones = self.ones
        dcol = ini.tile([HEADS, 1], F32, name="dcol")
        nc.sync.dma_start(dcol[:], self.d_damp[:])
        df = ini.tile([HEADS, 1], F32, name="dfsig")
        nc.scalar.activation(df[:], dcol[:], AF.Sigmoid)
        dfb = ini.tile([HEADS, HOR], F32, name="dfb")
        nc.scalar.activation(dfb[:], ones[0:HEADS, 0:HOR], AF.Identity,
                             scale=df[:, 0:1])
        zer = ini.tile([HEADS, HOR], F32, name="zer8")
        nc.vector.memset(zer[:], 0.0)
        dfp = ini.tile([HEADS, HOR], F32, name="dfp")
        nc.vector.tensor_tensor_scan(dfp[:], dfb[:], zer[:], 1.0,
                                     op0=ALU.mult, op1=ALU.add)
        cs8 = ini.tile([HEADS, HOR], F32, name="cs8")
        nc.vector.tensor_tensor_scan(cs8[:], ones[0:HEADS, 0:HOR], dfp[:], 0.0,
                                     op0=ALU.mult, op1=ALU.add)
        for dt in range(ND):
            pini = inips.tile([128, HOR], F32, tag=f"bk{dt}", name="pini")
            nc.tensor.matmul(pini[:], self.e8t[:, _sl(dt)], cs8[:],
                             start=True, stop=True)
            nc.scalar.copy(self.csdt[:, dt * HOR:(dt + 1) * HOR], pini[:])
        # hoisted FF post-LN gamma/beta broadcasts (layer-invariant)
        rows = ini.tile([1, 1024], F32, name="rows")
        nc.sync.dma_start(rows[0:1, 0:512], self.d_gpost[:])
        nc.sync.dma_start(rows[0:1, 512:1024], self.d_bpost[:])
        pgb = inips.tile([128, D], F32, tag="bk4", name="pgb")
        nc.tensor.matmul(pgb[:], self.ones[0:1, 0:128],
                         rows[0:1, 0:512], start=True, stop=True)
        nc.scalar.copy(self.gbt[:], pgb[:])
        pbb = inips.tile([128, D], F32, tag="bk5", name="pbb")
        nc.tensor.matmul(pbb[:], self.ones[0:1, 0:128],
                         rows[0:1, 512:1024], start=True, stop=True)
        nc.scalar.copy(self.bbt[:], pbb[:])

    # ---------- per-layer constants ----------
    def _layer_consts(self, l, layp):
        nc = self.nc
        ones = self.ones
        last = l == L - 1
        lay = {"l": l, "last": last}

        if last or PREC["mhesa0"] == "f32r":
            win = [layp.tile([128, D], F32R, name=f"win{k}") for k in range(ND)]
            wout = [layp.tile([128, D], F32R, name=f"wout{k}")
                    for k in range(ND)]
            for kt in range(ND):
                nc.scalar.dma_start(win[kt][:], self.d_winr[l, _sl(kt), :])
                nc.scalar.dma_start(wout[kt][:], self.d_woutr[l, _sl(kt), :])
        else:
            # bf16 hi|lo packed (cols 0:512 hi, 512:1024 lo)
            win = [layp.tile([128, 2 * D], BF16, name=f"win{k}")
                   for k in range(ND)]
            wout = [layp.tile([128, 2 * D], BF16, name=f"wout{k}")
                    for k in range(ND)]
            for kt in range(ND):
                nc.scalar.dma_start(win[kt][:], self.d_winhl[_sl(kt), :])
                nc.scalar.dma_start(wout[kt][:], self.d_wouthl[_sl(kt), :])

        # lrows: p0 = bout[512]; p32 = bg[7] then bp at cols 16..23
        lrows = layp.tile([128, 512], F32, name="lrows")
        nc.sync.dma_start(lrows[0:1, 0:D], self.d_bout[l, :, :])
        nc.sync.dma_start(lrows[32:33, 0:TF], self.d_bg[l, :, :])
        nc.sync.dma_start(lrows[32:33, 16:16 + TF], self.d_bp[l, :, :])

        # bout broadcast [128, D] (replaces per-tile bias matmuls)
        boutb = layp.tile([128, D], F32, name="boutb")
        pbo = self.psp.tile([128, D], F32, tag="bk7", name="pbo")
        nc.tensor.matmul(pbo[:], ones[0:1, 0:128], lrows[0:1, 0:D],
                         start=True, stop=True)
        nc.scalar.copy(boutb[:], pbo[:])

        # lcol pack [128, 16]: al(4) oma(4) init(4) bi(4); plus lv cols [7,1]
        # cols 18/19: level bg/bp as [7,1] columns
        lcol = layp.tile([128, 24], F32, name="lcol")
        nc.sync.dma_start(lcol[0:TF, 18:19],
                          self.d_bg[l, :, :].rearrange("a b -> b a"))
        nc.sync.dma_start(lcol[0:TF, 19:20],
                          self.d_bp[l, :, :].rearrange("a b -> b a"))
        al8 = layp.tile([HEADS, 1], F32, tag="al8t", name="al8")
        nc.sync.dma_start(al8[:], self.d_al8[l, :, :])
        al8s = layp.tile([HEADS, 1], F32, tag="al8s", name="al8s")
        nc.scalar.activation(al8s[:], al8[:], AF.Sigmoid)
        for dt in range(ND):
            pal = self.psp.tile([128, 1], F32, tag="bk0", name="pal")
            nc.tensor.matmul(pal[:], self.e8t[:, _sl(dt)], al8s[:],
                             start=True, stop=True)
            nc.scalar.copy(lcol[:, dt:dt + 1], pal[:])
        nc.sync.dma_start(lcol[:, 8:16], self.d_lcolp[l, :, :])
        for dt in range(ND):
            nc.vector.tensor_scalar(lcol[:, 4 + dt:5 + dt], lcol[:, dt:dt + 1],
                                    -1.0, 1.0, op0=ALU.mult, op1=ALU.add)
        nc.vector.tensor_sub(lcol[:, 12:16], lcol[:, 12:16], lcol[:, 8:12])
        # col 20:24 = al*(bi-init) + (1-al)*init -- the scan-initial folded
        # into xd[0] so the scan can run with a 0.0 immediate initial
        bi = layp.tile([128, ND], F32, tag="bitmp", name="bitmp")
        nc.vector.tensor_mul(lcol[:, 20:24], lcol[:, 0:4], lcol[:, 12:16])
        nc.vector.tensor_mul(bi[:], lcol[:, 4:8], lcol[:, 8:12])
        nc.vector.tensor_add(lcol[:, 20:24], lcol[:, 20:24], bi[:])
        # level alpha
        alv = layp.tile([1, 1], F32, tag="alvt", name="alv")
        nc.sync.dma_start(alv[:], self.d_alv[l, :, :])
        alvs = layp.tile([1, 1], F32, tag="alvst", name="alvs")
        nc.scalar.activation(alvs[:], alv[:], AF.Sigmoid)
        pv = self.psp.tile([TF, 1], F32, tag="bk1", name="palv")
        nc.tensor.matmul(pv[:], ones[0:1, 0:TF], alvs[:], start=True, stop=True)
        nc.scalar.copy(lcol[0:TF, 16:17], pv[:])
        nc.vector.tensor_scalar(lcol[0:TF, 17:18], lcol[0:TF, 16:17], -1.0, 1.0,
                                op0=ALU.mult, op1=ALU.add)

        # level weights [128, TF] x4 packed [128, 2*ND*TF], as fp32r
        lwf = layp.tile([128, 2 * ND * TF], F32, tag="lwf", name="lwf")
        for kt in range(ND):
            nc.sync.dma_start(lwf[:, kt * TF:(kt + 1) * TF], self.d_wg[l, _sl(kt), :])
            nc.sync.dma_start(lwf[:, (ND + kt) * TF:(ND + kt + 1) * TF],
                              self.d_wp[l, _sl(kt), :])
        lw = layp.tile([128, 2 * ND * TF], F32R, name="lw")
        nc.vector.tensor_copy(lw[:], lwf[:])

        lay.update(win=win, wout=wout, lrows=lrows, lcol=lcol, lw=lw,
                   boutb=boutb)
        return lay

    # ---------- one sample through one layer ----------
    def _sample(self, l, s, lay, wk, zin=None):
        nc = self.nc
        ones, idn = self.ones, self.idn
        last = lay["last"]
        agg = self.aggt
        irf_r = last or PREC["irfft0"] == "f32r"
        mh_r = last or PREC["mhesa0"] == "f32r"

        def aggsl(dt):
            return self.aggt[:, dt * HOR:(dt + 1) * HOR]

        # --- z input: conv (l0) or handed over in SBUF from l0 (l1)
        if l == 0:
            hr = PREC["l0head"] == "f32r"
            # agg is per-sample now; clear it (waits on prior _output read)
            nc.gpsimd.memset(agg[:], 0.0)
            z = [wk.tile([128, D], F32R, tag=f"B1_{tt}", name=f"z{tt}")
                 for tt in range(NT)]
            # low-rank path: x is rank-7, so z = xsh^T @ w2d (rows 32k+c hold
            # the 3 shifts of the 7 channels; row 95 = ones * conv_b) and
            # DFT(z) = w2d^T @ (xsh^T @ dft) -- the DFT runs in the 96-dim
            # input space instead of the 512-dim channel space.
            xshf = wk.tile([96, N], F32, tag="xsh", name="xshf")
            xts = wk.tile([TF, N], F32, tag="xts", name="xts")
            nc.sync.dma_start(xts[:], self.d_xT[s * TF:(s + 1) * TF, :])
            nc.gpsimd.memset(xshf[:], 0.0)
            nc.gpsimd.tensor_copy(xshf[0:TF, 1:N], xts[:, 0:N - 1])
            nc.gpsimd.tensor_copy(xshf[32:32 + TF, 0:N], xts[:, 0:N])
            nc.gpsimd.tensor_copy(xshf[64:64 + TF, 0:N - 1], xts[:, 1:N])
            nc.sync.dma_start(xshf[95:96, :], self.d_ones1[:])
            if hr:
                # Pool can't touch f32r (ISA); one DVE copy re-tags for PE
                xsh = wk.tile([96, N], F32R, tag="xshr", name="xsh")
                nc.vector.tensor_copy(xsh[:], xshf[:])
            else:
                xsh = xshf
            xshT = [wk.tile([128, 96], F32R if hr else F32,
                            tag=f"xshT{tt}", name=f"xshT{tt}")
                    for tt in range(NT)]
            psF1A = self.bank(2, shape=(96, 512))
            psF1B = self.bank(3, shape=(96, 512))
            tid = self.idnr if hr else idn
            d_dft_src = self.d_dftr if hr else self.d_dft
            for tt in range(NT):
                pz = self.bank(tt % 2)
                nc.tensor.matmul(pz[:], xsh[:, _sl(tt)], self.w2dt_[:],
                                 start=True, stop=True)
                nc.scalar.copy(z[tt][:], pz[:])
                pxT = self.bank(6, shape=(128, 96),
                                dtype=F32R if hr else F32)
                nc.tensor.transpose(pxT[:], xsh[:, _sl(tt)], tid[0:96, 0:96])
                nc.scalar.copy(xshT[tt][:], pxT[:])
                dftk = wk.tile([128, 1024], F32R if hr else F32,
                               tag=f"dftk{tt % 2}", name="dftk")
                nc.sync.dma_start(dftk[:], d_dft_src[_sl(tt), :])
                nc.tensor.matmul(psF1A[:], xshT[tt][:], dftk[:, 0:512],
                                 start=(tt == 0), stop=(tt == NT - 1))
                nc.tensor.matmul(psF1B[:], xshT[tt][:], dftk[:, 512:1024],
                                 start=(tt == 0), stop=(tt == NT - 1))
            F1s = wk.tile([96, 1024], F32R if hr else F32, tag="lvp",
                          name="F1s")
            nc.scalar.copy(F1s[:, 0:512], psF1A[:])
            nc.scalar.copy(F1s[:, 512:1024], psF1B[:])
            psA = [self.bank(ct) for ct in range(ND)]
            psB = [self.bank(4 + ct) for ct in range(ND)]
            for ct in range(ND):
                nc.tensor.matmul(psA[ct][:], self.w2dt_[:, _sl(ct)],
                                 F1s[:, 0:512], start=True, stop=True)
                nc.tensor.matmul(psB[ct][:], self.w2dt_[:, _sl(ct)],
                                 F1s[:, 512:1024], start=True, stop=True)
            ibkpf = []
            if irf_r:
                for pf in range(2):
                    ibkp = wk.tile([128, 1024], F32R, tag=f"dftk{pf % 2}",
                                   name="ibk")
                    nc.sync.dma_start(ibkp[:], self.d_ibr[_sl(pf), :])
                    ibkpf.append(ibkp)
            else:
                for pf in range(2):
                    ibkp = wk.tile([128, 2048], BF16, tag=f"dftk{pf % 2}",
                                   name="ibk")
                    nc.sync.dma_start(ibkp[:], self.d_ibhl[_sl(pf), :])
                    ibkpf.append(ibkp)
        else:
            z, zhl = zin

            psA = [self.bank(ct) for ct in range(ND)]
            psB = [self.bank(4 + ct) for ct in range(ND)]
            if PREC["rfft1"] == "f32r":
                # z tiles are F32R [t, d]; stationary slice [t, c-block]
                pfs = getattr(self, "_dftk_pf", None)
                self._dftk_pf = None
                for kt in range(NT):
                    if pfs is not None and kt < 2:
                        dftk = pfs[kt]
                    else:
                        dftk = wk.tile([128, 1024], F32R, tag=f"dftk{kt % 2}",
                                       name="dftk")
                        nc.sync.dma_start(dftk[:], self.d_dftr[_sl(kt), :])
                    st0 = kt == 0
                    sp = kt == NT - 1
                    for ct in range(ND):
                        zst = z[kt][:, _sl(ct)]
                        nc.tensor.matmul(psA[ct][:], zst, dftk[:, 0:512],
                                         start=st0, stop=sp)
                        nc.tensor.matmul(psB[ct][:], zst, dftk[:, 512:1024],
                                         start=st0, stop=sp)
            else:
                # rfft via bf16 hi/lo 3-term split (exact to ~2^-17)
                for kt in range(NT):
                    dftk = wk.tile([128, 2048], BF16, tag=f"dftk{kt % 2}",
                                   name="dftk")
                    nc.sync.dma_start(dftk[:], self.d_dfthl[_sl(kt), :])
                    st0 = kt == 0
                    sp = kt == NT - 1
                    for ct in range(ND):
                        zh = zhl[kt][:, _sl(ct)]
                        zl = zhl[kt][:, 512 + 128 * ct:640 + 128 * ct]
                        nc.tensor.matmul(psA[ct][:], zh, dftk[:, 0:512],
                                         start=st0, stop=False)
                        nc.tensor.matmul(psA[ct][:], zh, dftk[:, 1024:1536],
                                         start=False, stop=False)
                        nc.tensor.matmul(psB[ct][:], zh, dftk[:, 512:1024],
                                         start=st0, stop=False)
                        nc.tensor.matmul(psB[ct][:], zh, dftk[:, 1536:2048],
                                         start=False, stop=False)
                        nc.tensor.matmul(psA[ct][:], zl, dftk[:, 0:512],
                                         start=False, stop=sp)
                        nc.tensor.matmul(psB[ct][:], zl, dftk[:, 512:1024],
                                         start=False, stop=sp)
            # prefetch the first two irfft ib stripes while the mask runs
            ibkpf = []
            for pf in range(2):
                ibkp = wk.tile([128, 1024], F32R, tag=f"dftk{pf % 2}",
                               name="ibk")
                nc.sync.dma_start(ibkp[:], self.d_ibr[_sl(pf), :])
                ibkpf.append(ibkp)

        # --- top-4 mask -> filt [ND][128, 1024] ([c, f])
        # Pool has no PSUM port: psA/psB land in SBUF once (ACT), then the
        # whole chain (squares, add, is_ge mask) runs on the idle Pool
        # engine; only the top-8 max needs DVE.
        filt = [wk.tile([128, 1024], F32R if irf_r else F32,
                        tag=f"A1_{ct}", name=f"filt{ct}")
                for ct in range(ND)]
        for ct in range(ND):
            amp2 = wk.tile([128, 513], F32,
                           tag="amp2" if ct % 2 == 0 else "lnscr", name="amp2")
            nc.scalar.activation(amp2[:, 0:512], psA[ct][:], AF.Square)
            sqB = wk.tile([128, 512], F32,
                          tag="w2m0" if ct % 2 == 0 else "w2m1", name="sqB")
            nc.scalar.activation(sqB[:], psB[ct][:], AF.Square)
            # permuted B-half: amp2[f] = sqA[f] + sqB[f-1], amp2[512]=sqB[511]
            nc.gpsimd.tensor_add(amp2[:, 1:512], amp2[:, 1:512], sqB[:, 0:511])
            nc.gpsimd.tensor_copy(amp2[:, 512:513], sqB[:, 511:512])
            top8 = wk.tile([128, 8], F32, tag="top8", name="top8")
            nc.vector.max(top8[:], amp2[:])
            kth = top8[:, 3:4]
            nc.vector.scalar_tensor_tensor(filt[ct][:, 0:512], amp2[:, 0:512],
                                           kth, psA[ct][:],
                                           op0=ALU.is_ge, op1=ALU.mult)
            nc.vector.scalar_tensor_tensor(filt[ct][:, 512:1024], amp2[:, 1:513],
                                           kth, psB[ct][:],
                                           op0=ALU.is_ge, op1=ALU.mult)

        # --- transpose filt -> filtT [f, c]; hl splits to bf16 hi|lo
        if irf_r:
            filtT = [wk.tile([128, 512], F32R, tag=f"B2_{ft}",
                             name=f"filtT{ft}") for ft in range(NT)]
            for ft in range(NT):
                pT = self.bank(ft % 4, dtype=F32R)
                for ct in range(ND):
                    nc.tensor.transpose(pT[:, _sl(ct)], filt[ct][:, _sl(ft)],
                                        self.idnr[:])
                if ft % 2 == 0:
                    nc.scalar.copy(filtT[ft][:], pT[:])
                else:
                    nc.vector.tensor_copy(filtT[ft][:], pT[:])
        else:
            filtT = [wk.tile([128, 1024], BF16, tag=f"B2_{ft}",
                             name=f"fthl{ft}") for ft in range(NT)]
            for ft in range(NT):
                pT = self.bank(ft % 4)
                for ct in range(ND):
                    nc.tensor.transpose(pT[:, _sl(ct)], filt[ct][:, _sl(ft)],
                                        idn[:])
                nc.scalar.copy(filtT[ft][:, 0:512], pT[:])
                nc.vector.tensor_sub(filtT[ft][:, 512:1024], pT[:],
                                     filtT[ft][:, 0:512])

        # --- irfft (ib streamed, 8 banks) -> lp, z2
        pl = [self.bank(tt) for tt in range(NT)]
        if irf_r:
            for ft in range(NT):
                if ft < 2:
                    ibk = ibkpf[ft]
                else:
                    ibk = wk.tile([128, 1024], F32R, tag=f"dftk{ft % 2}",
                                  name="ibk")
                    nc.sync.dma_start(ibk[:], self.d_ibr[_sl(ft), :])
                for tt in range(NT):
                    nc.tensor.matmul(pl[tt][:], ibk[:, _sl(tt)], filtT[ft][:],
                                     start=(ft == 0), stop=(ft == NT - 1))
        else:
            for ft in range(NT):
                if ft < 2:
                    ibk = ibkpf[ft]
                else:
                    ibk = wk.tile([128, 2048], BF16, tag=f"dftk{ft % 2}",
                                  name="ibk")
                    nc.sync.dma_start(ibk[:], self.d_ibhl[_sl(ft), :])
                for tt in range(NT):
                    ibh = ibk[:, _sl(tt)]
                    ibl = ibk[:, 1024 + 128 * tt:1152 + 128 * tt]
                    nc.tensor.matmul(pl[tt][:], ibh, filtT[ft][:, 0:512],
                                     start=(ft == 0), stop=False)
                    nc.tensor.matmul(pl[tt][:], ibh, filtT[ft][:, 512:1024],
                                     start=False, stop=False)
                    nc.tensor.matmul(pl[tt][:], ibl, filtT[ft][:, 0:512],
                                     start=False, stop=(ft == NT - 1))
        lp = [wk.tile([128, D], F32R, tag=f"B3_{tt}", name=f"lp{tt}")
              for tt in range(NT)]
        z2 = [wk.tile([128, D], F32R if mh_r else F32,
                      tag=f"B4_{tt}", name=f"z2_{tt}")
              for tt in range(NT)]
        for tt in range(NT):
            # z2 before lp: in l1 the lp tiles reuse z's memory (tag B3)
            nc.vector.tensor_sub(z2[tt][:], z[tt][:], pl[tt][:])
            nc.scalar.copy(lp[tt][:], pl[tt][:])

        # --- lpT [ND][128, N] (tag A2) + extrap + perT; then free
        lpT = [wk.tile([128, N], F32R, tag=f"A2_{dt}", name=f"lpT{dt}")
               for dt in range(ND)]
        perT = wk.tile([TF, N], F32, tag="dftk0", name="perT")
        for h in range(2):
            for dt in range(ND):
                pT = self.bank(dt, dtype=F32R)
                for q in range(4):
                    nc.tensor.transpose(pT[:, _sl(q)], lp[h * 4 + q][:, _sl(dt)],
                                        self.idnr[:])
                if h == 0:
                    nc.vector.tensor_copy(lpT[dt][:, _hh(h)], pT[:])
                    nc.vector.tensor_add(aggsl(dt), aggsl(dt),
                                         lpT[dt][:, 0:HOR])
                else:
                    nc.vector.tensor_copy(lpT[dt][:, _hh(h)], pT[:])
            # perT for this half right away: fills the PE wait on the next
            # half's lp copies
            pp = self.bank(4 + h)
            for kt in range(ND):
                nc.tensor.matmul(pp[0:TF, :], lay["lw"][:, (ND + kt) * TF:(ND + kt + 1) * TF],
                                 lpT[kt][:, _hh(h)],
                                 start=(kt == 0), stop=(kt == ND - 1))
            nc.scalar.copy(perT[:, _hh(h)], pp[0:TF, :])

        # --- z2T (tag A2 reuse after lpT dead); hl packs bf16 hi|lo
        if mh_r:
            z2T = [wk.tile([128, N], F32R, tag=f"A2_{dt}", name=f"z2T{dt}")
                   for dt in range(ND)]
            for h in range(2):
                for dt in range(ND):
                    pT = self.bank(dt, dtype=F32R)
                    for q in range(4):
                        nc.tensor.transpose(pT[:, _sl(q)],
                                            z2[h * 4 + q][:, _sl(dt)],
                                            self.idnr[:])
                    nc.vector.tensor_copy(z2T[dt][:, _hh(h)], pT[:])
        else:
            z2T = [wk.tile([128, 2 * N], BF16, tag=f"A2_{dt}",
                           name=f"z2Thl{dt}") for dt in range(ND)]
            for h in range(2):
                for dt in range(ND):
                    pT = self.bank(dt)
                    for q in range(4):
                        nc.tensor.transpose(pT[:, _sl(q)],
                                            z2[h * 4 + q][:, _sl(dt)], idn[:])
                    nc.scalar.copy(z2T[dt][:, _hh(h)], pT[:])
                    nc.vector.tensor_sub(
                        z2T[dt][:, N + 512 * h:N + 512 * h + 512], pT[:],
                        z2T[dt][:, _hh(h)])

        # --- win GEMM -> xinT -> xd -> scan, interleaved per dt so the
        # serial DVE scan chain overlaps the next dt's win GEMMs on PE
        xinT = [wk.tile([128, N], F32, tag=f"A1_{dt}", name=f"xinT{dt}")
                for dt in range(ND)]
        lc = lay["lcol"]
        if mh_r:
            sT = [wk.tile([128, N], F32R, tag=f"A2_{dt}", name=f"sT{dt}")
                  for dt in range(ND)]
            sTsc = sT
            for dt in range(ND):
                for h in range(2):
                    px = self.bank(4 + h)
                    for kt in range(ND):
                        nc.tensor.matmul(px[:], lay["win"][kt][:, _sl(dt)],
                                         z2T[kt][:, _hh(h)],
                                         start=(kt == 0), stop=(kt == ND - 1))
                    # fold the per-head alpha scale into the psum->sbuf copy
                    nc.scalar.activation(xinT[dt][:, _hh(h)], px[:],
                                         AF.Identity,
                                         scale=lay["lcol"][:, dt:dt + 1])
                eng = nc.vector if dt % 2 == 0 else nc.gpsimd
                xd = wk.tile([128, N], F32,
                             tag="xdsc0" if dt % 2 == 0 else "xdsc1",
                             name="xd")
                eng.tensor_sub(xd[:, 1:N], xinT[dt][:, 1:N],
                               xinT[dt][:, 0:N - 1])
                # xinT is pre-scaled by alpha; col 20+dt folds the initial
                nc.vector.tensor_scalar_add(xd[:, 0:1], xinT[dt][:, 0:1],
                                            lc[:, 20 + dt:21 + dt])
                omab_ap = lc[:, 4 + dt:5 + dt].broadcast_to([128, N])
                nc.vector.tensor_tensor_scan(sTsc[dt][:], omab_ap, xd[:], 0.0,
                                             op0=ALU.mult, op1=ALU.add)
        else:
            for h in range(2):
                for dt in range(ND):
                    px = self.bank(4 + dt % 2)
                    for kt in range(ND):
                        wh = lay["win"][kt][:, _sl(dt)]
                        wl = lay["win"][kt][:, 512 + 128 * dt:640 + 128 * dt]
                        zh = z2T[kt][:, _hh(h)]
                        zl = z2T[kt][:, N + 512 * h:N + 512 * h + 512]
                        nc.tensor.matmul(px[:], wh, zh,
                                         start=(kt == 0), stop=False)
                        nc.tensor.matmul(px[:], wh, zl,
                                         start=False, stop=False)
                        nc.tensor.matmul(px[:], wl, zh,
                                         start=False, stop=(kt == ND - 1))
                    nc.scalar.activation(xinT[dt][:, _hh(h)], px[:],
                                         AF.Identity,
                                         scale=lay["lcol"][:, dt:dt + 1])
            sTsc = [wk.tile([128, N], F32, tag=f"A1_{dt}", name=f"sTf{dt}")
                    for dt in range(ND)]
            sT = [wk.tile([128, 2 * N], BF16, tag=f"A2_{dt}",
                          name=f"sThl{dt}") for dt in range(ND)]
            for dt in range(ND):
                eng = nc.vector if dt % 2 == 0 else nc.gpsimd
                xd = wk.tile([128, N], F32,
                             tag="xdsc0" if dt % 2 == 0 else "xdsc1",
                             name="xd")
                eng.tensor_sub(xd[:, 1:N], xinT[dt][:, 1:N],
                               xinT[dt][:, 0:N - 1])
                nc.vector.tensor_scalar_add(xd[:, 0:1], xinT[dt][:, 0:1],
                                            lc[:, 20 + dt:21 + dt])
                omab_ap = lc[:, 4 + dt:5 + dt].broadcast_to([128, N])
                nc.vector.tensor_tensor_scan(sTsc[dt][:], omab_ap, xd[:], 0.0,
                                             op0=ALU.mult, op1=ALU.add)
                eng.tensor_copy(sT[dt][:, 0:N], sTsc[dt][:])
                eng.tensor_sub(sT[dt][:, N:2 * N], sTsc[dt][:],
                               sT[dt][:, 0:N])

        # --- wout GEMM -> lg [t,d] (tag B2 reuse: filtT dead) (+ z3 if l0)
        # pre-LN stats chains interleave per tt right behind the z3 subs so
        # DVE starts them 8 tiles earlier than a post-wout batch would
        prep = None
        if not last and PREC["ff"] == "f32r":
            stpre = wk.tile([128, 8 * NT], F32, tag="stpre", name="stpre")
            h_ = [wk.tile([128, D], F32R, tag=f"B4_{tt}", name=f"h{tt}")
                  for tt in range(NT)]

            def prep(tt):
                scr = wk.tile([128, D], F32,
                              tag="lnscr" if tt % 2 == 0 else "lnscr2",
                              name="lnscr")
                st = stpre
                mu = st[:, tt:tt + 1]
                s2 = st[:, NT + tt:NT + tt + 1]
                nc.vector.tensor_reduce(mu, z[tt][:], mybir.AxisListType.X,
                                        op=ALU.add)
                nc.scalar.activation(scr[:], z[tt][:], AF.Square, accum_out=s2)
                mun = st[:, 2 * NT + tt:2 * NT + tt + 1]
                nc.vector.tensor_scalar_mul(mun, mu, 1.0 / D)
                musq = st[:, 3 * NT + tt:3 * NT + tt + 1]
                nc.scalar.activation(musq, mun, AF.Square)
                var = st[:, 4 * NT + tt:4 * NT + tt + 1]
                nc.vector.scalar_tensor_tensor(var, s2, 1.0 / D, musq,
                                               op0=ALU.mult, op1=ALU.subtract)
                sd = st[:, 5 * NT + tt:5 * NT + tt + 1]
                nc.scalar.activation(sd, var, AF.Sqrt, bias=self.epst[:, 0:1])
                rs = st[:, 6 * NT + tt:6 * NT + tt + 1]
                nc.vector.reciprocal(rs, sd)
                nmurs = st[:, 7 * NT + tt:7 * NT + tt + 1]
                nc.vector.scalar_tensor_tensor(nmurs, mun, -1.0, rs,
                                               op0=ALU.mult, op1=ALU.mult)
                nc.scalar.activation(h_[tt][:], z[tt][:], AF.Identity,
                                     scale=rs, bias=nmurs)
        lg = [wk.tile([128, D], F32R, tag=f"B2_{tt}", name=f"lg{tt}")
              for tt in range(NT)]
        for tt in range(NT):
            pg = self.bank(tt % 2)
            if mh_r:
                for kt in range(ND):
                    nc.tensor.matmul(pg[:], sT[kt][:, _sl(tt)],
                                     lay["wout"][kt][:],
                                     start=(kt == 0), stop=(kt == ND - 1))
            else:
                for kt in range(ND):
                    sh = sT[kt][:, _sl(tt)]
                    sl_ = sT[kt][:, N + 128 * tt:N + 128 * tt + 128]
                    nc.tensor.matmul(pg[:], sh, lay["wout"][kt][:, 0:512],
                                     start=(kt == 0), stop=False)
                    nc.tensor.matmul(pg[:], sh, lay["wout"][kt][:, 512:1024],
                                     start=False, stop=False)
                    nc.tensor.matmul(pg[:], sl_, lay["wout"][kt][:, 0:512],
                                     start=False, stop=(kt == ND - 1))
            nc.vector.tensor_add(lg[tt][:], pg[:], lay["boutb"][:])
            if not last:
                # z3 overwrites z (tag B1): z dead after z2
                nc.vector.tensor_sub(z[tt][:], z2[tt][:], lg[tt][:])
                if prep is not None:
                    prep(tt)
        z3 = z

        def emit_tail():
            # lglast/lgT/grT/damp/level-step. For l0 this is DEFERRED into
            # the FF (emitted after the h0 GEMM loop) so its PE work (lgT,
            # grT on banks 2/3) and DVE work overlap the FF GEMMs instead
            # of stalling the pre-LN stats chain.
            lglast = wk.tile([1, D], F32, tag="sqA", name="lglast")
            nc.gpsimd.dma_start(lglast[:], lg[NT - 1][127:128, :])
            lgl4 = wk.tile([128, ND], F32, tag="top8", name="lgl4")
            pTl = self.bank(2, shape=(128, ND))
            for dt in range(ND):
                nc.tensor.matmul(pTl[:, dt:dt + 1], lglast[0:1, _sl(dt)],
                                 ones[0:1, 0:1], start=True, stop=True)
            nc.scalar.copy(lgl4[:], pTl[:])

            # lgT via transposes (tag A1 reuse: xinT dead)
            lgT = [wk.tile([128, N], F32R, tag=f"A1_{dt}", name=f"lgT{dt}")
                   for dt in range(ND)]
            for h in range(2):
                for dt in range(ND):
                    pT = self.bank(2 + dt % 2, dtype=F32R)
                    for q in range(4):
                        nc.tensor.transpose(pT[:, _sl(q)],
                                            lg[h * 4 + q][:, _sl(dt)],
                                            self.idnr[:])
                    if h == 0:
                        nc.scalar.copy(lgT[dt][:, _hh(h)], pT[:])
                    else:
                        nc.vector.tensor_copy(lgT[dt][:, _hh(h)], pT[:])
            for dt in range(ND):
                # damp: agg += lg_last * csd
                nc.vector.scalar_tensor_tensor(
                    aggsl(dt), self.csdt[:, dt * HOR:(dt + 1) * HOR],
                    lgl4[:, dt:dt + 1], aggsl(dt), op0=ALU.mult, op1=ALU.add)

            # level: grT; scans update xtmid
            grT = wk.tile([TF, N], F32, tag="grT", name="grT")
            for h in range(2):
                pgr = self.bank(2 + h)
                for kt in range(ND):
                    nc.tensor.matmul(pgr[0:TF, :],
                                     lay["lw"][:, kt * TF:(kt + 1) * TF],
                                     lgT[kt][:, _hh(h)],
                                     start=(kt == 0), stop=(kt == ND - 1))
                # fold level bg bias (lcol col 18) into the psum->sbuf copy
                nc.vector.tensor_scalar_add(grT[:, _hh(h)], pgr[0:TF, :],
                                            lc[0:TF, 18:19])

            xts2 = wk.tile([TF, N], F32, tag="xts", name="xts2")
            if l == 0:
                nc.sync.dma_start(xts2[:], self.d_xT[s * TF:(s + 1) * TF, :])
            else:
                nc.sync.dma_start(xts2[:], self.xtmid[s, :, :])
            v = wk.tile([TF, N], F32, tag="lvv", name="lvv")
            # v = (xts2 - bp) - perT (DVE: Pool has no TensorScalarPtr)
            nc.vector.scalar_tensor_tensor(v[:], xts2[:], lc[0:TF, 19:20],
                                           perT[:],
                                           op0=ALU.subtract, op1=ALU.subtract)
            nc.vector.tensor_scalar_mul(v[:], v[:], lc[0:TF, 16:17])
            omlv_ap = lc[0:TF, 17:18].broadcast_to([TF, N])
            pt = wk.tile([TF, N], F32, tag="lvp", name="lvp")
            nc.vector.tensor_tensor_scan(pt[:], omlv_ap, v[:], 0.0,
                                         op0=ALU.mult, op1=ALU.add)
            gt = wk.tile([TF, N], F32, tag="lvv", name="lvg")
            nc.vector.tensor_tensor_scan(gt[:], omlv_ap, grT[:], 0.0,
                                         op0=ALU.mult, op1=ALU.add)
            xnew = wk.tile([TF, N], F32, tag="grT", name="xnew")
            nc.gpsimd.tensor_add(xnew[:], pt[:], gt[:])
            if l == 0:
                # on Pool: keeps this late-blocking store off the DMA queues
                nc.gpsimd.dma_start(self.xtmid[s, :, :], xnew[:])
            else:
                # l1's level output feeds only _output: skip the DRAM trip
                self._xnew_last = xnew
            if l == 0 and PREC["rfft1"] == "f32r":
                # prefetch l1's first two rfft dft stripes on the ACT hwdge
                # queue: the SP queue is still draining FF w1/w2 triggers
                pfs = []
                for i in range(2):
                    t = wk.tile([128, 1024], F32R, tag=f"dftk{i}",
                                name="dftkpf")
                    nc.scalar.dma_start(t[:], self.d_dftr[_sl(i), :])
                    pfs.append(t)
                self._dftk_pf = pfs

        # --- FF (layer 0 only); z4 stays in SBUF for l1
        if not last:
            return self._ff(s, z3, wk, emit_tail,
                            h_ if prep is not None else None)
        emit_tail()
        return None

    # ---------- LN stats ----------
    def _ln_stats(self, zset, wk, tagp):
        nc = self.nc
        st = wk.tile([128, 8 * NT], F32, tag=f"st{tagp}", name=f"st{tagp}")
        mu8 = st[:, 0:NT]
        s28 = st[:, NT:2 * NT]
        for tt in range(NT):
            scr = wk.tile([128, D], F32,
                          tag="lnscr" if tt % 2 == 0 else "lnscr2",
                          name="lnscr")
            nc.vector.tensor_reduce(st[:, tt:tt + 1], zset[tt][:],
                                    mybir.AxisListType.X, op=ALU.add)
            nc.scalar.activation(scr[:], zset[tt][:], AF.Square,
                                 accum_out=st[:, NT + tt:NT + tt + 1])
        mun = st[:, 2 * NT:3 * NT]
        nc.vector.tensor_scalar_mul(mun, mu8, 1.0 / D)
        ex2 = st[:, 3 * NT:4 * NT]
        nc.vector.tensor_scalar_mul(ex2, s28, 1.0 / D)
        musq = st[:, 4 * NT:5 * NT]
        nc.scalar.activation(musq, mun, AF.Square)
        var = st[:, 5 * NT:6 * NT]
        nc.vector.tensor_sub(var, ex2, musq)
        sd = st[:, 6 * NT:7 * NT]
        nc.scalar.activation(sd, var, AF.Sqrt, bias=self.epst[:, 0:1])
        rs = st[:, 7 * NT:8 * NT]
        nc.vector.reciprocal(rs, sd)
        nmurs = st[:, 4 * NT:5 * NT]  # overwrite musq slot
        nc.vector.tensor_mul(nmurs, mun, rs)
        nc.vector.tensor_scalar_mul(nmurs, nmurs, -1.0)
        return rs, nmurs

    # ---------- FF block ----------
    def _ff(self, s, z3, wk, tail, h_=None):
        if PREC["ff"] == "f32r":
            return self._ff_f32r(s, z3, wk, tail, h_)
        return self._ff_hl(s, z3, wk, tail)

    def _ff_f32r(self, s, z3, wk, tail, h_):
        nc = self.nc
        cpk = self.cpk
        # h_ (pre-LN normalized tiles) were produced per-tt inside the wout
        # loop by _sample's prep closure
        hT = [wk.tile([128, N], F32R, tag=f"A2_{dt}", name=f"hT{dt}")
              for dt in range(ND)]
        znT = [wk.tile([128, N], F32R, tag=f"A1_{dt}", name=f"znT{dt}")
               for dt in range(ND)]
        for h in range(2):
            for dt in range(ND):
                pT = self.bank(dt, dtype=F32R)
                for q in range(4):
                    nc.tensor.transpose(pT[:, _sl(q)], h_[h * 4 + q][:, _sl(dt)],
                                        self.idnr[:])
                if h == 0:
                    nc.scalar.copy(hT[dt][:, _hh(h)], pT[:])
                else:
                    nc.vector.tensor_copy(hT[dt][:, _hh(h)], pT[:])
                # znT per (h, dt) immediately: the first w1 matmul only
                # needs the four h0 halves
                nc.vector.tensor_scalar(znT[dt][:, _hh(h)], hT[dt][:, _hh(h)],
                                        cpk[:, dt:dt + 1],
                                        cpk[:, 4 + dt:5 + dt],
                                        op0=ALU.mult, op1=ALU.add)

        yT = [wk.tile([128, N], F32R, tag=f"A2_{dt}", name=f"yT{dt}")
              for dt in range(ND)]
        for h in range(2):
            pzf = [self.bank(b) for b in (0, 1, 6, 7)]
            # software-pipelined: w2(m-1) is emitted AFTER w1(m), so the PE
            # never sits head-of-line waiting on sig(m-1)'s ACT latency
            sigs = [None, None]
            w2ms = [None, None]

            def w2_stage(m):
                for dt in range(ND):
                    nc.tensor.matmul(pzf[dt][:], w2ms[m % 2][:, _sl(dt)],
                                     sigs[m % 2][:],
                                     start=(m == 0), stop=(m == NM - 1))

            for m in range(NM):
                w1m = wk.tile([128, 512], F32R, tag=f"w1mh{m % 2}", name="w1m")
                nc.sync.dma_start(w1m[:], self.d_ffw1r[m, :, :])
                ph = self.bank(4 + m % 2)
                for kt in range(ND):
                    nc.tensor.matmul(ph[:], w1m[:, _sl(kt)],
                                     znT[kt][:, _hh(h)],
                                     start=(kt == 0), stop=(kt == ND - 1))
                if m > 0:
                    w2_stage(m - 1)
                sig = wk.tile([128, 512], F32R, tag=f"sig{m % 2}", name="sig")
                nc.scalar.activation(sig[:], ph[:], AF.Sigmoid,
                                     bias=cpk[:, 8 + m:9 + m])
                sigs[m % 2] = sig
                w2m = wk.tile([128, 512], F32R, tag=f"w2m{m % 2}", name="w2m")
                nc.sync.dma_start(w2m[:], self.d_ffw2r[_sl(m), :])
                w2ms[m % 2] = w2m
            w2_stage(NM - 1)
            for dt in range(ND):
                nc.vector.scalar_tensor_tensor(yT[dt][:, _hh(h)], pzf[dt][:],
                                               cpk[:, 24 + dt:25 + dt],
                                               znT[dt][:, _hh(h)],
                                               op0=ALU.add, op1=ALU.add)
            if h == 0:
                tail()
        return self._post_ln(s, yT, wk, yr=True)

    def _ff_hl(self, s, z3, wk, tail):
        nc = self.nc
        idn = self.idn
        cpk = self.cpk
        tail()
        rs, nmurs = self._ln_stats(z3, wk, "pre")
        h_ = [wk.tile([128, D], F32, tag=f"B2_{tt}", name=f"h{tt}")
              for tt in range(NT)]
        for tt in range(NT):
            nc.scalar.activation(h_[tt][:], z3[tt][:], AF.Identity,
                                 scale=rs[:, tt:tt + 1], bias=nmurs[:, tt:tt + 1])
        hT = [wk.tile([128, N], F32, tag=f"A2_{dt}", name=f"hT{dt}")
              for dt in range(ND)]
        for h in range(2):
            for dt in range(ND):
                pT = self.bank(dt)
                for q in range(4):
                    nc.tensor.transpose(pT[:, _sl(q)], h_[h * 4 + q][:, _sl(dt)],
                                        idn[:])
                if h == 0:
                    nc.scalar.copy(hT[dt][:, _hh(h)], pT[:])
                else:
                    nc.vector.tensor_copy(hT[dt][:, _hh(h)], pT[:])
        znT = [wk.tile([128, N], F32, tag=f"A1_{dt}", name=f"znT{dt}")
               for dt in range(ND)]
        for h in range(2):
            for dt in range(ND):
                nc.vector.tensor_scalar(znT[dt][:, _hh(h)], hT[dt][:, _hh(h)],
                                        cpk[:, dt:dt + 1],
                                        cpk[:, 4 + dt:5 + dt],
                                        op0=ALU.mult, op1=ALU.add)

        yT = [wk.tile([128, N], F32, tag=f"A2_{dt}", name=f"yT{dt}")
              for dt in range(ND)]
        for h in range(2):
            znb = [wk.tile([128, 1024], BF16, tag=f"B3_{kt}", name=f"znb{kt}")
                   for kt in range(ND)]
            for kt in range(ND):
                nc.vector.tensor_copy(znb[kt][:, 0:512], znT[kt][:, _hh(h)])
                nc.vector.tensor_sub(znb[kt][:, 512:1024], znT[kt][:, _hh(h)],
                                     znb[kt][:, 0:512])
            pzf = [self.bank(dt) for dt in range(ND)]
            for m in range(NM):
                w1m = wk.tile([128, 2 * ND * 128], BF16,
                              tag=f"w1mh{m % 2}", name="w1m")
                nc.sync.dma_start(w1m[:], self.d_ffw1t[m, :, :])
                ph = self.bank(4 + m % 2)
                for kt in range(ND):
                    nc.tensor.matmul(ph[:], w1m[:, _sl(kt)], znb[kt][:, 0:512],
                                     start=(kt == 0), stop=False)
                    nc.tensor.matmul(ph[:], w1m[:, _sl(kt)], znb[kt][:, 512:1024],
                                     start=False, stop=False)
                    nc.tensor.matmul(ph[:], w1m[:, 512 + 128 * kt:640 + 128 * kt],
                                     znb[kt][:, 0:512],
                                     start=False, stop=(kt == ND - 1))
                sig = wk.tile([128, 512], F32, tag=f"sig{m % 2}", name="sig")
                nc.scalar.activation(sig[:], ph[:], AF.Sigmoid,
                                     bias=cpk[:, 8 + m:9 + m])
                sighl = wk.tile([128, 1024], BF16,
                                tag="amp2" if m % 2 == 0 else "lnscr",
                                name="sighl")
                nc.vector.tensor_copy(sighl[:, 0:512], sig[:])
                nc.vector.tensor_sub(sighl[:, 512:1024], sig[:],
                                     sighl[:, 0:512])
                w2m = wk.tile([128, 1024], BF16, tag=f"w2m{m % 2}", name="w2m")
                nc.sync.dma_start(w2m[:], self.d_ffw2hl[_sl(m), :])
                for dt in range(ND):
                    nc.tensor.matmul(pzf[dt][:], w2m[:, _sl(dt)],
                                     sighl[:, 0:512],
                                     start=(m == 0), stop=False)
                    nc.tensor.matmul(pzf[dt][:], w2m[:, _sl(dt)],
                                     sighl[:, 512:1024],
                                     start=False, stop=False)
                    nc.tensor.matmul(pzf[dt][:], w2m[:, 512 + dt * 128:
                                                     640 + dt * 128],
                                     sighl[:, 0:512],
                                     start=False, stop=(m == NM - 1))
            for dt in range(ND):
                nc.vector.scalar_tensor_tensor(yT[dt][:, _hh(h)], pzf[dt][:],
                                               cpk[:, 24 + dt:25 + dt],
                                               znT[dt][:, _hh(h)],
                                               op0=ALU.add, op1=ALU.add)
        return self._post_ln(s, yT, wk, yr=False)

    def _post_ln(self, s, yT, wk, yr):
        # fully per-tt post-LN chains: z4[0] is ready before the last yT
        # transposes finish, so l1's rfft starts with no barrier on the
        # batched stats
        nc = self.nc
        idn = self.idn
        rfr = PREC["rfft1"] == "f32r"
        gb, bb = self.gbt, self.bbt
        z4 = [wk.tile([128, D], F32R, tag=f"B3_{tt}", name=f"z4_{tt}")
              for tt in range(NT)]
        if not rfr:
            zhl = [wk.tile([128, 1024], BF16, tag=f"B2_{tt}", name=f"zhl{tt}")
                   for tt in range(NT)]
        st = wk.tile([128, 8 * NT], F32, tag="stpost", name="stpost")
        for tt in range(NT):
            pT = self.bank(6 + tt % 2, dtype=F32R if yr else F32)
            for dt in range(ND):
                nc.tensor.transpose(pT[:, _sl(dt)], yT[dt][:, _sl(tt)],
                                    self.idnr[:] if yr else idn[:])
            y_t = wk.tile([128, D], F32, tag=f"B4_{tt}", name=f"y{tt}")
            nc.scalar.copy(y_t[:], pT[:])
            scr = wk.tile([128, D], F32,
                          tag="lnscr" if tt % 2 == 0 else "lnscr2",
                          name="lnscr")
            mu = st[:, tt:tt + 1]
            s2 = st[:, NT + tt:NT + tt + 1]
            nc.vector.tensor_reduce(mu, y_t[:], mybir.AxisListType.X,
                                    op=ALU.add)
            nc.scalar.activation(scr[:], y_t[:], AF.Square, accum_out=s2)
            mun = st[:, 2 * NT + tt:2 * NT + tt + 1]
            nc.vector.tensor_scalar_mul(mun, mu, 1.0 / D)
            musq = st[:, 3 * NT + tt:3 * NT + tt + 1]
            nc.scalar.activation(musq, mun, AF.Square)
            var = st[:, 4 * NT + tt:4 * NT + tt + 1]
            nc.vector.scalar_tensor_tensor(var, s2, 1.0 / D, musq,
                                           op0=ALU.mult, op1=ALU.subtract)
            sd = st[:, 5 * NT + tt:5 * NT + tt + 1]
            nc.scalar.activation(sd, var, AF.Sqrt, bias=self.epst[:, 0:1])
            rs = st[:, 6 * NT + tt:6 * NT + tt + 1]
            nc.vector.reciprocal(rs, sd)
            nmurs = st[:, 7 * NT + tt:7 * NT + tt + 1]
            nc.vector.scalar_tensor_tensor(nmurs, mun, -1.0, rs,
                                           op0=ALU.mult, op1=ALU.mult)
            nc.scalar.activation(scr[:], y_t[:], AF.Identity,
                                 scale=rs, bias=nmurs)
            nc.vector.tensor_mul(z4[tt][:], scr[:], gb[:])
            nc.vector.tensor_add(z4[tt][:], z4[tt][:], bb[:])
            if not rfr:
                nc.gpsimd.tensor_copy(zhl[tt][:, 0:512], z4[tt][:])
                nc.gpsimd.tensor_sub(zhl[tt][:, 512:1024], z4[tt][:],
                                     zhl[tt][:, 0:512])
        if rfr:
            return z4, None
        return z4, zhl

    # ---------- output head ----------
    def _output(self, s, wk):
        nc = self.nc
        ones = self.ones
        po = self.bank(7)
        for kt in range(ND):
            nc.tensor.matmul(po[0:TF, 0:HOR], self.outwt[:, kt * TF:(kt + 1) * TF],
                             self.aggt[:, kt * HOR:(kt + 1) * HOR],
                             start=(kt == 0), stop=False)
        nc.tensor.matmul(po[0:TF, 0:HOR], self.outbt[0:1, 0:TF],
                         ones[0:1, 0:HOR], start=False, stop=True)
        xfin = self._xnew_last
        oT = wk.tile([TF, HOR], F32, tag="lvv", name="oT")
        nc.vector.tensor_scalar_add(oT[:], po[0:TF, 0:HOR], xfin[:, N - 1:N])
        nc.gpsimd.dma_start(self.d_out[s * TF:(s + 1) * TF, :], oT[:])


def _get_nc():
    if "nc" not in _CACHE:
        _CACHE["nc"] = K().build()
    return _CACHE["nc"]


def _common_maps(inputs, w2d, dft, ib, e8):
    m = dict(
        w2d=_rne11(w2d) if PREC["l0head"] == "f32r" else w2d,
        ones1=np.ones((1, N), np.float32),
        idn=np.eye(128, dtype=np.float32),
        e8=e8,
        ibr=_rne11(ib),
        winr=_rne11(np.asarray(inputs["mhesa_win"], np.float32)),
        woutr=_rne11(np.asarray(inputs["mhesa_wout"], np.float32)),
        boutr=np.asarray(inputs["mhesa_bout"], np.float32).reshape(L, 1, D),
        lcolp=_pack_lcol(inputs),
        alpha8=np.asarray(inputs["mhesa_alpha"], np.float32).reshape(L, HEADS, 1),
        cpkp=_pack_cpk(inputs),
        gpostr=np.asarray(inputs["ff_post_g"], np.float32).reshape(1, D),
        bpostr=np.asarray(inputs["ff_post_b"], np.float32).reshape(1, D),
        lvwg=np.asarray(inputs["level_wg"], np.float32),
        lvwp=np.asarray(inputs["level_wp"], np.float32),
        lvbg=np.asarray(inputs["level_bg"], np.float32).reshape(L, 1, TF),
        lvbp=np.asarray(inputs["level_bp"], np.float32).reshape(L, 1, TF),
        lvalpha=np.asarray(inputs["level_alpha"], np.float32).reshape(L, 1, 1),
        damp8=np.asarray(inputs["dampen_factor"], np.float32).reshape(HEADS, 1),
        outw=np.asarray(inputs["out_w"], np.float32)
            .reshape(ND, 128, TF).transpose(1, 0, 2).reshape(128, ND * TF)
            .copy(),
        outbr=np.asarray(inputs["out_b"], np.float32).reshape(1, TF),
    )
    if PREC["l0head"] == "f32":
        m["dft"] = dft
    if PREC["l0head"] == "f32r" or PREC["rfft1"] == "f32r":
        m["dftr"] = _rne11(dft)
    if PREC["rfft1"] == "hl":
        m["dfthl"] = np.concatenate([_split_hi(dft), _split_lo(dft)], axis=1)
    if PREC["irfft0"] == "hl":
        m["ibhl"] = np.concatenate([_split_hi(ib), _split_lo(ib)], axis=1)
    if PREC["mhesa0"] == "hl":
        win0 = np.asarray(inputs["mhesa_win"][0], np.float32)
        wout0 = np.asarray(inputs["mhesa_wout"][0], np.float32)
        m["winhl"] = np.concatenate([_split_hi(win0), _split_lo(win0)], axis=1)
        m["wouthl"] = np.concatenate([_split_hi(wout0), _split_lo(wout0)],
                                     axis=1)
    w1 = np.asarray(inputs["ff_w1"], np.float32)
    w2 = np.asarray(inputs["ff_w2"], np.float32)
    if PREC["ff"] == "f32r":
        m["ffw1r"] = _rne11(_pack_w1r(w1))
        m["ffw2r"] = _rne11(w2)
    else:
        m["ffw1t"] = _pack_w1(w1)
        m["ffw2hl"] = np.concatenate([_split_hi(w2), _split_lo(w2)], axis=1)
    return m


def _kernel_impl(inputs, runner):
    x = np.asarray(inputs["x"], np.float32)
    assert (x.shape[0], x.shape[1], x.shape[2]) == (32, N, TF)
    assert int(inputs["forecast_horizon"]) == HOR
    dft, ib = _dft_consts()
    conv_w = np.asarray(inputs["conv_w"], np.float32)
    w2d = _build_w2d(conv_w, np.asarray(inputs["conv_b"], np.float32))
    e8 = np.repeat(np.eye(HEADS, dtype=np.float32), DH, axis=1)
    nc = _get_nc()
    common = _common_maps(inputs, w2d, dft, ib, e8)
    in_maps = []
    for c in range(NCORES):
        xs = x[c * S:(c + 1) * S]
        xT = xs.transpose(0, 2, 1).reshape(S * TF, N).copy()
        in_maps.append(dict(common, xT=xT))
    res = runner(nc, in_maps)
    out = np.zeros((x.shape[0], HOR, TF), np.float32)
    for c in range(NCORES):
        oT = res.results[c]["outT"].reshape(S, TF, HOR)
        out[c * S:(c + 1) * S] = oT.transpose(0, 2, 1)
    return out, res


def kernel(**inputs):
    out, _ = _kernel_impl(
        inputs,
        lambda nc, im: run_bass_kernel_spmd(nc, im, list(range(NCORES))))
    return out


def kernel_traced(**inputs):
    """Like kernel() but with NTFF profiling; returns (out, BassKernelResults)."""
    return _kernel_impl(
        inputs,
        lambda nc, im: run_bass_kernel_spmd(nc, im, list(range(NCORES)),
                                            trace=True))


# revision 47
# speedup vs baseline: 1.5770x; 1.0117x over previous
"""ETSFormer forward pass on 8 Trainium2 NeuronCores (Bass/Tile).

Data-parallel over batch: 32 samples -> 8 cores x 4 samples, weights
replicated, no collectives. The reference's FFT machinery is computed
exactly without FFTs:
  - freq_attention: dense DFT matmuls + hardware top-8 (vector.max) mask
  - mhesa / level exponential smoothing: the reference FFT cross-correlation
    is exactly a first-order EMA -> hardware prefix scan (tensor_tensor_scan)
  - fourier_extrapolate: exact slice (Dirichlet kernel identity)

Precision: PREC selects per-GEMM-group dtype. "f32r" = fp32-reduced
(FP22 truncated, 1 cyc/row on PE -- same speed as bf16) vs the fallback
"hl" = bf16 hi/lo 3-term split (~2^-16, 3 cyc/row) / "f32" = true fp32
(4 cyc/row). The top-4 frequency mask is rank-sensitive; flags are
tuned empirically against the end-to-end error gate.
"""
import numpy as np
from contextlib import ExitStack

import concourse.bass as bass
import concourse.bacc as bacc
import concourse.tile as tile
from concourse import mybir
from concourse.bass_utils import run_bass_kernel_spmd

F32 = mybir.dt.float32
F32R = mybir.dt.float32r
BF16 = mybir.dt.bfloat16
AF = mybir.ActivationFunctionType
ALU = mybir.AluOpType

N = 1024
D = 512
TF = 7
HEADS = 8
DH = D // HEADS
L = 2
S = 4
NCORES = 8
HOR = 96
FD = 2048
NT = N // 128   # 8
ND = D // 128   # 4
NM = FD // 128  # 16

_CACHE = {}

# per-stage precision: "f32r" fast path vs baseline "hl" (bf16 3-term)
# / "f32" (true fp32) fallback.
PREC = dict(
    l0head="f32",    # conv z GEMM + low-rank DFT: feeds the layer-0 top-4
                     # ranking, which flips even under 2^-12 weight rounding
                     # (emulation: 52 flips, 2.7e-2 err) -- keep exact fp32
    irfft0="f32r",   # layer-0 irfft (feeds layer-1 ranking path)
    mhesa0="f32r",   # layer-0 win/wout GEMMs
    ff="f32r",       # FF block w1/w2 GEMMs
    rfft1="f32r",    # layer-1 rfft (feeds layer-1 ranking directly)
)


def _rne11(x):
    # round fp32 mantissa to 11 explicit bits (fp22): the PE's f32r mode
    # truncates operands to fp22, so pre-rounded weights pass through
    # losslessly -- halves f32r noise and removes the truncation bias
    xi = np.ascontiguousarray(np.asarray(x, np.float32)).view(np.uint32)
    return ((xi + np.uint32(0x800)) & np.uint32(0xFFFFF000)).view(np.float32)


def _dft_consts():
    if "dft" not in _CACHE:
        t = np.arange(N)
        f = np.arange(513)
        ang = 2.0 * np.pi * np.outer(t, f) / N
        cos = np.cos(ang)
        sin = np.sin(ang)
        # B-half layout [sin(1..511), cos(512)] (cos512 moved to the END):
        # then amp2[f] = sqA[f] + sqB[f-1] for f=1..512 is a single shifted
        # add, and the f>=512 mask is ONE scalar_tensor_tensor over psB.
        dft = np.zeros((N, 1024), np.float64)
        dft[:, 0:512] = cos[:, 0:512]
        dft[:, 512:1023] = sin[:, 1:512]
        dft[:, 1023] = cos[:, 512]
        c = np.full(513, 2.0)
        c[0] = 1.0
        c[512] = 1.0
        ib = np.zeros((1024, N), np.float64)
        ib[0:512, :] = (c[0:512, None] / N) * cos[:, 0:512].T
        ib[512:1023, :] = (2.0 / N) * sin[:, 1:512].T
        ib[1023, :] = (1.0 / N) * cos[:, 512]
        _CACHE["dft"] = dft.astype(np.float32)
        _CACHE["ib"] = ib.astype(np.float32)
    return _CACHE["dft"], _CACHE["ib"]


def _sl(i, w=128):
    return slice(i * w, (i + 1) * w)


def _split_hi(x):
    import ml_dtypes
    return x.astype(ml_dtypes.bfloat16)


def _split_lo(x):
    import ml_dtypes
    hi = x.astype(ml_dtypes.bfloat16).astype(np.float32)
    return (x - hi).astype(ml_dtypes.bfloat16)


def _pack_w1(w1):
    # bf16 hi|lo tiles for the "hl" fallback FF path
    hi, lo = _split_hi(w1), _split_lo(w1)
    out = np.zeros((NM, 128, 1024), hi.dtype)
    for m in range(NM):
        for kt in range(ND):
            out[m, :, 128 * kt:128 * (kt + 1)] = hi[_sl(kt), _sl(m)]
            out[m, :, 512 + 128 * kt:640 + 128 * kt] = lo[_sl(kt), _sl(m)]
    return out


def _pack_w1r(w1):
    # f32r per-m contiguous [128(k), 4x128(m)] tiles
    out = np.zeros((NM, 128, 512), np.float32)
    for m in range(NM):
        for kt in range(ND):
            out[m, :, 128 * kt:128 * (kt + 1)] = w1[_sl(kt), _sl(m)]
    return out


def _pack_cpk(inputs):
    # cols: gpre(4) | bpre(4) | ffb1(16) | ffb2(4), each D/FD vector folded
    # into [128, k] column blocks -- one DMA instead of 24
    out = np.zeros((128, 28), np.float32)
    out[:, 0:4] = np.asarray(inputs["ff_pre_g"], np.float32).reshape(4, 128).T
    out[:, 4:8] = np.asarray(inputs["ff_pre_b"], np.float32).reshape(4, 128).T
    out[:, 8:24] = np.asarray(inputs["ff_b1"], np.float32).reshape(16, 128).T
    out[:, 24:28] = np.asarray(inputs["ff_b2"], np.float32).reshape(4, 128).T
    return out


def _pack_lcol(inputs):
    # per layer: init(4 cols) | bin(4 cols)
    out = np.zeros((L, 128, 8), np.float32)
    ini = np.asarray(inputs["mhesa_init"], np.float32).reshape(L, D)
    bi = np.asarray(inputs["mhesa_bin"], np.float32)
    for l in range(L):
        out[l, :, 0:4] = ini[l].reshape(4, 128).T
        out[l, :, 4:8] = bi[l].reshape(4, 128).T
    return out


def _build_w2d(conv_w, conv_b):
    # rows 32k+c hold conv_w[:, c, k] (32-aligned partition groups so the
    # on-device shifted copies keep legal base partitions); row 95 is the
    # bias row, paired with an all-ones row 95 of xsh on device.
    w2d = np.zeros((96, D), np.float32)
    for k in range(3):
        for c in range(TF):
            w2d[32 * k + c] = conv_w[:, c, k]
    w2d[95] = conv_b
    return w2d


def _hh(h):
    return slice(h * 512, (h + 1) * 512)


class K:
    def __init__(self):
        nc = bacc.Bacc()
        self.nc = nc
        p = nc.declare_dram_parameter
        self.d_xT = p("xT", [S * TF, N], F32, isOutput=False)
        self.d_w2d = p("w2d", [96, D],
                       F32R if PREC["l0head"] == "f32r" else F32,
                       isOutput=False)
        self.d_ones1 = p("ones1", [1, N], F32, isOutput=False)
        if PREC["l0head"] == "f32":
            self.d_dft = p("dft", [N, 1024], F32, isOutput=False)
        if PREC["l0head"] == "f32r" or PREC["rfft1"] == "f32r":
            self.d_dftr = p("dftr", [N, 1024], F32R, isOutput=False)
        if PREC["rfft1"] == "hl":
            self.d_dfthl = p("dfthl", [N, 2048], BF16, isOutput=False)
        self.d_ibr = p("ibr", [1024, N], F32R, isOutput=False)
        self.d_winr = p("winr", [L, D, D], F32R, isOutput=False)
        self.d_woutr = p("woutr", [L, D, D], F32R, isOutput=False)
        if PREC["irfft0"] == "hl":
            self.d_ibhl = p("ibhl", [1024, 2048], BF16, isOutput=False)
        if PREC["mhesa0"] == "hl":
            self.d_winhl = p("winhl", [D, 2 * D], BF16, isOutput=False)
            self.d_wouthl = p("wouthl", [D, 2 * D], BF16, isOutput=False)
        self.d_idn = p("idn", [128, 128], F32, isOutput=False)
        self.d_e8 = p("e8", [HEADS, D], F32, isOutput=False)
        self.d_bout = p("boutr", [L, 1, D], F32, isOutput=False)
        self.d_lcolp = p("lcolp", [L, 128, 8], F32, isOutput=False)
        self.d_al8 = p("alpha8", [L, HEADS, 1], F32, isOutput=False)
        if PREC["ff"] == "f32r":
            self.d_ffw1r = p("ffw1r", [NM, 128, 512], F32R, isOutput=False)
            self.d_ffw2r = p("ffw2r", [FD, D], F32R, isOutput=False)
        else:
            self.d_ffw1t = p("ffw1t", [NM, 128, 2 * ND * 128], BF16,
                             isOutput=False)
            self.d_ffw2hl = p("ffw2hl", [FD, 2 * D], BF16, isOutput=False)
        self.d_cpkp = p("cpkp", [128, 28], F32, isOutput=False)
        self.d_gpost = p("gpostr", [1, D], F32, isOutput=False)
        self.d_bpost = p("bpostr", [1, D], F32, isOutput=False)
        self.d_wg = p("lvwg", [L, D, TF], F32, isOutput=False)
        self.d_wp = p("lvwp", [L, D, TF], F32, isOutput=False)
        self.d_bg = p("lvbg", [L, 1, TF], F32, isOutput=False)
        self.d_bp = p("lvbp", [L, 1, TF], F32, isOutput=False)
        self.d_alv = p("lvalpha", [L, 1, 1], F32, isOutput=False)
        self.d_damp = p("damp8", [HEADS, 1], F32, isOutput=False)
        self.d_outw = p("outw", [128, ND * TF], F32, isOutput=False)
        self.d_outb = p("outbr", [1, TF], F32, isOutput=False)
        self.d_out = p("outT", [S * TF, HOR], F32, isOutput=True)
        self.xtmid = nc.dram_tensor("xtmid", [S, TF, N], F32)

    # psum bank helper: tag-based reuse of the 8 banks
    def bank(self, i, shape=(128, 512), dtype=F32):
        tl = self.psp.tile(list(shape), dtype, tag=f"bk{i}", name=f"bk{i}")
        return tl

    def build(self):
        nc = self.nc
        with ExitStack() as ctx:
            self.tc = ctx.enter_context(tile.TileContext(nc))
            tc = self.tc
            top = ctx.enter_context(tc.tile_pool(name="top", bufs=1))

            idn = top.tile([128, 128], F32, name="idn")
            nc.sync.dma_start(idn[:], self.d_idn[:])
            idnr = top.tile([128, 128], F32R, name="idnr")
            nc.vector.tensor_copy(idnr[:], idn[:])
            self.idnr = idnr
            ones = top.tile([128, 128], F32, name="ones")
            nc.vector.memset(ones[:], 1.0)
            w2d = top.tile([96, D],
                           F32R if PREC["l0head"] == "f32r" else F32,
                           name="w2d")
            nc.sync.dma_start(w2d[:], self.d_w2d[:])
            outbr = top.tile([1, TF], F32, name="outbr")
            nc.sync.dma_start(outbr[:], self.d_outb[:])
            self.outbt = outbr
            # col pack: gpre(4) | bpre(4)
            cpk = top.tile([128, 28], F32, name="cpk")
            nc.sync.dma_start(cpk[:], self.d_cpkp[:])
            outw = top.tile([128, ND * TF], F32, name="outw")
            nc.sync.dma_start(outw[:], self.d_outw[:])
            eps = top.tile([128, 1], F32, name="eps")
            nc.vector.memset(eps[:], 1e-5)
            self.epst = eps
            gbt = top.tile([128, D], F32, name="gbt")
            bbt = top.tile([128, D], F32, name="bbt")
            self.gbt, self.bbt = gbt, bbt
            agg = top.tile([128, ND * HOR], F32, name="agg")
            csd = top.tile([128, ND * HOR], F32, name="csd")

            self.idn, self.ones, self.cpk = idn, ones, cpk
            self.w2dt_, self.aggt, self.csdt = w2d, agg, csd
            self.outwt = outw

            self.psp = ctx.enter_context(
                tc.tile_pool(name="ps", bufs=1, space="PSUM"))
            # both layers' constants resident; samples run L0->L1 back to
            # back so L1's DVE-heavy tail overlaps the next sample's
            # PE-heavy head, and z4 never round-trips through DRAM
            lay0p = ctx.enter_context(tc.tile_pool(name="lay0", bufs=1))
            lay1p = ctx.enter_context(tc.tile_pool(name="lay1", bufs=1))
            with tc.tile_pool(name="ini", bufs=1) as ini:
                e8 = ini.tile([HEADS, D], F32, name="e8")
                nc.sync.dma_start(e8[:], self.d_e8[:])
                self.e8t = e8
                self._damp_cs(ini, self.psp)
                lay = [self._layer_consts(0, lay0p),
                       self._layer_consts(1, lay1p)]
            wk = ctx.enter_context(tc.tile_pool(name="wk", bufs=1))
            for s in range(S):
                z4 = self._sample(0, s, lay[0], wk)
                self._sample(1, s, lay[1], wk, zin=z4)
                self._output(s, wk)

        nc.compile()
        return nc

    # ---------- dampening cumsum -> csd [128, ND*HOR] ----------
    def _damp_cs(self, ini, inips):
        nc = self.nc
        ones = self.ones
        dcol = ini.tile([HEADS, 1], F32, name="dcol")
        nc.sync.dma_start(dcol[:], self.d_damp[:])
        df = ini.tile([HEADS, 1], F32, name="dfsig")
        nc.scalar.activation(df[:], dcol[:], AF.Sigmoid)
        dfb = ini.tile([HEADS, HOR], F32, name="dfb")
        nc.scalar.activation(dfb[:], ones[0:HEADS, 0:HOR], AF.Identity,
                             scale=df[:, 0:1])
        zer = ini.tile([HEADS, HOR], F32, name="zer8")
        nc.vector.memset(zer[:], 0.0)
        dfp = ini.tile([HEADS, HOR], F32, name="dfp")
        nc.vector.tensor_tensor_scan(dfp[:], dfb[:], zer[:], 1.0,
                                     op0=ALU.mult, op1=ALU.add)
        cs8 = ini.tile([HEADS, HOR], F32, name="cs8")
        nc.vector.tensor_tensor_scan(cs8[:], ones[0:HEADS, 0:HOR], dfp[:], 0.0,
                                     op0=ALU.mult, op1=ALU.add)
        for dt in range(ND):
            pini = inips.tile([128, HOR], F32, tag=f"bk{dt}", name="pini")
            nc.tensor.matmul(pini[:], self.e8t[:, _sl(dt)], cs8[:],
                             start=True, stop=True)
            nc.scalar.copy(self.csdt[:, dt * HOR:(dt + 1) * HOR], pini[:])
        # hoisted FF post-LN gamma/beta broadcasts (layer-invariant)
        rows = ini.tile([1, 1024], F32, name="rows")
        nc.sync.dma_start(rows[0:1, 0:512], self.d_gpost[:])
        nc.sync.dma_start(rows[0:1, 512:1024], self.d_bpost[:])
        pgb = inips.tile([128, D], F32, tag="bk4", name="pgb")
        nc.tensor.matmul(pgb[:], self.ones[0:1, 0:128],
                         rows[0:1, 0:512], start=True, stop=True)
        nc.scalar.copy(self.gbt[:], pgb[:])
        pbb = inips.tile([128, D], F32, tag="bk5", name="pbb")
        nc.tensor.matmul(pbb[:], self.ones[0:1, 0:128],
                         rows[0:1, 512:1024], start=True, stop=True)
        nc.scalar.copy(self.bbt[:], pbb[:])

    # ---------- per-layer constants ----------
    def _layer_consts(self, l, layp):
        nc = self.nc
        ones = self.ones
        last = l == L - 1
        lay = {"l": l, "last": last}

        if last or PREC["mhesa0"] == "f32r":
            win = [layp.tile([128, D], F32R, name=f"win{k}") for k in range(ND)]
            wout = [layp.tile([128, D], F32R, name=f"wout{k}")
                    for k in range(ND)]
            for kt in range(ND):
                nc.scalar.dma_start(win[kt][:], self.d_winr[l, _sl(kt), :])
                nc.scalar.dma_start(wout[kt][:], self.d_woutr[l, _sl(kt), :])
        else:
            # bf16 hi|lo packed (cols 0:512 hi, 512:1024 lo)
            win = [layp.tile([128, 2 * D], BF16, name=f"win{k}")
                   for k in range(ND)]
            wout = [layp.tile([128, 2 * D], BF16, name=f"wout{k}")
                    for k in range(ND)]
            for kt in range(ND):
                nc.scalar.dma_start(win[kt][:], self.d_winhl[_sl(kt), :])
                nc.scalar.dma_start(wout[kt][:], self.d_wouthl[_sl(kt), :])

        # lrows: p0 = bout[512]; p32 = bg[7] then bp at cols 16..23
        lrows = layp.tile([128, 512], F32, name="lrows")
        nc.sync.dma_start(lrows[0:1, 0:D], self.d_bout[l, :, :])
        nc.sync.dma_start(lrows[32:33, 0:TF], self.d_bg[l, :, :])
        nc.sync.dma_start(lrows[32:33, 16:16 + TF], self.d_bp[l, :, :])

        # bout broadcast [128, D] (replaces per-tile bias matmuls)
        boutb = layp.tile([128, D], F32, name="boutb")
        pbo = self.psp.tile([128, D], F32, tag="bk7", name="pbo")
        nc.tensor.matmul(pbo[:], ones[0:1, 0:128], lrows[0:1, 0:D],
                         start=True, stop=True)
        nc.scalar.copy(boutb[:], pbo[:])

        # lcol pack [128, 16]: al(4) oma(4) init(4) bi(4); plus lv cols [7,1]
        # cols 18/19: level bg/bp as [7,1] columns
        lcol = layp.tile([128, 24], F32, name="lcol")
        nc.sync.dma_start(lcol[0:TF, 18:19],
                          self.d_bg[l, :, :].rearrange("a b -> b a"))
        nc.sync.dma_start(lcol[0:TF, 19:20],
                          self.d_bp[l, :, :].rearrange("a b -> b a"))
        al8 = layp.tile([HEADS, 1], F32, tag="al8t", name="al8")
        nc.sync.dma_start(al8[:], self.d_al8[l, :, :])
        al8s = layp.tile([HEADS, 1], F32, tag="al8s", name="al8s")
        nc.scalar.activation(al8s[:], al8[:], AF.Sigmoid)
        for dt in range(ND):
            pal = self.psp.tile([128, 1], F32, tag="bk0", name="pal")
            nc.tensor.matmul(pal[:], self.e8t[:, _sl(dt)], al8s[:],
                             start=True, stop=True)
            nc.scalar.copy(lcol[:, dt:dt + 1], pal[:])
        nc.sync.dma_start(lcol[:, 8:16], self.d_lcolp[l, :, :])
        for dt in range(ND):
            nc.vector.tensor_scalar(lcol[:, 4 + dt:5 + dt], lcol[:, dt:dt + 1],
                                    -1.0, 1.0, op0=ALU.mult, op1=ALU.add)
        nc.vector.tensor_sub(lcol[:, 12:16], lcol[:, 12:16], lcol[:, 8:12])
        # col 20:24 = al*(bi-init) + (1-al)*init -- the scan-initial folded
        # into xd[0] so the scan can run with a 0.0 immediate initial
        bi = layp.tile([128, ND], F32, tag="bitmp", name="bitmp")
        nc.vector.tensor_mul(lcol[:, 20:24], lcol[:, 0:4], lcol[:, 12:16])
        nc.vector.tensor_mul(bi[:], lcol[:, 4:8], lcol[:, 8:12])
        nc.vector.tensor_add(lcol[:, 20:24], lcol[:, 20:24], bi[:])
        # level alpha
        alv = layp.tile([1, 1], F32, tag="alvt", name="alv")
        nc.sync.dma_start(alv[:], self.d_alv[l, :, :])
        alvs = layp.tile([1, 1], F32, tag="alvst", name="alvs")
        nc.scalar.activation(alvs[:], alv[:], AF.Sigmoid)
        pv = self.psp.tile([TF, 1], F32, tag="bk1", name="palv")
        nc.tensor.matmul(pv[:], ones[0:1, 0:TF], alvs[:], start=True, stop=True)
        nc.scalar.copy(lcol[0:TF, 16:17], pv[:])
        nc.vector.tensor_scalar(lcol[0:TF, 17:18], lcol[0:TF, 16:17], -1.0, 1.0,
                                op0=ALU.mult, op1=ALU.add)

        # level weights [128, TF] x4 packed [128, 2*ND*TF], as fp32r
        lwf = layp.tile([128, 2 * ND * TF], F32, tag="lwf", name="lwf")
        for kt in range(ND):
            nc.sync.dma_start(lwf[:, kt * TF:(kt + 1) * TF], self.d_wg[l, _sl(kt), :])
            nc.sync.dma_start(lwf[:, (ND + kt) * TF:(ND + kt + 1) * TF],
                              self.d_wp[l, _sl(kt), :])
        lw = layp.tile([128, 2 * ND * TF], F32R, name="lw")
        nc.vector.tensor_copy(lw[:], lwf[:])

        lay.update(win=win, wout=wout, lrows=lrows, lcol=lcol, lw=lw,
                   boutb=boutb)
        return lay

    # ---------- one sample through one layer ----------
    def _sample(self, l, s, lay, wk, zin=None):
        nc = self.nc
        ones, idn = self.ones, self.idn
        last = lay["last"]
        agg = self.aggt
        irf_r = last or PREC["irfft0"] == "f32r"
        mh_r = last or PREC["mhesa0"] == "f32r"

        def aggsl(dt):
            return self.aggt[:, dt * HOR:(dt + 1) * HOR]

        # --- z input: conv (l0) or handed over in SBUF from l0 (l1)
        if l == 0:
            hr = PREC["l0head"] == "f32r"
            # agg is per-sample now; clear it (waits on prior _output read)
            nc.gpsimd.memset(agg[:], 0.0)
            z = [wk.tile([128, D], F32R, tag=f"B1_{tt}", name=f"z{tt}")
                 for tt in range(NT)]
            # low-rank path: x is rank-7, so z = xsh^T @ w2d (rows 32k+c hold
            # the 3 shifts of the 7 channels; row 95 = ones * conv_b) and
            # DFT(z) = w2d^T @ (xsh^T @ dft) -- the DFT runs in the 96-dim
            # input space instead of the 512-dim channel space.
            xshf = wk.tile([96, N], F32, tag="xsh", name="xshf")
            xts = wk.tile([TF, N], F32, tag="xts", name="xts")
            nc.sync.dma_start(xts[:], self.d_xT[s * TF:(s + 1) * TF, :])
            nc.gpsimd.memset(xshf[:], 0.0)
            nc.gpsimd.tensor_copy(xshf[0:TF, 1:N], xts[:, 0:N - 1])
            nc.gpsimd.tensor_copy(xshf[32:32 + TF, 0:N], xts[:, 0:N])
            nc.gpsimd.tensor_copy(xshf[64:64 + TF, 0:N - 1], xts[:, 1:N])
            nc.sync.dma_start(xshf[95:96, :], self.d_ones1[:])
            if hr:
                # Pool can't touch f32r (ISA); one DVE copy re-tags for PE
                xsh = wk.tile([96, N], F32R, tag="xshr", name="xsh")
                nc.vector.tensor_copy(xsh[:], xshf[:])
            else:
                xsh = xshf
            xshT = [wk.tile([128, 96], F32R if hr else F32,
                            tag=f"xshT{tt}", name=f"xshT{tt}")
                    for tt in range(NT)]
            psF1A = self.bank(2, shape=(96, 512))
            psF1B = self.bank(3, shape=(96, 512))
            tid = self.idnr if hr else idn
            d_dft_src = self.d_dftr if hr else self.d_dft
            for tt in range(NT):
                pz = self.bank(tt % 2)
                nc.tensor.matmul(pz[:], xsh[:, _sl(tt)], self.w2dt_[:],
                                 start=True, stop=True)
                nc.scalar.copy(z[tt][:], pz[:])
                pxT = self.bank(6, shape=(128, 96),
                                dtype=F32R if hr else F32)
                nc.tensor.transpose(pxT[:], xsh[:, _sl(tt)], tid[0:96, 0:96])
                nc.scalar.copy(xshT[tt][:], pxT[:])
                dftk = wk.tile([128, 1024], F32R if hr else F32,
                               tag=f"dftk{tt % 2}", name="dftk")
                nc.sync.dma_start(dftk[:], d_dft_src[_sl(tt), :])
                nc.tensor.matmul(psF1A[:], xshT[tt][:], dftk[:, 0:512],
                                 start=(tt == 0), stop=(tt == NT - 1))
                nc.tensor.matmul(psF1B[:], xshT[tt][:], dftk[:, 512:1024],
                                 start=(tt == 0), stop=(tt == NT - 1))
            F1s = wk.tile([96, 1024], F32R if hr else F32, tag="lvp",
                          name="F1s")
            nc.scalar.copy(F1s[:, 0:512], psF1A[:])
            nc.scalar.copy(F1s[:, 512:1024], psF1B[:])
            psA = [self.bank(ct) for ct in range(ND)]
            psB = [self.bank(4 + ct) for ct in range(ND)]
            for ct in range(ND):
                nc.tensor.matmul(psA[ct][:], self.w2dt_[:, _sl(ct)],
                                 F1s[:, 0:512], start=True, stop=True)
                nc.tensor.matmul(psB[ct][:], self.w2dt_[:, _sl(ct)],
                                 F1s[:, 512:1024], start=True, stop=True)
            ibkpf = []
            if irf_r:
                for pf in range(2):
                    ibkp = wk.tile([128, 1024], F32R, tag=f"dftk{pf % 2}",
                                   name="ibk")
                    nc.sync.dma_start(ibkp[:], self.d_ibr[_sl(pf), :])
                    ibkpf.append(ibkp)
            else:
                for pf in range(2):
                    ibkp = wk.tile([128, 2048], BF16, tag=f"dftk{pf % 2}",
                                   name="ibk")
                    nc.sync.dma_start(ibkp[:], self.d_ibhl[_sl(pf), :])
                    ibkpf.append(ibkp)
        else:
            z, zhl = zin

            psA = [self.bank(ct) for ct in range(ND)]
            psB = [self.bank(4 + ct) for ct in range(ND)]
            if PREC["rfft1"] == "f32r":
                # z tiles are F32R [t, d]; stationary slice [t, c-block]
                pfs = getattr(self, "_dftk_pf", None)
                self._dftk_pf = None
                for kt in range(NT):
                    if pfs is not None and kt < 2:
                        dftk = pfs[kt]
                    else:
                        dftk = wk.tile([128, 1024], F32R, tag=f"dftk{kt % 2}",
                                       name="dftk")
                        nc.sync.dma_start(dftk[:], self.d_dftr[_sl(kt), :])
                    st0 = kt == 0
                    sp = kt == NT - 1
                    for ct in range(ND):
                        zst = z[kt][:, _sl(ct)]
                        nc.tensor.matmul(psA[ct][:], zst, dftk[:, 0:512],
                                         start=st0, stop=sp)
                        nc.tensor.matmul(psB[ct][:], zst, dftk[:, 512:1024],
                                         start=st0, stop=sp)
            else:
                # rfft via bf16 hi/lo 3-term split (exact to ~2^-17)
                for kt in range(NT):
                    dftk = wk.tile([128, 2048], BF16, tag=f"dftk{kt % 2}",
                                   name="dftk")
                    nc.sync.dma_start(dftk[:], self.d_dfthl[_sl(kt), :])
                    st0 = kt == 0
                    sp = kt == NT - 1
                    for ct in range(ND):
                        zh = zhl[kt][:, _sl(ct)]
                        zl = zhl[kt][:, 512 + 128 * ct:640 + 128 * ct]
                        nc.tensor.matmul(psA[ct][:], zh, dftk[:, 0:512],
                                         start=st0, stop=False)
                        nc.tensor.matmul(psA[ct][:], zh, dftk[:, 1024:1536],
                                         start=False, stop=False)
                        nc.tensor.matmul(psB[ct][:], zh, dftk[:, 512:1024],
                                         start=st0, stop=False)
                        nc.tensor.matmul(psB[ct][:], zh, dftk[:, 1536:2048],
                                         start=False, stop=False)
                        nc.tensor.matmul(psA[ct][:], zl, dftk[:, 0:512],
                                         start=False, stop=sp)
                        nc.tensor.matmul(psB[ct][:], zl, dftk[:, 512:1024],
                                         start=False, stop=sp)
            # prefetch the first two irfft ib stripes while the mask runs
            ibkpf = []
            for pf in range(2):
                ibkp = wk.tile([128, 1024], F32R, tag=f"dftk{pf % 2}",
                               name="ibk")
                nc.sync.dma_start(ibkp[:], self.d_ibr[_sl(pf), :])
                ibkpf.append(ibkp)

        # --- top-4 mask -> filt [ND][128, 1024] ([c, f])
        # Pool has no PSUM port: psA/psB land in SBUF once (ACT), then the
        # whole chain (squares, add, is_ge mask) runs on the idle Pool
        # engine; only the top-8 max needs DVE.
        filt = [wk.tile([128, 1024], F32R if irf_r else F32,
                        tag=f"A1_{ct}", name=f"filt{ct}")
                for ct in range(ND)]
        for ct in range(ND):
            amp2 = wk.tile([128, 513], F32,
                           tag="amp2" if ct % 2 == 0 else "lnscr", name="amp2")
            nc.scalar.activation(amp2[:, 0:512], psA[ct][:], AF.Square)
            sqB = wk.tile([128, 512], F32,
                          tag="w2m0" if ct % 2 == 0 else "w2m1", name="sqB")
            nc.scalar.activation(sqB[:], psB[ct][:], AF.Square)
            # permuted B-half: amp2[f] = sqA[f] + sqB[f-1], amp2[512]=sqB[511]
            nc.gpsimd.tensor_add(amp2[:, 1:512], amp2[:, 1:512], sqB[:, 0:511])
            nc.gpsimd.tensor_copy(amp2[:, 512:513], sqB[:, 511:512])
            top8 = wk.tile([128, 8], F32, tag="top8", name="top8")
            nc.vector.max(top8[:], amp2[:])
            kth = top8[:, 3:4]
            nc.vector.scalar_tensor_tensor(filt[ct][:, 0:512], amp2[:, 0:512],
                                           kth, psA[ct][:],
                                           op0=ALU.is_ge, op1=ALU.mult)
            nc.vector.scalar_tensor_tensor(filt[ct][:, 512:1024], amp2[:, 1:513],
                                           kth, psB[ct][:],
                                           op0=ALU.is_ge, op1=ALU.mult)

        # --- transpose filt -> filtT [f, c]; hl splits to bf16 hi|lo
        if irf_r:
            filtT = [wk.tile([128, 512], F32R, tag=f"B2_{ft}",
                             name=f"filtT{ft}") for ft in range(NT)]
            for ft in range(NT):
                pT = self.bank(ft % 4, dtype=F32R)
                for ct in range(ND):
                    nc.tensor.transpose(pT[:, _sl(ct)], filt[ct][:, _sl(ft)],
                                        self.idnr[:])
                if ft % 2 == 0:
                    nc.scalar.copy(filtT[ft][:], pT[:])
                else:
                    nc.vector.tensor_copy(filtT[ft][:], pT[:])
        else:
            filtT = [wk.tile([128, 1024], BF16, tag=f"B2_{ft}",
                             name=f"fthl{ft}") for ft in range(NT)]
            for ft in range(NT):
                pT = self.bank(ft % 4)
                for ct in range(ND):
                    nc.tensor.transpose(pT[:, _sl(ct)], filt[ct][:, _sl(ft)],
                                        idn[:])
                nc.scalar.copy(filtT[ft][:, 0:512], pT[:])
                nc.vector.tensor_sub(filtT[ft][:, 512:1024], pT[:],
                                     filtT[ft][:, 0:512])

        # --- irfft (ib streamed, 8 banks) -> lp, z2
        pl = [self.bank(tt) for tt in range(NT)]
        if irf_r:
            for ft in range(NT):
                if ft < 2:
                    ibk = ibkpf[ft]
                else:
                    ibk = wk.tile([128, 1024], F32R, tag=f"dftk{ft % 2}",
                                  name="ibk")
                    nc.sync.dma_start(ibk[:], self.d_ibr[_sl(ft), :])
                for tt in range(NT):
                    nc.tensor.matmul(pl[tt][:], ibk[:, _sl(tt)], filtT[ft][:],
                                     start=(ft == 0), stop=(ft == NT - 1))
        else:
            for ft in range(NT):
                if ft < 2:
                    ibk = ibkpf[ft]
                else:
                    ibk = wk.tile([128, 2048], BF16, tag=f"dftk{ft % 2}",
                                  name="ibk")
                    nc.sync.dma_start(ibk[:], self.d_ibhl[_sl(ft), :])
                for tt in range(NT):
                    ibh = ibk[:, _sl(tt)]
                    ibl = ibk[:, 1024 + 128 * tt:1152 + 128 * tt]
                    nc.tensor.matmul(pl[tt][:], ibh, filtT[ft][:, 0:512],
                                     start=(ft == 0), stop=False)
                    nc.tensor.matmul(pl[tt][:], ibh, filtT[ft][:, 512:1024],
                                     start=False, stop=False)
                    nc.tensor.matmul(pl[tt][:], ibl, filtT[ft][:, 0:512],
                                     start=False, stop=(ft == NT - 1))
        lp = [wk.tile([128, D], F32R, tag=f"B3_{tt}", name=f"lp{tt}")
              for tt in range(NT)]
        z2 = [wk.tile([128, D], F32R if mh_r else F32,
                      tag=f"B4_{tt}", name=f"z2_{tt}")
              for tt in range(NT)]
        for tt in range(NT):
            # z2 before lp: in l1 the lp tiles reuse z's memory (tag B3)
            nc.vector.tensor_sub(z2[tt][:], z[tt][:], pl[tt][:])
            nc.scalar.copy(lp[tt][:], pl[tt][:])

        # --- lpT [ND][128, N] (tag A2) + extrap + perT; then free
        lpT = [wk.tile([128, N], F32R, tag=f"A2_{dt}", name=f"lpT{dt}")
               for dt in range(ND)]
        perT = wk.tile([TF, N], F32, tag="dftk0", name="perT")
        for h in range(2):
            for dt in range(ND):
                pT = self.bank(dt, dtype=F32R)
                for q in range(4):
                    nc.tensor.transpose(pT[:, _sl(q)], lp[h * 4 + q][:, _sl(dt)],
                                        self.idnr[:])
                if h == 0:
                    nc.vector.tensor_copy(lpT[dt][:, _hh(h)], pT[:])
                    nc.vector.tensor_add(aggsl(dt), aggsl(dt),
                                         lpT[dt][:, 0:HOR])
                else:
                    nc.vector.tensor_copy(lpT[dt][:, _hh(h)], pT[:])
            # perT for this half right away: fills the PE wait on the next
            # half's lp copies
            pp = self.bank(4 + h)
            for kt in range(ND):
                nc.tensor.matmul(pp[0:TF, :], lay["lw"][:, (ND + kt) * TF:(ND + kt + 1) * TF],
                                 lpT[kt][:, _hh(h)],
                                 start=(kt == 0), stop=(kt == ND - 1))
            nc.scalar.copy(perT[:, _hh(h)], pp[0:TF, :])

        # --- z2T (tag A2 reuse after lpT dead); hl packs bf16 hi|lo
        if mh_r:
            z2T = [wk.tile([128, N], F32R, tag=f"A2_{dt}", name=f"z2T{dt}")
                   for dt in range(ND)]
            for h in range(2):
                for dt in range(ND):
                    pT = self.bank(dt, dtype=F32R)
                    for q in range(4):
                        nc.tensor.transpose(pT[:, _sl(q)],
                                            z2[h * 4 + q][:, _sl(dt)],
                                            self.idnr[:])
                    nc.vector.tensor_copy(z2T[dt][:, _hh(h)], pT[:])
        else:
            z2T = [wk.tile([128, 2 * N], BF16, tag=f"A2_{dt}",
                           name=f"z2Thl{dt}") for dt in range(ND)]
            for h in range(2):
                for dt in range(ND):
                    pT = self.bank(dt)
                    for q in range(4):
                        nc.tensor.transpose(pT[:, _sl(q)],
                                            z2[h * 4 + q][:, _sl(dt)], idn[:])
                    nc.scalar.copy(z2T[dt][:, _hh(h)], pT[:])
                    nc.vector.tensor_sub(
                        z2T[dt][:, N + 512 * h:N + 512 * h + 512], pT[:],
                        z2T[dt][:, _hh(h)])

        # --- win GEMM -> xinT -> xd -> scan, interleaved per dt so the
        # serial DVE scan chain overlaps the next dt's win GEMMs on PE
        xinT = [wk.tile([128, N], F32, tag=f"A1_{dt}", name=f"xinT{dt}")
                for dt in range(ND)]
        lc = lay["lcol"]
        if mh_r:
            sT = [wk.tile([128, N], F32R, tag=f"A2_{dt}", name=f"sT{dt}")
                  for dt in range(ND)]
            sTsc = sT
            for dt in range(ND):
                for h in range(2):
                    px = self.bank(4 + h)
                    for kt in range(ND):
                        nc.tensor.matmul(px[:], lay["win"][kt][:, _sl(dt)],
                                         z2T[kt][:, _hh(h)],
                                         start=(kt == 0), stop=(kt == ND - 1))
                    # fold the per-head alpha scale into the psum->sbuf copy
                    nc.scalar.activation(xinT[dt][:, _hh(h)], px[:],
                                         AF.Identity,
                                         scale=lay["lcol"][:, dt:dt + 1])
                eng = nc.vector if dt % 2 == 0 else nc.gpsimd
                xd = wk.tile([128, N], F32,
                             tag="xdsc0" if dt % 2 == 0 else "xdsc1",
                             name="xd")
                eng.tensor_sub(xd[:, 1:N], xinT[dt][:, 1:N],
                               xinT[dt][:, 0:N - 1])
                # xinT is pre-scaled by alpha; col 20+dt folds the initial
                nc.vector.tensor_scalar_add(xd[:, 0:1], xinT[dt][:, 0:1],
                                            lc[:, 20 + dt:21 + dt])
                omab_ap = lc[:, 4 + dt:5 + dt].broadcast_to([128, N])
                nc.vector.tensor_tensor_scan(sTsc[dt][:], omab_ap, xd[:], 0.0,
                                             op0=ALU.mult, op1=ALU.add)
        else:
            for h in range(2):
                for dt in range(ND):
                    px = self.bank(4 + dt % 2)
                    for kt in range(ND):
                        wh = lay["win"][kt][:, _sl(dt)]
                        wl = lay["win"][kt][:, 512 + 128 * dt:640 + 128 * dt]
                        zh = z2T[kt][:, _hh(h)]
                        zl = z2T[kt][:, N + 512 * h:N + 512 * h + 512]
                        nc.tensor.matmul(px[:], wh, zh,
                                         start=(kt == 0), stop=False)
                        nc.tensor.matmul(px[:], wh, zl,
                                         start=False, stop=False)
                        nc.tensor.matmul(px[:], wl, zh,
                                         start=False, stop=(kt == ND - 1))
                    nc.scalar.activation(xinT[dt][:, _hh(h)], px[:],
                                         AF.Identity,
                                         scale=lay["lcol"][:, dt:dt + 1])
            sTsc = [wk.tile([128, N], F32, tag=f"A1_{dt}", name=f"sTf{dt}")
                    for dt in range(ND)]
            sT = [wk.tile([128, 2 * N], BF16, tag=f"A2_{dt}",
                          name=f"sThl{dt}") for dt in range(ND)]
            for dt in range(ND):
                eng = nc.vector if dt % 2 == 0 else nc.gpsimd
                xd = wk.tile([128, N], F32,
                             tag="xdsc0" if dt % 2 == 0 else "xdsc1",
                             name="xd")
                eng.tensor_sub(xd[:, 1:N], xinT[dt][:, 1:N],
                               xinT[dt][:, 0:N - 1])
                nc.vector.tensor_scalar_add(xd[:, 0:1], xinT[dt][:, 0:1],
                                            lc[:, 20 + dt:21 + dt])
                omab_ap = lc[:, 4 + dt:5 + dt].broadcast_to([128, N])
                nc.vector.tensor_tensor_scan(sTsc[dt][:], omab_ap, xd[:], 0.0,
                                             op0=ALU.mult, op1=ALU.add)
                eng.tensor_copy(sT[dt][:, 0:N], sTsc[dt][:])
                eng.tensor_sub(sT[dt][:, N:2 * N], sTsc[dt][:],
                               sT[dt][:, 0:N])

        # --- wout GEMM -> lg [t,d] (tag B2 reuse: filtT dead) (+ z3 if l0)
        # pre-LN stats chains interleave per tt right behind the z3 subs so
        # DVE starts them 8 tiles earlier than a post-wout batch would
        prep = None
        if not last and PREC["ff"] == "f32r":
            stpre = wk.tile([128, 8 * NT], F32, tag="stpre", name="stpre")
            h_ = [wk.tile([128, D], F32R, tag=f"B4_{tt}", name=f"h{tt}")
                  for tt in range(NT)]

            def prep(tt):
                scr = wk.tile([128, D], F32,
                              tag="lnscr" if tt % 2 == 0 else "lnscr2",
                              name="lnscr")
                st = stpre
                mu = st[:, tt:tt + 1]
                s2 = st[:, NT + tt:NT + tt + 1]
                nc.vector.tensor_reduce(mu, z[tt][:], mybir.AxisListType.X,
                                        op=ALU.add)
                nc.scalar.activation(scr[:], z[tt][:], AF.Square, accum_out=s2)
                mun = st[:, 2 * NT + tt:2 * NT + tt + 1]
                nc.vector.tensor_scalar_mul(mun, mu, 1.0 / D)
                musq = st[:, 3 * NT + tt:3 * NT + tt + 1]
                nc.scalar.activation(musq, mun, AF.Square)
                var = st[:, 4 * NT + tt:4 * NT + tt + 1]
                nc.vector.scalar_tensor_tensor(var, s2, 1.0 / D, musq,
                                               op0=ALU.mult, op1=ALU.subtract)
                sd = st[:, 5 * NT + tt:5 * NT + tt + 1]
                nc.scalar.activation(sd, var, AF.Sqrt, bias=self.epst[:, 0:1])
                rs = st[:, 6 * NT + tt:6 * NT + tt + 1]
                nc.vector.reciprocal(rs, sd)
                nmurs = st[:, 7 * NT + tt:7 * NT + tt + 1]
                nc.vector.scalar_tensor_tensor(nmurs, mun, -1.0, rs,
                                               op0=ALU.mult, op1=ALU.mult)
                nc.scalar.activation(h_[tt][:], z[tt][:], AF.Identity,
                                     scale=rs, bias=nmurs)
        lg = [wk.tile([128, D], F32R, tag=f"B2_{tt}", name=f"lg{tt}")
              for tt in range(NT)]
        for tt in range(NT):
            # 4-deep rotation: the interleaved prep chains sit between the
            # pg reads on DVE, so a 2-bank rotation WAR-throttles the PE
            pg = self.bank(tt % 4)
            if mh_r:
                for kt in range(ND):
                    nc.tensor.matmul(pg[:], sT[kt][:, _sl(tt)],
                                     lay["wout"][kt][:],
                                     start=(kt == 0), stop=(kt == ND - 1))
            else:
                for kt in range(ND):
                    sh = sT[kt][:, _sl(tt)]
                    sl_ = sT[kt][:, N + 128 * tt:N + 128 * tt + 128]
                    nc.tensor.matmul(pg[:], sh, lay["wout"][kt][:, 0:512],
                                     start=(kt == 0), stop=False)
                    nc.tensor.matmul(pg[:], sh, lay["wout"][kt][:, 512:1024],
                                     start=False, stop=False)
                    nc.tensor.matmul(pg[:], sl_, lay["wout"][kt][:, 0:512],
                                     start=False, stop=(kt == ND - 1))
            nc.vector.tensor_add(lg[tt][:], pg[:], lay["boutb"][:])
            if not last:
                # z3 overwrites z (tag B1): z dead after z2
                nc.vector.tensor_sub(z[tt][:], z2[tt][:], lg[tt][:])
                if prep is not None:
                    prep(tt)
        z3 = z

        def emit_tail():
            # lglast/lgT/grT/damp/level-step. For l0 this is DEFERRED into
            # the FF (emitted after the h0 GEMM loop) so its PE work (lgT,
            # grT on banks 2/3) and DVE work overlap the FF GEMMs instead
            # of stalling the pre-LN stats chain.
            lglast = wk.tile([1, D], F32, tag="sqA", name="lglast")
            nc.gpsimd.dma_start(lglast[:], lg[NT - 1][127:128, :])
            lgl4 = wk.tile([128, ND], F32, tag="top8", name="lgl4")
            pTl = self.bank(2, shape=(128, ND))
            for dt in range(ND):
                nc.tensor.matmul(pTl[:, dt:dt + 1], lglast[0:1, _sl(dt)],
                                 ones[0:1, 0:1], start=True, stop=True)
            nc.scalar.copy(lgl4[:], pTl[:])

            # lgT via transposes (tag A1 reuse: xinT dead)
            lgT = [wk.tile([128, N], F32R, tag=f"A1_{dt}", name=f"lgT{dt}")
                   for dt in range(ND)]
            for h in range(2):
                for dt in range(ND):
                    pT = self.bank(2 + dt % 2, dtype=F32R)
                    for q in range(4):
                        nc.tensor.transpose(pT[:, _sl(q)],
                                            lg[h * 4 + q][:, _sl(dt)],
                                            self.idnr[:])
                    if h == 0:
                        nc.scalar.copy(lgT[dt][:, _hh(h)], pT[:])
                    else:
                        nc.vector.tensor_copy(lgT[dt][:, _hh(h)], pT[:])
            for dt in range(ND):
                # damp: agg += lg_last * csd
                nc.vector.scalar_tensor_tensor(
                    aggsl(dt), self.csdt[:, dt * HOR:(dt + 1) * HOR],
                    lgl4[:, dt:dt + 1], aggsl(dt), op0=ALU.mult, op1=ALU.add)

            # level: grT; scans update xtmid
            grT = wk.tile([TF, N], F32, tag="grT", name="grT")
            for h in range(2):
                pgr = self.bank(2 + h)
                for kt in range(ND):
                    nc.tensor.matmul(pgr[0:TF, :],
                                     lay["lw"][:, kt * TF:(kt + 1) * TF],
                                     lgT[kt][:, _hh(h)],
                                     start=(kt == 0), stop=(kt == ND - 1))
                # fold level bg bias (lcol col 18) into the psum->sbuf copy
                nc.vector.tensor_scalar_add(grT[:, _hh(h)], pgr[0:TF, :],
                                            lc[0:TF, 18:19])

            xts2 = wk.tile([TF, N], F32, tag="xts", name="xts2")
            if l == 0:
                nc.sync.dma_start(xts2[:], self.d_xT[s * TF:(s + 1) * TF, :])
            else:
                nc.sync.dma_start(xts2[:], self.xtmid[s, :, :])
            v = wk.tile([TF, N], F32, tag="lvv", name="lvv")
            # v = (xts2 - bp) - perT (DVE: Pool has no TensorScalarPtr)
            nc.vector.scalar_tensor_tensor(v[:], xts2[:], lc[0:TF, 19:20],
                                           perT[:],
                                           op0=ALU.subtract, op1=ALU.subtract)
            nc.vector.tensor_scalar_mul(v[:], v[:], lc[0:TF, 16:17])
            omlv_ap = lc[0:TF, 17:18].broadcast_to([TF, N])
            pt = wk.tile([TF, N], F32, tag="lvp", name="lvp")
            nc.vector.tensor_tensor_scan(pt[:], omlv_ap, v[:], 0.0,
                                         op0=ALU.mult, op1=ALU.add)
            gt = wk.tile([TF, N], F32, tag="lvv", name="lvg")
            nc.vector.tensor_tensor_scan(gt[:], omlv_ap, grT[:], 0.0,
                                         op0=ALU.mult, op1=ALU.add)
            xnew = wk.tile([TF, N], F32, tag="grT", name="xnew")
            nc.gpsimd.tensor_add(xnew[:], pt[:], gt[:])
            if l == 0:
                # on Pool: keeps this late-blocking store off the DMA queues
                nc.gpsimd.dma_start(self.xtmid[s, :, :], xnew[:])
            else:
                # l1's level output feeds only _output: skip the DRAM trip
                self._xnew_last = xnew
            if l == 0 and PREC["rfft1"] == "f32r":
                # prefetch l1's first two rfft dft stripes on the ACT hwdge
                # queue: the SP queue is still draining FF w1/w2 triggers
                pfs = []
                for i in range(2):
                    t = wk.tile([128, 1024], F32R, tag=f"dftk{i}",
                                name="dftkpf")
                    nc.scalar.dma_start(t[:], self.d_dftr[_sl(i), :])
                    pfs.append(t)
                self._dftk_pf = pfs

        # --- FF (layer 0 only); z4 stays in SBUF for l1
        if not last:
            return self._ff(s, z3, wk, emit_tail,
                            h_ if prep is not None else None)
        emit_tail()
        return None

    # ---------- LN stats ----------
    def _ln_stats(self, zset, wk, tagp):
        nc = self.nc
        st = wk.tile([128, 8 * NT], F32, tag=f"st{tagp}", name=f"st{tagp}")
        mu8 = st[:, 0:NT]
        s28 = st[:, NT:2 * NT]
        for tt in range(NT):
            scr = wk.tile([128, D], F32,
                          tag="lnscr" if tt % 2 == 0 else "lnscr2",
                          name="lnscr")
            nc.vector.tensor_reduce(st[:, tt:tt + 1], zset[tt][:],
                                    mybir.AxisListType.X, op=ALU.add)
            nc.scalar.activation(scr[:], zset[tt][:], AF.Square,
                                 accum_out=st[:, NT + tt:NT + tt + 1])
        mun = st[:, 2 * NT:3 * NT]
        nc.vector.tensor_scalar_mul(mun, mu8, 1.0 / D)
        ex2 = st[:, 3 * NT:4 * NT]
        nc.vector.tensor_scalar_mul(ex2, s28, 1.0 / D)
        musq = st[:, 4 * NT:5 * NT]
        nc.scalar.activation(musq, mun, AF.Square)
        var = st[:, 5 * NT:6 * NT]
        nc.vector.tensor_sub(var, ex2, musq)
        sd = st[:, 6 * NT:7 * NT]
        nc.scalar.activation(sd, var, AF.Sqrt, bias=self.epst[:, 0:1])
        rs = st[:, 7 * NT:8 * NT]
        nc.vector.reciprocal(rs, sd)
        nmurs = st[:, 4 * NT:5 * NT]  # overwrite musq slot
        nc.vector.tensor_mul(nmurs, mun, rs)
        nc.vector.tensor_scalar_mul(nmurs, nmurs, -1.0)
        return rs, nmurs

    # ---------- FF block ----------
    def _ff(self, s, z3, wk, tail, h_=None):
        if PREC["ff"] == "f32r":
            return self._ff_f32r(s, z3, wk, tail, h_)
        return self._ff_hl(s, z3, wk, tail)

    def _ff_f32r(self, s, z3, wk, tail, h_):
        nc = self.nc
        cpk = self.cpk
        # h_ (pre-LN normalized tiles) were produced per-tt inside the wout
        # loop by _sample's prep closure
        hT = [wk.tile([128, N], F32R, tag=f"A2_{dt}", name=f"hT{dt}")
              for dt in range(ND)]
        znT = [wk.tile([128, N], F32R, tag=f"A1_{dt}", name=f"znT{dt}")
               for dt in range(ND)]
        for h in range(2):
            for dt in range(ND):
                pT = self.bank(dt, dtype=F32R)
                for q in range(4):
                    nc.tensor.transpose(pT[:, _sl(q)], h_[h * 4 + q][:, _sl(dt)],
                                        self.idnr[:])
                if h == 0:
                    nc.scalar.copy(hT[dt][:, _hh(h)], pT[:])
                else:
                    nc.vector.tensor_copy(hT[dt][:, _hh(h)], pT[:])
                # znT per (h, dt) immediately: the first w1 matmul only
                # needs the four h0 halves
                nc.vector.tensor_scalar(znT[dt][:, _hh(h)], hT[dt][:, _hh(h)],
                                        cpk[:, dt:dt + 1],
                                        cpk[:, 4 + dt:5 + dt],
                                        op0=ALU.mult, op1=ALU.add)

        yT = [wk.tile([128, N], F32R, tag=f"A2_{dt}", name=f"yT{dt}")
              for dt in range(ND)]
        for h in range(2):
            pzf = [self.bank(b) for b in (0, 1, 6, 7)]
            # software-pipelined: w2(m-1) is emitted AFTER w1(m), so the PE
            # never sits head-of-line waiting on sig(m-1)'s ACT latency
            sigs = [None, None]
            w2ms = [None, None]

            def w2_stage(m):
                for dt in range(ND):
                    nc.tensor.matmul(pzf[dt][:], w2ms[m % 2][:, _sl(dt)],
                                     sigs[m % 2][:],
                                     start=(m == 0), stop=(m == NM - 1))

            for m in range(NM):
                w1m = wk.tile([128, 512], F32R, tag=f"w1mh{m % 2}", name="w1m")
                nc.sync.dma_start(w1m[:], self.d_ffw1r[m, :, :])
                ph = self.bank(4 + m % 2)
                for kt in range(ND):
                    nc.tensor.matmul(ph[:], w1m[:, _sl(kt)],
                                     znT[kt][:, _hh(h)],
                                     start=(kt == 0), stop=(kt == ND - 1))
                if m > 0:
                    w2_stage(m - 1)
                sig = wk.tile([128, 512], F32R, tag=f"sig{m % 2}", name="sig")
                nc.scalar.activation(sig[:], ph[:], AF.Sigmoid,
                                     bias=cpk[:, 8 + m:9 + m])
                sigs[m % 2] = sig
                w2m = wk.tile([128, 512], F32R, tag=f"w2m{m % 2}", name="w2m")
                nc.sync.dma_start(w2m[:], self.d_ffw2r[_sl(m), :])
                w2ms[m % 2] = w2m
            w2_stage(NM - 1)
            for dt in range(ND):
                nc.vector.scalar_tensor_tensor(yT[dt][:, _hh(h)], pzf[dt][:],
                                               cpk[:, 24 + dt:25 + dt],
                                               znT[dt][:, _hh(h)],
                                               op0=ALU.add, op1=ALU.add)
            if h == 0:
                tail()
        return self._post_ln(s, yT, wk, yr=True)

    def _ff_hl(self, s, z3, wk, tail):
        nc = self.nc
        idn = self.idn
        cpk = self.cpk
        tail()
        rs, nmurs = self._ln_stats(z3, wk, "pre")
        h_ = [wk.tile([128, D], F32, tag=f"B2_{tt}", name=f"h{tt}")
              for tt in range(NT)]
        for tt in range(NT):
            nc.scalar.activation(h_[tt][:], z3[tt][:], AF.Identity,
                                 scale=rs[:, tt:tt + 1], bias=nmurs[:, tt:tt + 1])
        hT = [wk.tile([128, N], F32, tag=f"A2_{dt}", name=f"hT{dt}")
              for dt in range(ND)]
        for h in range(2):
            for dt in range(ND):
                pT = self.bank(dt)
                for q in range(4):
                    nc.tensor.transpose(pT[:, _sl(q)], h_[h * 4 + q][:, _sl(dt)],
                                        idn[:])
                if h == 0:
                    nc.scalar.copy(hT[dt][:, _hh(h)], pT[:])
                else:
                    nc.vector.tensor_copy(hT[dt][:, _hh(h)], pT[:])
        znT = [wk.tile([128, N], F32, tag=f"A1_{dt}", name=f"znT{dt}")
               for dt in range(ND)]
        for h in range(2):
            for dt in range(ND):
                nc.vector.tensor_scalar(znT[dt][:, _hh(h)], hT[dt][:, _hh(h)],
                                        cpk[:, dt:dt + 1],
                                        cpk[:, 4 + dt:5 + dt],
                                        op0=ALU.mult, op1=ALU.add)

        yT = [wk.tile([128, N], F32, tag=f"A2_{dt}", name=f"yT{dt}")
              for dt in range(ND)]
        for h in range(2):
            znb = [wk.tile([128, 1024], BF16, tag=f"B3_{kt}", name=f"znb{kt}")
                   for kt in range(ND)]
            for kt in range(ND):
                nc.vector.tensor_copy(znb[kt][:, 0:512], znT[kt][:, _hh(h)])
                nc.vector.tensor_sub(znb[kt][:, 512:1024], znT[kt][:, _hh(h)],
                                     znb[kt][:, 0:512])
            pzf = [self.bank(dt) for dt in range(ND)]
            for m in range(NM):
                w1m = wk.tile([128, 2 * ND * 128], BF16,
                              tag=f"w1mh{m % 2}", name="w1m")
                nc.sync.dma_start(w1m[:], self.d_ffw1t[m, :, :])
                ph = self.bank(4 + m % 2)
                for kt in range(ND):
                    nc.tensor.matmul(ph[:], w1m[:, _sl(kt)], znb[kt][:, 0:512],
                                     start=(kt == 0), stop=False)
                    nc.tensor.matmul(ph[:], w1m[:, _sl(kt)], znb[kt][:, 512:1024],
                                     start=False, stop=False)
                    nc.tensor.matmul(ph[:], w1m[:, 512 + 128 * kt:640 + 128 * kt],
                                     znb[kt][:, 0:512],
                                     start=False, stop=(kt == ND - 1))
                sig = wk.tile([128, 512], F32, tag=f"sig{m % 2}", name="sig")
                nc.scalar.activation(sig[:], ph[:], AF.Sigmoid,
                                     bias=cpk[:, 8 + m:9 + m])
                sighl = wk.tile([128, 1024], BF16,
                                tag="amp2" if m % 2 == 0 else "lnscr",
                                name="sighl")
                nc.vector.tensor_copy(sighl[:, 0:512], sig[:])
                nc.vector.tensor_sub(sighl[:, 512:1024], sig[:],
                                     sighl[:, 0:512])
                w2m = wk.tile([128, 1024], BF16, tag=f"w2m{m % 2}", name="w2m")
                nc.sync.dma_start(w2m[:], self.d_ffw2hl[_sl(m), :])
                for dt in range(ND):
                    nc.tensor.matmul(pzf[dt][:], w2m[:, _sl(dt)],
                                     sighl[:, 0:512],
                                     start=(m == 0), stop=False)
                    nc.tensor.matmul(pzf[dt][:], w2m[:, _sl(dt)],
                                     sighl[:, 512:1024],
                                     start=False, stop=False)
                    nc.tensor.matmul(pzf[dt][:], w2m[:, 512 + dt * 128:
                                                     640 + dt * 128],
                                     sighl[:, 0:512],
                                     start=False, stop=(m == NM - 1))
            for dt in range(ND):
                nc.vector.scalar_tensor_tensor(yT[dt][:, _hh(h)], pzf[dt][:],
                                               cpk[:, 24 + dt:25 + dt],
                                               znT[dt][:, _hh(h)],
                                               op0=ALU.add, op1=ALU.add)
        return self._post_ln(s, yT, wk, yr=False)

    def _post_ln(self, s, yT, wk, yr):
        # fully per-tt post-LN chains: z4[0] is ready before the last yT
        # transposes finish, so l1's rfft starts with no barrier on the
        # batched stats
        nc = self.nc
        idn = self.idn
        rfr = PREC["rfft1"] == "f32r"
        gb, bb = self.gbt, self.bbt
        z4 = [wk.tile([128, D], F32R, tag=f"B3_{tt}", name=f"z4_{tt}")
              for tt in range(NT)]
        if not rfr:
            zhl = [wk.tile([128, 1024], BF16, tag=f"B2_{tt}", name=f"zhl{tt}")
                   for tt in range(NT)]
        st = wk.tile([128, 8 * NT], F32, tag="stpost", name="stpost")
        for tt in range(NT):
            pT = self.bank(6 + tt % 2, dtype=F32R if yr else F32)
            for dt in range(ND):
                nc.tensor.transpose(pT[:, _sl(dt)], yT[dt][:, _sl(tt)],
                                    self.idnr[:] if yr else idn[:])
            y_t = wk.tile([128, D], F32, tag=f"B4_{tt}", name=f"y{tt}")
            nc.scalar.copy(y_t[:], pT[:])
            scr = wk.tile([128, D], F32,
                          tag="lnscr" if tt % 2 == 0 else "lnscr2",
                          name="lnscr")
            mu = st[:, tt:tt + 1]
            s2 = st[:, NT + tt:NT + tt + 1]
            nc.vector.tensor_reduce(mu, y_t[:], mybir.AxisListType.X,
                                    op=ALU.add)
            nc.scalar.activation(scr[:], y_t[:], AF.Square, accum_out=s2)
            mun = st[:, 2 * NT + tt:2 * NT + tt + 1]
            nc.vector.tensor_scalar_mul(mun, mu, 1.0 / D)
            musq = st[:, 3 * NT + tt:3 * NT + tt + 1]
            nc.scalar.activation(musq, mun, AF.Square)
            var = st[:, 4 * NT + tt:4 * NT + tt + 1]
            nc.vector.scalar_tensor_tensor(var, s2, 1.0 / D, musq,
                                           op0=ALU.mult, op1=ALU.subtract)
            sd = st[:, 5 * NT + tt:5 * NT + tt + 1]
            nc.scalar.activation(sd, var, AF.Sqrt, bias=self.epst[:, 0:1])
            rs = st[:, 6 * NT + tt:6 * NT + tt + 1]
            nc.vector.reciprocal(rs, sd)
            nmurs = st[:, 7 * NT + tt:7 * NT + tt + 1]
            nc.vector.scalar_tensor_tensor(nmurs, mun, -1.0, rs,
                                           op0=ALU.mult, op1=ALU.mult)
            nc.scalar.activation(scr[:], y_t[:], AF.Identity,
                                 scale=rs, bias=nmurs)
            nc.vector.tensor_mul(z4[tt][:], scr[:], gb[:])
            nc.vector.tensor_add(z4[tt][:], z4[tt][:], bb[:])
            if not rfr:
                nc.gpsimd.tensor_copy(zhl[tt][:, 0:512], z4[tt][:])
                nc.gpsimd.tensor_sub(zhl[tt][:, 512:1024], z4[tt][:],
                                     zhl[tt][:, 0:512])
        if rfr:
            return z4, None
        return z4, zhl

    # ---------- output head ----------
    def _output(self, s, wk):
        nc = self.nc
        ones = self.ones
        po = self.bank(7)
        for kt in range(ND):
            nc.tensor.matmul(po[0:TF, 0:HOR], self.outwt[:, kt * TF:(kt + 1) * TF],
                             self.aggt[:, kt * HOR:(kt + 1) * HOR],
                             start=(kt == 0), stop=False)
        nc.tensor.matmul(po[0:TF, 0:HOR], self.outbt[0:1, 0:TF],
                         ones[0:1, 0:HOR], start=False, stop=True)
        xfin = self._xnew_last
        oT = wk.tile([TF, HOR], F32, tag="lvv", name="oT")
        nc.vector.tensor_scalar_add(oT[:], po[0:TF, 0:HOR], xfin[:, N - 1:N])
        nc.gpsimd.dma_start(self.d_out[s * TF:(s + 1) * TF, :], oT[:])


def _get_nc():
    if "nc" not in _CACHE:
        _CACHE["nc"] = K().build()
    return _CACHE["nc"]


def _common_maps(inputs, w2d, dft, ib, e8):
    m = dict(
        w2d=_rne11(w2d) if PREC["l0head"] == "f32r" else w2d,
        ones1=np.ones((1, N), np.float32),
        idn=np.eye(128, dtype=np.float32),
        e8=e8,
        ibr=_rne11(ib),
        winr=_rne11(np.asarray(inputs["mhesa_win"], np.float32)),
        woutr=_rne11(np.asarray(inputs["mhesa_wout"], np.float32)),
        boutr=np.asarray(inputs["mhesa_bout"], np.float32).reshape(L, 1, D),
        lcolp=_pack_lcol(inputs),
        alpha8=np.asarray(inputs["mhesa_alpha"], np.float32).reshape(L, HEADS, 1),
        cpkp=_pack_cpk(inputs),
        gpostr=np.asarray(inputs["ff_post_g"], np.float32).reshape(1, D),
        bpostr=np.asarray(inputs["ff_post_b"], np.float32).reshape(1, D),
        lvwg=np.asarray(inputs["level_wg"], np.float32),
        lvwp=np.asarray(inputs["level_wp"], np.float32),
        lvbg=np.asarray(inputs["level_bg"], np.float32).reshape(L, 1, TF),
        lvbp=np.asarray(inputs["level_bp"], np.float32).reshape(L, 1, TF),
        lvalpha=np.asarray(inputs["level_alpha"], np.float32).reshape(L, 1, 1),
        damp8=np.asarray(inputs["dampen_factor"], np.float32).reshape(HEADS, 1),
        outw=np.asarray(inputs["out_w"], np.float32)
            .reshape(ND, 128, TF).transpose(1, 0, 2).reshape(128, ND * TF)
            .copy(),
        outbr=np.asarray(inputs["out_b"], np.float32).reshape(1, TF),
    )
    if PREC["l0head"] == "f32":
        m["dft"] = dft
    if PREC["l0head"] == "f32r" or PREC["rfft1"] == "f32r":
        m["dftr"] = _rne11(dft)
    if PREC["rfft1"] == "hl":
        m["dfthl"] = np.concatenate([_split_hi(dft), _split_lo(dft)], axis=1)
    if PREC["irfft0"] == "hl":
        m["ibhl"] = np.concatenate([_split_hi(ib), _split_lo(ib)], axis=1)
    if PREC["mhesa0"] == "hl":
        win0 = np.asarray(inputs["mhesa_win"][0], np.float32)
        wout0 = np.asarray(inputs["mhesa_wout"][0], np.float32)
        m["winhl"] = np.concatenate([_split_hi(win0), _split_lo(win0)], axis=1)
        m["wouthl"] = np.concatenate([_split_hi(wout0), _split_lo(wout0)],
                                     axis=1)
    w1 = np.asarray(inputs["ff_w1"], np.float32)
    w2 = np.asarray(inputs["ff_w2"], np.float32)
    if PREC["ff"] == "f32r":
        m["ffw1r"] = _rne11(_pack_w1r(w1))
        m["ffw2r"] = _rne11(w2)
    else:
        m["ffw1t"] = _pack_w1(w1)
        m["ffw2hl"] = np.concatenate([_split_hi(w2), _split_lo(w2)], axis=1)
    return m


def _kernel_impl(inputs, runner):
    x = np.asarray(inputs["x"], np.float32)
    assert (x.shape[0], x.shape[1], x.shape[2]) == (32, N, TF)
    assert int(inputs["forecast_horizon"]) == HOR
    dft, ib = _dft_consts()
    conv_w = np.asarray(inputs["conv_w"], np.float32)
    w2d = _build_w2d(conv_w, np.asarray(inputs["conv_b"], np.float32))
    e8 = np.repeat(np.eye(HEADS, dtype=np.float32), DH, axis=1)
    nc = _get_nc()
    common = _common_maps(inputs, w2d, dft, ib, e8)
    in_maps = []
    for c in range(NCORES):
        xs = x[c * S:(c + 1) * S]
        xT = xs.transpose(0, 2, 1).reshape(S * TF, N).copy()
        in_maps.append(dict(common, xT=xT))
    res = runner(nc, in_maps)
    out = np.zeros((x.shape[0], HOR, TF), np.float32)
    for c in range(NCORES):
        oT = res.results[c]["outT"].reshape(S, TF, HOR)
        out[c * S:(c + 1) * S] = oT.transpose(0, 2, 1)
    return out, res


def kernel(**inputs):
    out, _ = _kernel_impl(
        inputs,
        lambda nc, im: run_bass_kernel_spmd(nc, im, list(range(NCORES))))
    return out


def kernel_traced(**inputs):
    """Like kernel() but with NTFF profiling; returns (out, BassKernelResults)."""
    return _kernel_impl(
        inputs,
        lambda nc, im: run_bass_kernel_spmd(nc, im, list(range(NCORES)),
                                            trace=True))
